# revision 102
# baseline (speedup 1.0000x reference)
"""Trainium2 Bass kernel for nn_BottleneckFusion (STCN memory readout + ResBlock
+ CBAM + PSP + bottleneck), 8-core SPMD.

Sharding: core c -> (batch b = c//2, half h = c%2).
  Phase A (attention): TM split across the pair (4 memory frames each);
    software-pipelined affinity/exp/value loop; flash-style combine of
    (unnormalized value, sumexp) via a pairwise AllGather TRIMMED to the
    21-image-row window the partner actually needs (dynamic partition-id
    driven send/receive offsets).
  Phase B (convs/CBAM/PSP): row-half split with halo recompute. The
    batch-query half of the ResBlock conv accumulation is emitted before the
    value AllGather so it fills the collective window; PE-warming spin
    matmuls keep the tensor clock ramped through the other collectives.
    CBAM sigmoids use 1/(1+exp(-z)) so only the exp act table is ever
    loaded; the spatial-gate channel max uses a gpsimd cross-partition
    reduce (no transposes); PSP 1x1 convs are emitted directly in
    transposed (block-major) layout with the own-half computed from local
    pools during the pools AllGather and dynamic Wup slot slices.

kernel(**inputs) takes the FULL unsharded inputs and returns the FULL output.
"""
import sys

sys.path.insert(0, "/opt/trn_rl_repo")

import numpy as np
import ml_dtypes

import concourse.bass as bass
import concourse.bacc as bacc
import concourse.bass_isa as bass_isa
import concourse.mybir as mybir
import concourse.tile as tile
from concourse.bass_utils import run_bass_kernel_spmd

BF16 = ml_dtypes.bfloat16
F16 = np.float16
bf = mybir.dt.bfloat16
f16 = mybir.dt.float16
f32 = mybir.dt.float32
AF = mybir.ActivationFunctionType
ALU = mybir.AluOpType
AX = mybir.AxisListType

N_CORES = 8
B, TM, CIN, CK, CV, COUT, H, W = 4, 8, 256, 64, 256, 256, 32, 32
EPS = 1e-5

# local row coordinates: l = image_row - (r0 - 5), l in 0..25
XROWS = 26                 # x window rows (image r0-5 .. r0+20)
CROWS = 22                 # xb/xc/comp local rows (image r0-3 .. r0+18)
PIXPAD = 768               # padded xc free size (22*34=748 -> 768)
PAIRS = [[0, 1], [2, 3], [4, 5], [6, 7]]
UPS = (2, 4, 8)            # upsampled PSP scales
# full pool pyramid offsets [s1, s2, s4, s8] and own-partial offsets
FOFF = {1: 0, 2: 1, 4: 5, 8: 21}
POFF = {1: 0, 2: 1, 4: 3, 8: 11}


def interp_matrix(s_in, s_out=32):
    if s_in == 1:
        return np.ones((s_out, 1), np.float32)
    c = np.arange(s_out) * (s_in - 1) / (s_out - 1)
    lo = np.floor(c).astype(np.int64)
    hi = np.minimum(lo + 1, s_in - 1)
    w = (c - lo).astype(np.float32)
    M = np.zeros((s_out, s_in), np.float32)
    M[np.arange(s_out), lo] += 1.0 - w
    M[np.arange(s_out), hi] += w
    return M


# ---------------------------------------------------------------------------
# Host-side input preparation
# ---------------------------------------------------------------------------

def _pad_hw(a):
    out = np.zeros(a.shape[:-2] + (34, 34), a.dtype)
    out[..., 1:33, 1:33] = a
    return out


def _chw_chunks(a):
    """[256, ...] -> [128, 2, ...] (partition, chunk)."""
    return a.reshape(2, 128, *a.shape[1:]).transpose(
        1, 0, *range(2, a.ndim + 1))


def prep_core_inputs(inputs, core):
    b, h = core // 2, core % 2
    r0 = 16 * h
    g = {}

    f16_q = np.asarray(inputs["f16_q"], np.float32)
    f16_m = np.asarray(inputs["f16_m"], np.float32)
    value_m = np.asarray(inputs["value_m"], np.float32)

    # xm: [128, 2, 4, 34, 34] padded memory frames
    src = f16_m[b, 4 * h: 4 * h + 4]                        # [4, 256, 32, 32]
    src = src.reshape(4, 2, 128, 32, 32).transpose(2, 1, 0, 3, 4)
    g["xm"] = _pad_hw(src).astype(F16)

    # xq: [128, 2, 34, 34] padded query
    q = _chw_chunks(f16_q[b, 0])                            # [128, 2, 32, 32]
    g["xq"] = _pad_hw(q).astype(F16)

    # vT: [128, 32, 256] transposed value
    V = value_m[b][:, 4 * h: 4 * h + 4].reshape(CV, 4096)
    g["vT"] = np.ascontiguousarray(
        V.T.reshape(32, 128, CV).transpose(1, 0, 2)).astype(BF16)  # stays bf16 (matches e)

    # x window q-part: [128, 2, 26, 34]
    qw = np.zeros((128, 2, XROWS, 34), np.float32)
    for l in range(XROWS):
        img = r0 - 5 + l
        if 0 <= img <= 31:
            qw[:, :, l, 1:33] = q[:, :, img, :]
    g["xqb_raw"] = qw.astype(F16)
    g["xqb_relu"] = np.maximum(qw, 0.0).astype(F16)

    pk_w = np.asarray(inputs["pk_w"], np.float32)
    g["pk_wT"] = np.ascontiguousarray(
        pk_w.reshape(CK, 2, 128, 3, 3).transpose(2, 1, 3, 4, 0)).astype(F16)
    pk_b = np.asarray(inputs["pk_b"], np.float32)
    g["pkb2"] = np.concatenate([pk_b, pk_b]).reshape(128, 1).astype(np.float32)

    def conv_lhsT(w, kc):
        co = w.shape[0]
        return np.ascontiguousarray(
            w.reshape(co, kc, 128, 3, 3).transpose(2, 1, 3, 4, 0)).astype(F16)

    g["rb1_wT"] = conv_lhsT(np.asarray(inputs["rb1_w"], np.float32), 4)
    g["rb2_wT"] = conv_lhsT(np.asarray(inputs["rb2_w"], np.float32), 2)
    g["rbd_wT"] = conv_lhsT(np.asarray(inputs["rbd_w"], np.float32), 4)
    g["rb1_b"] = np.asarray(inputs["rb1_b"], np.float32).reshape(2, 128).T.copy()
    g["xb_bias"] = (np.asarray(inputs["rb2_b"], np.float32)
                    + np.asarray(inputs["rbd_b"], np.float32)
                    ).reshape(2, 128).T.copy()

    w1 = np.asarray(inputs["mlp_w1"], np.float32)           # [16, 256]
    g["mlp_w1T"] = np.ascontiguousarray(
        w1.reshape(16, 2, 128).transpose(2, 1, 0)).copy()   # [128, 2, 16]
    g["mlp_b1"] = np.asarray(inputs["mlp_b1"], np.float32).reshape(16, 1).copy()
    g["mlp_w2T"] = np.ascontiguousarray(
        np.asarray(inputs["mlp_w2"], np.float32).T).astype(F16)  # [16, 256]
    # sigmoid is computed as 1/(1+exp(-z)) on the exp table, so biases are
    # stored pre-negated for the exp activation
    g["mlp_b2x2n"] = (-2.0 * np.asarray(inputs["mlp_b2"], np.float32)
                      ).reshape(2, 128).T.copy()
    g["mlp_b2row"] = (2.0 * np.asarray(inputs["mlp_b2"], np.float32)
                      ).reshape(1, 256).astype(F16)

    spw = np.asarray(inputs["sp_w"], np.float32)[0]       # [2, 7, 7]
    g["spw_r"] = np.ascontiguousarray(
        spw.reshape(14, 7)).astype(np.float16)                # [(ch,dy), dx]
    bn_scale = float(np.asarray(inputs["sp_g"], np.float32)[0]) / float(
        np.sqrt(1.0 + EPS))
    bn_bias = float(np.asarray(inputs["sp_b"], np.float32)[0])
    g["bn_nsb"] = np.array([[-bn_scale, -bn_bias]], np.float32)

    maskT = np.zeros((128, 6, 1), np.float16)
    mask_mean = np.zeros((1, 768), np.float16)
    for pix in range(CROWS * 34):
        img = r0 - 3 + pix // 34
        if 0 <= img <= 31:
            maskT[pix % 128, pix // 128, 0] = 1.0
            mask_mean[0, pix] = 1.0
    g["comp_maskT"] = maskT
    g["mask_mean"] = mask_mean

    pw = np.zeros((128, 2, 4, 64), np.float32)
    for si, s in enumerate((1, 2, 4, 8)):
        wc = np.asarray(inputs[f"psp_w{s}"], np.float32)[:, :, 0, 0]
        scale = 1.0 / ((32 // s) ** 2)
        pw[:, :, si, :] = (wc.T * scale).reshape(2, 128, 64).transpose(1, 0, 2)
    g["psp_wT"] = pw.astype(F16)

    # folded upsample operators, split per AG slot so every matmul operand
    # sits at partition base 0: Wup[k_local, sl, si, (r*32+c)] with the
    # global block k = jr*s+jc split as sl = k // (s*s/2), k_local = k % ..
    Wup = np.zeros((32, 2, 3, 512), np.float32)
    for si, s in enumerate(UPS):
        M = interp_matrix(s)
        Mrr = M[r0: r0 + 16, :]                 # [16, s]
        half = s * s // 2
        for jr in range(s):
            for jc in range(s):
                k = jr * s + jc
                Wup[k % half, k // half, si, :] = np.outer(
                    Mrr[:, jr], M[:, jc]).reshape(512)
    g["Wup"] = Wup.astype(F16)

    bott_w = np.asarray(inputs["bott_w"], np.float32)[:, :, 0, 0]
    g["bott_wT"] = np.ascontiguousarray(
        bott_w.reshape(COUT, 4, 128).transpose(2, 1, 0)).astype(F16)
    g["bott_b"] = np.asarray(inputs["bott_b"], np.float32).reshape(2, 128).T.copy()

    rmask = np.zeros((1, XROWS, 34), np.float16)
    for l in range(XROWS):
        if 0 <= r0 - 5 + l <= 31:
            rmask[0, l, :] = 1.0
    g["rmask"] = rmask

    g["ident"] = np.eye(128, dtype=F16)
    return g


INPUT_SPECS = [
    ("xm", [128, 2, 4, 34, 34], f16),
    ("xq", [128, 2, 34, 34], f16),
    ("vT", [128, 32, 256], bf),
    ("xqb_raw", [128, 2, XROWS, 34], f16),
    ("xqb_relu", [128, 2, XROWS, 34], f16),
    ("pk_wT", [128, 2, 3, 3, 64], f16),
    ("pkb2", [128, 1], f32),
    ("rb1_wT", [128, 4, 3, 3, 256], f16),
    ("rb2_wT", [128, 2, 3, 3, 256], f16),
    ("rbd_wT", [128, 4, 3, 3, 256], f16),
    ("rb1_b", [128, 2], f32),
    ("xb_bias", [128, 2], f32),
    ("mlp_w1T", [128, 2, 16], f32),
    ("mlp_b1", [16, 1], f32),
    ("mlp_w2T", [16, 256], f16),
    ("mlp_b2x2n", [128, 2], f32),
    ("mlp_b2row", [1, 256], f16),
    ("spw_r", [14, 7], f16),
    ("bn_nsb", [1, 2], f32),
    ("comp_maskT", [128, 6, 1], f16),
    ("mask_mean", [1, 768], f16),
    ("psp_wT", [128, 2, 4, 64], f16),
    ("Wup", [32, 2, 3, 512], f16),
    ("bott_wT", [128, 4, 256], f16),
    ("bott_b", [128, 2], f32),
    ("ident", [128, 128], f16),
    ("rmask", [1, XROWS, 34], f16),
]


# ---------------------------------------------------------------------------
# Device kernel
# ---------------------------------------------------------------------------

def build(stage="full"):
    nc = bacc.Bacc("TRN2", target_bir_lowering=False, debug=False,
                   num_devices=N_CORES)
    prm = {n: nc.declare_dram_parameter(n, sh, dt, isOutput=False)
           for n, sh, dt in INPUT_SPECS}
    if stage == "A":
        out_prm = nc.declare_dram_parameter("out_a", [257, 1024], f32,
                                            isOutput=True)
    else:
        out_prm = nc.declare_dram_parameter("out", [128, 2, 16, 32], f32,
                                            isOutput=True)
    if stage == "dbg":
        for n, sh, dt in [("dbg_xraw", [128, 4, XROWS, 34], f16),
                          ("dbg_xb", [128, 2, CROWS, 34], f16),
                          ("dbg_gate", [128, 2, 1], f32),
                          ("dbg_sig", [1, 512], f16),
                          ("dbg_fused", [128, 2, 16, 32], f16),
                          ("dbg_pd", [65, 64], f16),
                          ("dbg_il", [14, 16, 38], f16),
                          ("dbg_cmp", [1, 768], f16),
                          ("dbg_pri0", [128, 512], f16),
                          ("dbg_pri1", [128, 512], f16)]:
            prm[n] = nc.declare_dram_parameter(n, sh, dt, isOutput=True)
    with tile.TileContext(nc) as tc:
        _emit(tc, nc, prm, stage, out_prm)
    nc.compile()
    return nc


def _emit(tc, nc, prm, stage, out_prm):
    import contextlib
    es = contextlib.ExitStack()
    with es:
        wpool = es.enter_context(tc.tile_pool(name="wpool", bufs=1))
        apool = es.enter_context(tc.tile_pool(name="apool", bufs=1))
        dram = es.enter_context(tc.tile_pool(name="dram", bufs=1, space="DRAM"))
        aonly_cm = tc.tile_pool(name="aonly", bufs=1)
        aonly = aonly_cm.__enter__()

        def load(name, pool=wpool):
            t = pool.tile(list(prm[name].shape), prm[name].dtype,
                          name=f"{name}_sb")
            nc.sync.dma_start(t[:], prm[name][:])
            return t

        pk_wT = load("pk_wT")
        pkb2 = load("pkb2")
        xm_sb = aonly.tile([128, 2, 4, 34, 34], f16, name="xm_sb")
        # frames 0/1 rows 0:19 land first so the mk conv starts early
        for t in range(2):
            nc.sync.dma_start(xm_sb[:, :, t, 0:19, :],
                              prm["xm"][:, :, t, 0:19, :])
        for t in range(2):
            nc.sync.dma_start(xm_sb[:, :, t, 19:34, :],
                              prm["xm"][:, :, t, 19:34, :])
        for t in range(2, 4):
            nc.sync.dma_start(xm_sb[:, :, t, :, :], prm["xm"][:, :, t, :, :])
        xq_sb = load("xq", aonly)
        vT_sb = load("vT", aonly)

        ones_bf = wpool.tile([128, 1], bf)
        nc.vector.memset(ones_bf[:], 1.0)
        # spin sources: tiny constant operands for PE-warming matmuls that
        # keep the tensor clock ramped through collective windows
        spin_w = wpool.tile([1, 1], f16)
        nc.vector.memset(spin_w[:], 0.0)
        spin_src = wpool.tile([1, 512], f16)
        nc.vector.memset(spin_src[:], 0.0)
        zero128 = wpool.tile([1, 128], f16)
        nc.vector.memset(zero128[:], 0.0)

        def spin_pe(pool, n, rows=512):
            for _ in range(n):
                sp = pool.tile([1, rows], f32, tag="spin", name="sp")
                nc.tensor.matmul(sp[0:1, :], spin_w[0:1, 0:1],
                                 spin_src[0:1, 0:rows],
                                 start=True, stop=True,
                                 skip_group_check=True)

        # ================= phase A =================
        mk_sb = aonly.tile([128, 2, 1024], f16)
        qk_sb = aonly.tile([128, 1024], f16)

        with tc.tile_pool(name="psA", bufs=2, space="PSUM") as psA:
            for tp in range(2):
                for n in range(2):
                    pm = psA.tile([128, 512], f32, tag="mkps", name="pm")
                    for par in range(2):
                        t = 2 * tp + par
                        k = 0
                        for j in range(2):
                            for dy in range(3):
                                for dx in range(3):
                                    nc.tensor.matmul(
                                        pm[64 * par: 64 * par + 64, :],
                                        pk_wT[:, j, dy, dx, :],
                                        xm_sb[:, j, t,
                                              n * 16 + dy: n * 16 + dy + 16,
                                              dx: dx + 32],
                                        start=(k == 0), stop=(k == 17),
                                        tile_position=(0, 64 * par),
                                    )
                                    k += 1
                    nc.scalar.activation(
                        mk_sb[:, tp, n * 512: (n + 1) * 512], pm[:, :],
                        AF.Identity, bias=pkb2[:, 0:1])

            for n in range(2):
                pq = psA.tile([64, 512], f32, tag="qkps", name="pq")
                k = 0
                for j in range(2):
                    for dy in range(3):
                        for dx in range(3):
                            nc.tensor.matmul(
                                pq[:, :], pk_wT[:, j, dy, dx, :],
                                xq_sb[:, j, n * 16 + dy: n * 16 + dy + 16,
                                      dx: dx + 32],
                                start=(k == 0), stop=(k == 17))
                            k += 1
                nc.scalar.activation(
                    qk_sb[0:64, n * 512: (n + 1) * 512], pq[:, :],
                    AF.Identity, bias=pkb2[0:64, 0:1])
            # replicate qk to partitions 64..127 so odd-frame mk slices
            # (base partition 64) can stream against it
            nc.sync.dma_start(qk_sb[64:128, :], qk_sb[0:64, :])

        # pair exchange buffers: full [257,1024] for the debug stage, a
        # 21-image-row window (the part the partner actually needs) otherwise
        if stage == "A":
            arv = dram.tile([257, 1024], bf)
            arvg = dram.tile([2, 257, 1024], bf)
        else:
            arv2 = dram.tile([257, 672], bf)
            arvg2 = dram.tile([2, 257, 672], bf)
        pid = nc.partition_id()
        pid2 = pid % 2
        omh = (pid + 1) % 2
        sendoff = omh * 352
        myoff = pid2 * 352
        vstart160 = omh * 160

        with (
            tc.tile_pool(name="psAff", bufs=2, space="PSUM") as psAff,
            tc.tile_pool(name="psV", bufs=1, space="PSUM") as psV,
        ):
            vps = [psV.tile([128, 1024], f32, name=f"vps{j}") for j in range(2)]
            s_acc = aonly.tile([128, 1024], bf, name="s_acc")

            order = [16 * h + o + 8 * par for h in range(2) for o in range(8)
                     for par in range(2)]

            def lhs_aff(i):
                t = i >> 3
                pb = i & 7
                tp, par = t >> 1, t & 1
                return par, mk_sb[64 * par: 64 * par + 64, tp,
                                  pb * 128: pb * 128 + 128]

            # software-pipelined: affinity matmuls + exp run one chunk ahead
            # of the value accumulation so the PE never waits on the exp.
            e_tiles = {}

            def emit_aff(idx):
                i = order[idx]
                par, lhs = lhs_aff(i)
                e_t = aonly.tile([128, 1024], bf, tag="e", name="e_t", bufs=4)
                pa = psAff.tile([128, 1024], f32, tag="affp", name="pa")
                for qn in range(2):
                    nc.tensor.matmul(
                        pa[:, qn * 512: (qn + 1) * 512], lhs,
                        qk_sb[64 * par: 64 * par + 64,
                              qn * 512: (qn + 1) * 512],
                        start=True, stop=True)
                nc.scalar.activation(e_t[:, :], pa[:, :], AF.Exp, scale=0.125)
                e_tiles[idx] = e_t

            emit_aff(0)
            for idx in range(32):
                if idx + 1 < 32:
                    emit_aff(idx + 1)
                i = order[idx]
                e_t = e_tiles.pop(idx)
                for j in range(2):
                    for qn in range(2):
                        nc.tensor.matmul(
                            vps[j][:, qn * 512: (qn + 1) * 512],
                            vT_sb[:, i, j * 128: (j + 1) * 128],
                            e_t[:, qn * 512: (qn + 1) * 512],
                            start=(idx == 0), stop=(idx == 31),
                            skip_group_check=True)
                if idx == 0:
                    nc.vector.tensor_copy(s_acc[:, :], e_t[:, :])
                else:
                    nc.vector.tensor_add(s_acc[:, :], s_acc[:, :], e_t[:, :])

            v_sb = apool.tile([128, 2, 1024], bf, name="v_sb")
            s_sb = apool.tile([1, 1024], bf, name="s_sb")
            # one PSUM->SBUF copy on DVE, one on Act so they run concurrently
            nc.vector.tensor_copy(v_sb[:, 0, :], vps[0][:, :])
            nc.scalar.copy(v_sb[:, 1, :], vps[1][:, :])
            if stage == "A":
                for j in range(2):
                    nc.sync.dma_start(arv[128 * j: 128 * j + 128, :],
                                      v_sb[:, j, :])
            else:
                nc.sync.dma_start(
                    arv2[0:256, :].rearrange("(j p) w -> p j w", j=2),
                    v_sb[:, :, bass.ds(sendoff, 672)])
            # fold the 128-partition sumexp accumulator with a ones matmul
            for qn in range(2):
                sfold = psAff.tile([1, 512], f32, tag="affp", name="sfold")
                nc.tensor.matmul(sfold[0:1, :],
                                 ones_bf[:, 0:1],
                                 s_acc[:, qn * 512: (qn + 1) * 512],
                                 start=True, stop=True)
                nc.vector.tensor_copy(s_sb[:, qn * 512: (qn + 1) * 512],
                                      sfold[0:1, :])
            if stage == "A":
                nc.sync.dma_start(arv[256:257, :], s_sb[:, :])
            else:
                nc.sync.dma_start(arv2[256:257, :],
                                  s_sb[0:1, bass.ds(sendoff, 672)])

        if stage == "A":
            nc.gpsimd.collective_compute(
                "AllGather", ALU.bypass, replica_groups=PAIRS,
                ins=[arv[:].opt()], outs=[arvg[:].opt()])
        else:
            nc.gpsimd.collective_compute(
                "AllGather", ALU.bypass, replica_groups=PAIRS,
                ins=[arv2[:].opt()], outs=[arvg2[:].opt()])


        aonly_cm.__exit__(None, None, None)

        if stage == "A":
            with tc.tile_pool(name="cmb", bufs=1) as cmb:
                cs0 = cmb.tile([1, 1024], bf, name="cs0")
                cs1 = cmb.tile([1, 1024], bf, name="cs1")
                cso = cmb.tile([1, 1024], f32, name="cso")
                nc.sync.dma_start(cs0[:], arvg[0, 256:257, :])
                nc.sync.dma_start(cs1[:], arvg[1, 256:257, :])
                nc.vector.tensor_add(cso[:, :], cs0[:, :], cs1[:, :])
                nc.sync.dma_start(out_prm[256:257, :], cso[:, :])
                for j in range(2):
                    ca = cmb.tile([128, 1024], bf, tag="ca", name="ca")
                    cb = cmb.tile([128, 1024], bf, tag="cb", name="cb")
                    co = cmb.tile([128, 1024], f32, tag="co", name="co")
                    nc.sync.dma_start(ca[:, :], arvg[0, 128 * j: 128 * j + 128, :])
                    nc.sync.dma_start(cb[:, :], arvg[1, 128 * j: 128 * j + 128, :])
                    nc.vector.tensor_add(co[:, :], ca[:, :], cb[:, :])
                    nc.sync.dma_start(out_prm[128 * j: 128 * j + 128, :],
                                      co[:, :])
            return

        # ================= phase B =================
        wk = es.enter_context(tc.tile_pool(name="wk", bufs=1))
        rb1_wT = load("rb1_wT")
        rb2_wT = load("rb2_wT")
        rbd_wT = load("rbd_wT")
        rb1_b = load("rb1_b")
        xb_bias = load("xb_bias")
        mlp_w1T = load("mlp_w1T")
        mlp_b1 = load("mlp_b1")
        mlp_w2T = load("mlp_w2T")
        mlp_b2x2n = load("mlp_b2x2n")
        spw_r = load("spw_r")
        bn_nsb = load("bn_nsb")
        mask_mean = load("mask_mean")
        psp_wT = load("psp_wT")
        Wup = load("Wup")
        bott_wT = load("bott_wT")
        bott_b = load("bott_b")
        ident = load("ident")

        r0v = (nc.vector.partition_id() % 2) * 16

        # ---- val-independent prep: x tiles, query-side loads, masks ----
        x_raw = apool.tile([128, 4, XROWS, 34], f16)
        x_relu = apool.tile([128, 4, XROWS, 34], f16)
        for tt in (x_raw, x_relu):
            nc.vector.memset(tt[:, 2:4, :, 0:1], 0.0)
            nc.vector.memset(tt[:, 2:4, :, 33:34], 0.0)
        nc.sync.dma_start(x_raw[:, 0:2, :, :], prm["xqb_raw"][:])
        nc.sync.dma_start(x_relu[:, 0:2, :, :], prm["xqb_relu"][:])

        r1_relu = apool.tile([128, 2, XROWS, 34], f16)
        nc.vector.memset(r1_relu[:, :, 0:1, :], 0.0)
        nc.vector.memset(r1_relu[:, :, 25:26, :], 0.0)
        nc.vector.memset(r1_relu[:, :, :, 0:1], 0.0)
        nc.vector.memset(r1_relu[:, :, :, 33:34], 0.0)
        rmaskb = apool.tile([128, XROWS, 34], f16)
        nc.sync.dma_start(rmaskb[:], prm["rmask"][:].partition_broadcast(128))
        xb = apool.tile([128, 2, PIXPAD], f16)
        xbv = [xb[:, j, 0: CROWS * 34].rearrange("p (r c) -> p r c", c=34)
               for j in range(2)]
        for j in range(2):
            nc.vector.memset(xbv[j][:, :, 0:1], 0.0)
            nc.vector.memset(xbv[j][:, :, 33:34], 0.0)
        nc.vector.memset(xb[:, :, CROWS * 34:], 0.0)

        # val window tiles in x-window coordinates (26 rows = XROWS); the
        # 21-row valid band sits at dynamic row offset 5*(1-h). Pad rows are
        # zeroed statically (both possible pad bands); the valid-band writes
        # land after and overwrite any overlap.
        val_pad = apool.tile([128, 2, 832], f32)
        nc.vector.memset(val_pad[:, :, 0:160], 0.0)
        nc.vector.memset(val_pad[:, :, 672:832], 0.0)
        # x val-part pad bands (rows outside the 21-row valid window) are
        # zeroed statically; the valid band is written at a dynamic offset
        for tt in (x_raw, x_relu):
            nc.vector.memset(tt[:, 2:4, 0:5, 1:33], 0.0)
            nc.vector.memset(tt[:, 2:4, 21:26, 1:33], 0.0)

        # ---- query-side conv accumulation: fills the PE while the val
        # AllGather is in flight (j=0,1 of x are batch-query channels) ----
        psR1_cm = tc.tile_pool(name="psR1", bufs=1, space="PSUM")
        psR1 = psR1_cm.__enter__()
        psXB_cm = tc.tile_pool(name="psXB", bufs=1, space="PSUM")
        psXB = psXB_cm.__enter__()
        psW1_cm = tc.tile_pool(name="psW1", bufs=1, space="PSUM")
        psW1 = psW1_cm.__enter__()
        R1G = ((0, 1, 16), (0, 17, 8), (1, 1, 16), (1, 17, 8))
        XBG = ((0, 2, 16), (0, 18, 6), (1, 2, 16), (1, 18, 6))
        # the two short row-groups per producer share one PSUM bank
        # (independent column ranges) so all 8 accumulators fit in 6 banks
        pr_t = {}
        px_t = {}
        r1sm = psR1.tile([128, 512], f32, tag="r1s", name="r1sm")
        xbsm = psXB.tile([128, 384], f32, tag="xbs", name="xbsm")
        # a start=True matmul resets the whole PSUM bank, so shared banks are
        # zeroed once up front and every accumulation into them avoids start
        nc.tensor.matmul(r1sm[:, :], zero128[0:1, :], spin_src[0:1, 0:512],
                         start=True, stop=False, skip_group_check=True)
        nc.tensor.matmul(xbsm[:, :], zero128[0:1, :], spin_src[0:1, 0:384],
                         start=True, stop=False, skip_group_check=True)
        for m in range(2):
            pr_t[(m, 1)] = psR1.tile([128, 512], f32, tag=f"r1b{m}",
                                     name="prb")
            pr_t[(m, 17)] = r1sm[:, m * 256: m * 256 + 256]
            px_t[(m, 2)] = psXB.tile([128, 512], f32, tag=f"xbb{m}",
                                     name="pxb")
            px_t[(m, 18)] = xbsm[:, m * 192: m * 192 + 192]
        for (m, l0, nr) in R1G:
            pr = pr_t[(m, l0)]
            k = 0
            for j in range(2):
                for dy in range(3):
                    for dx in range(3):
                        nc.tensor.matmul(
                            pr[:, : nr * 32],
                            rb1_wT[:, j, dy, dx, m * 128: m * 128 + 128],
                            x_relu[:, j, l0 + dy - 1: l0 + dy - 1 + nr,
                                   dx: dx + 32],
                            start=(k == 0 and nr == 16), stop=False,
                            skip_group_check=True)
                        k += 1
        for (m, l0, nr) in XBG:
            px = px_t[(m, l0)]
            k = 0
            for j in range(2):
                for dy in range(3):
                    for dx in range(3):
                        nc.tensor.matmul(
                            px[:, : nr * 32],
                            rbd_wT[:, j, dy, dx, m * 128: m * 128 + 128],
                            x_raw[:, j, l0 + dy - 1: l0 + dy - 1 + nr,
                                  dx: dx + 32],
                            start=(k == 0 and nr == 16), stop=False,
                            skip_group_check=True)
                        k += 1
        # keep the PE clock ramped through the rest of the AllGather window
        spin_pe(psW1, 48)

        # ---- val: own window (SBUF) + partner window (AG slot), normalize,
        # window into x ----
        vs_p = wk.tile([128, 2, 672], bf, name="vs_p")
        nc.sync.dma_start(
            vs_p[:, :, :],
            arvg2[bass.ds(omh, 1), 0:256, :].rearrange(
                "s (j p) w -> s p j w", j=2))
        sp_row = wk.tile([1, 672], bf, name="sp_row")
        nc.gpsimd.dma_start(sp_row[:, :], arvg2[bass.ds(omh, 1), 256:257, :])
        for j in range(2):
            nc.vector.tensor_add(val_pad[:, j, bass.ds(vstart160, 672)],
                                 v_sb[:, j, bass.ds(myoff, 672)],
                                 vs_p[:, j, :])
        s_row = wk.tile([1, 672], f32, name="s_row")
        nc.vector.tensor_add(s_row[:, :], s_sb[0:1, bass.ds(myoff, 672)],
                             sp_row[:, :])
        inv_row = wk.tile([1, 672], f32, name="inv_row")
        nc.vector.reciprocal(inv_row[:, :], s_row[:, :])
        inv_d = dram.tile([1, 672], f32)
        nc.sync.dma_start(inv_d[:], inv_row[:, :])
        inv_b = wk.tile([128, 21, 32], f32, name="inv_b")
        nc.sync.dma_start(inv_b.rearrange("p r c -> p (r c)"),
                          inv_d.partition_broadcast(128))
        vp_v = val_pad.rearrange("p j (r c) -> p j r c", c=32)
        omh5 = omh * 5
        for j in range(2):
            nc.vector.tensor_mul(x_raw[:, 2 + j, bass.ds(omh5, 21), 1:33],
                                 vp_v[:, j, bass.ds(omh5, 21), :],
                                 inv_b[:, :, :])
            nc.scalar.activation(x_relu[:, 2 + j, :, 1:33],
                                 x_raw[:, 2 + j, :, 1:33], AF.Relu)

        # ---- val-side conv accumulation + activations ----
        for (m, l0, nr) in R1G:
            pr = pr_t[(m, l0)]
            k = 0
            for j in (2, 3):
                for dy in range(3):
                    for dx in range(3):
                        nc.tensor.matmul(
                            pr[:, : nr * 32],
                            rb1_wT[:, j, dy, dx, m * 128: m * 128 + 128],
                            x_relu[:, j, l0 + dy - 1: l0 + dy - 1 + nr,
                                   dx: dx + 32],
                            start=False, stop=(k == 17),
                            skip_group_check=True)
                        k += 1
            nc.scalar.activation(
                r1_relu[:, m, l0: l0 + nr, 1:33], pr[:, : nr * 32],
                AF.Relu, bias=rb1_b[:, m: m + 1])
            nc.vector.tensor_mul(r1_relu[:, m, l0: l0 + nr, 1:33],
                                 r1_relu[:, m, l0: l0 + nr, 1:33],
                                 rmaskb[:, l0: l0 + nr, 1:33])
        for (m, l0, nr) in XBG:
            px = px_t[(m, l0)]
            k = 0
            for j in (2, 3):
                for dy in range(3):
                    for dx in range(3):
                        nc.tensor.matmul(
                            px[:, : nr * 32],
                            rbd_wT[:, j, dy, dx, m * 128: m * 128 + 128],
                            x_raw[:, j, l0 + dy - 1: l0 + dy - 1 + nr,
                                  dx: dx + 32],
                            start=False, stop=False,
                            skip_group_check=True)
                        k += 1
            for j in range(2):
                for dy in range(3):
                    for dx in range(3):
                        nc.tensor.matmul(
                            px[:, : nr * 32],
                            rb2_wT[:, j, dy, dx, m * 128: m * 128 + 128],
                            r1_relu[:, j, l0 + dy - 1: l0 + dy - 1 + nr,
                                    dx: dx + 32],
                            start=False, stop=(k == 35),
                            skip_group_check=True)
                        k += 1
            nc.scalar.activation(
                xbv[m][:, l0 - 2: l0 - 2 + nr, 1:33], px[:, : nr * 32],
                AF.Identity, bias=xb_bias[:, m: m + 1])
        psW1_cm.__exit__(None, None, None)
        psXB_cm.__exit__(None, None, None)
        psR1_cm.__exit__(None, None, None)

        if stage == "dbg":
            nc.sync.dma_start(prm["dbg_xraw"][:], x_raw[:])
            for j in range(2):
                nc.sync.dma_start(prm["dbg_xb"][:, j], xbv[j])

        # ---- CBAM channel gate ----
        stats = wk.tile([128, 2, 2], f32, name="stats")
        for j in range(2):
            nc.vector.tensor_reduce(stats[:, j, 0:1], xbv[j][:, 3:19, 1:33],
                                    AX.XY, ALU.add)
            nc.vector.tensor_reduce(stats[:, j, 1:2], xbv[j][:, 3:19, 1:33],
                                    AX.XY, ALU.max)
        stats_d = dram.tile([256, 2], f32)
        stats_o = dram.tile([2, 256, 2], f32)
        nc.sync.dma_start(stats_d.rearrange("(j p) k -> p j k", j=2),
                          stats[:, :, :])
        # zero the comp scratch (incl. halo borders) well before it is used
        comp_d = dram.tile([2, CROWS, 38], f16)
        zz = wk.tile([2, CROWS * 38], f16, name="zz")
        nc.vector.memset(zz[:], 0.0)
        nc.gpsimd.dma_start(comp_d.rearrange("s r c -> s (r c)"), zz[:, :])
        nc.gpsimd.collective_compute(
            "AllGather", ALU.bypass, replica_groups=PAIRS,
            ins=[stats_d[:].opt()], outs=[stats_o[:].opt()])
        slb = wk.tile([128, 2, 2, 2], f32, name="slb")  # [p, slot, j, stat]
        nc.sync.dma_start(slb[:, :, :, :],
                          stats_o.rearrange("s (j p) k -> p s j k", j=2))
        gate_in = wk.tile([128, 2, 2], f32, name="gate_in")
        tsum = wk.tile([128, 2, 1], f32, name="tsum")
        nc.vector.tensor_add(tsum[:, :, :], slb[:, 0, :, 0:1],
                             slb[:, 1, :, 0:1])
        nc.scalar.mul(gate_in[:, :, 0:1], tsum[:, :, :], 1.0 / 1024.0)
        nc.vector.tensor_max(gate_in[:, :, 1:2], slb[:, 0, :, 1:2],
                             slb[:, 1, :, 1:2])

        gate = wk.tile([128, 2, 1], f32, name="gate")
        ones1 = wk.tile([1, 128], f16, name="ones1")
        nc.vector.memset(ones1[:], 1.0)
        with tc.tile_pool(name="psG", bufs=1, space="PSUM") as psG:
            ph1 = psG.tile([16, 2], f32, name="ph1")
            for j in range(2):
                nc.tensor.matmul(ph1[:, :], mlp_w1T[:, j, :], gate_in[:, j, :],
                                 start=(j == 0), stop=(j == 1))
            h1 = wk.tile([16, 2], f16, name="h1")
            nc.scalar.activation(h1[:, :], ph1[:, :], AF.Relu,
                                 bias=mlp_b1[:, 0:1])
            # per-partition gate (sigmoid via the already-loaded exp table)
            for j in range(2):
                ph2 = psG.tile([128, 2], f32, tag="ph2", name="ph2")
                nc.tensor.matmul(ph2[:, :], mlp_w2T[:, j * 128: j * 128 + 128],
                                 h1[:, :], start=True, stop=True)
                h2 = wk.tile([128, 2], f32, tag="h2", name="h2")
                nc.vector.tensor_copy(h2[:, :], ph2[:, :])
                t2 = wk.tile([128, 1], f32, tag="t2", name="t2")
                nc.vector.tensor_add(t2[:, :], h2[:, 0:1], h2[:, 1:2])
                ev = wk.tile([128, 1], f32, tag="ev", name="ev")
                nc.scalar.activation(ev[:, :], t2[:, :], AF.Exp, scale=-1.0,
                                     bias=mlp_b2x2n[:, j: j + 1])
                e1 = wk.tile([128, 1], f32, tag="e1", name="e1")
                nc.scalar.activation(e1[:, :], ev[:, :], AF.Identity,
                                     bias=1.0)
                nc.vector.reciprocal(gate[:, j, :], e1[:, :])

        if stage == "dbg":
            nc.sync.dma_start(prm["dbg_gate"][:], gate[:])

        gate_sc = wk.tile([128, 2, 1], f16, name="gate_sc")
        nc.scalar.mul(gate_sc[:, :, :], gate[:, :, :], 1.0 / 256.0)

        # channel max of xb*gate via a cross-partition gpsimd reduce -- the
        # result lands directly in pixel-major layout, skipping the PE
        # transposes and one DRAM staging hop
        xcj = wk.tile([128, 2, 768], f16, name="xcj")
        for j in range(2):
            nc.vector.tensor_scalar_mul(xcj[:, j, :], xb[:, j, :],
                                        gate[:, j, 0:1])
        cmx = wk.tile([128, 768], f32, name="cmx")
        cmx2 = wk.tile([128, 768], f32, name="cmx2")
        nc.gpsimd.partition_all_reduce(cmx[:, :], xcj[:, 0, :], 128,
                                       bass_isa.ReduceOp.max)
        nc.gpsimd.partition_all_reduce(cmx2[:, :], xcj[:, 1, :], 128,
                                       bass_isa.ReduceOp.max)
        comp_row = wk.tile([1, 768], f16, name="comp_row")
        nc.vector.tensor_max(comp_row[0:1, :], cmx[0:1, :], cmx2[0:1, :])
        nc.vector.tensor_mul(comp_row[0:1, :], comp_row[0:1, :],
                             mask_mean[:, 0:768])

        # channel mean of xb*gate via gate-weighted ones-matmul; the mean
        # half of comp then flows through its DRAM hops on the Pool queue
        # while the max half (slower DVE path) catches up on the SP queue.
        il = wk.tile([14, 16, 38], f16, name="il")
        mean_sb = wk.tile([1, 748], f16, name="mean_sb")
        psW2_cm = tc.tile_pool(name="psW2", bufs=1, space="PSUM")
        psW2 = psW2_cm.__enter__()
        with tc.tile_pool(name="psM", bufs=1, space="PSUM") as psM:
            pm1 = psM.tile([1, 748], f32, name="pm1")
            for j in range(2):
                for (o0, nn) in ((0, 512), (512, 236)):
                    nc.tensor.matmul(pm1[0:1, o0: o0 + nn],
                                     gate_sc[:, j, :],
                                     xb[:, j, o0: o0 + nn],
                                     start=(j == 0), stop=(j == 1))
            nc.scalar.copy(mean_sb[:, :], pm1[:, :])
        nc.vector.tensor_mul(mean_sb[:, :], mean_sb[:, :],
                             mask_mean[:, 0:748])
        # keep the PE clock ramped until the spatial-conv operands land
        spin_pe(psW2, 40)

        nc.gpsimd.dma_start(
            comp_d[1:2, :, 2:36],
            mean_sb[0:1, :].rearrange("o (r c) -> o r c", c=34))
        nc.gpsimd.dma_start(
            il[7:14, :, :],
            bass.AP(comp_d.tensor, 836, [[38, 7], [38, 16], [1, 38]]))
        nc.sync.dma_start(
            comp_d[0:1, :, 2:36],
            comp_row[0:1, 0:748].rearrange("o (r c) -> o r c", c=34))
        nc.sync.dma_start(
            il[0:7, :, :],
            bass.AP(comp_d.tensor, 0, [[38, 7], [38, 16], [1, 38]]))
        sig_row = wk.tile([1, 512], f16, name="sig_row")
        sigb = wk.tile([128, 16, 32], f16, name="sigb")
        with tc.tile_pool(name="psS", bufs=1, space="PSUM") as psS:
            pss = psS.tile([1, 512], f32, name="pss")
            for dx in range(7):
                nc.tensor.matmul(pss[:, :], spw_r[:, dx: dx + 1],
                                 il[:, :, dx: dx + 32],
                                 start=(dx == 0), stop=(dx == 6))
            # sigmoid via 1/(1+exp(-z)) on the exp table (z = bn affine)
            se = wk.tile([1, 512], f32, name="se")
            nc.scalar.activation(se[:, :], pss[:, :], AF.Exp,
                                 scale=bn_nsb[0:1, 0:1], bias=bn_nsb[0:1, 1:2])
            se1 = wk.tile([1, 512], f32, name="se1")
            nc.scalar.activation(se1[:, :], se[:, :], AF.Identity, bias=1.0)
            sgr = wk.tile([1, 512], f32, name="sgr")
            nc.vector.reciprocal(sgr[:, :], se1[:, :])
            nc.vector.tensor_copy(sig_row[:, :], sgr[:, :])
            # broadcast along partitions with a ones-matmul (no DRAM hop)
            sigb_ps = psS.tile([128, 512], f32, tag="sigbps", name="sigb_ps")
            nc.tensor.matmul(sigb_ps[:, :], ones1[0:1, :], sig_row[0:1, :],
                             start=True, stop=True)
            nc.scalar.copy(sigb.rearrange("p r c -> p (r c)"), sigb_ps[:, :])
        psW2_cm.__exit__(None, None, None)

        if stage == "dbg":
            nc.sync.dma_start(prm["dbg_sig"][:], sig_row[:])
            nc.sync.dma_start(prm["dbg_il"][:], il[:])
            nc.sync.dma_start(prm["dbg_cmp"][:], comp_row[:])

        # fused = xb_own + (xb_own * gate) * sigb
        fused = apool.tile([128, 2, 16, 32], f16)
        for j in range(2):
            xc_own = wk.tile([128, 16, 32], f16, tag="xc_own", name="xc_own")
            nc.scalar.mul(xc_own[:, :, :], xbv[j][:, 3:19, 1:33],
                          gate[:, j, 0:1])
            tm = wk.tile([128, 16, 32], f16, tag="tm", name="tm")
            nc.vector.tensor_mul(tm[:, :, :], xc_own[:, :, :], sigb[:, :, :])
            nc.vector.tensor_add(fused[:, j, :, :], xbv[j][:, 3:19, 1:33],
                                 tm[:, :, :])

        if stage == "dbg":
            nc.sync.dma_start(prm["dbg_fused"][:], fused[:])

        # bottleneck conv: accumulate the fused-input chunks now so the PE
        # works during the pools AllGather; priors chunks finish the group
        # after the collective.
        fbv = fused.rearrange("p j r c -> p j (r c)")
        psO_cm = tc.tile_pool(name="psO", bufs=2, space="PSUM")
        psO = psO_cm.__enter__()
        po_t = []
        for m in range(2):
            po = psO.tile([128, 512], f32, tag="po", name="po")
            po_t.append(po)
            for jj in range(2):
                nc.tensor.matmul(po[:, :],
                                 bott_wT[:, 2 + jj, m * 128: m * 128 + 128],
                                 fbv[:, jj, :],
                                 start=(jj == 0), stop=False,
                                 skip_group_check=True)

        # ---- PSP pools (raw block sums over own rows) ----
        pools = wk.tile([128, 2, 43], f32, name="pools")
        for j in range(2):
            f8 = fused[:, j].rearrange("p (rb ri) (cb ci) -> p rb cb ri ci",
                                       ri=4, ci=4)
            p8v = pools[:, j, 11:43].rearrange("p (rb cb) -> p rb cb", cb=8)
            nc.vector.tensor_reduce(p8v, f8, AX.XY, ALU.add)
            p8i = pools[:, j, 11:43].rearrange(
                "p (rb ri cb ci) -> p rb cb ri ci", rb=2, ri=2, cb=4, ci=2)
            p4v = pools[:, j, 3:11].rearrange("p (rb cb) -> p rb cb", cb=4)
            nc.vector.tensor_reduce(p4v, p8i, AX.XY, ALU.add)
        p4i = pools[:, :, 3:11].rearrange(
            "p j (rb cb ci) -> p j cb rb ci", rb=2, cb=2, ci=2)
        nc.vector.tensor_reduce(
            pools[:, :, 1:3].rearrange("p j (a k) -> p j a k", a=2, k=1),
            p4i, AX.XY, ALU.add)
        nc.vector.tensor_reduce(pools[:, :, 0:1], pools[:, :, 1:3], AX.X,
                                ALU.add)

        pools16 = wk.tile([128, 2, 43], f16, name="pools16")
        nc.vector.tensor_copy(pools16[:, :, :], pools[:, :, :])
        pools_d = dram.tile([2, 128, 43], f16)
        pools_o = dram.tile([2, 2, 128, 43], f16)
        nc.sync.dma_start(pools_d.rearrange("j p k -> p j k"),
                          pools16[:, :, :])
        nc.gpsimd.collective_compute(
            "AllGather", ALU.bypass, replica_groups=PAIRS,
            ins=[pools_d[:].opt()], outs=[pools_o[:].opt()])
        # 1x1 convs on pools in TRANSPOSED layout (out partition = pool
        # block, free = psp channel). The OWN half comes straight from
        # pools16 in SBUF and runs DURING the AllGather; only the partner
        # half waits for the collective. Wup's slot dim is indexed
        # dynamically (own = rank-in-pair, partner = the other).
        SI = {1: 0, 2: 1, 4: 2, 8: 3}
        pdT_a8 = [wk.tile([32, 64], f16, tag=f"pdT_a8{s}", name=f"pdT_a8{s}")
                  for s in range(2)]
        pdT_s1 = wk.tile([1, 64], f16, name="pdT_s1")
        pdT_b = [wk.tile([8, 64], f16, tag=f"pdT_b{s}", name=f"pdT_b{s}")
                 for s in range(2)]
        pdT_c = [wk.tile([2, 64], f16, tag=f"pdT_c{s}", name=f"pdT_c{s}")
                 for s in range(2)]
        pri = [wk.tile([128, 512], f16, tag=f"pri{i}", name=f"pri{i}")
               for i in range(2)]
        ones_f = wk.tile([128, 512], f16, name="ones_f")
        nc.vector.memset(ones_f[:], 1.0)
        psP_cm = tc.tile_pool(name="psP", bufs=1, space="PSUM")
        psP = psP_cm.__enter__()
        psR_cm = tc.tile_pool(name="psR", bufs=1, space="PSUM")
        psR = psR_cm.__enter__()
        pdm_ps = psP.tile([65, 64], f32, name="pdm_ps")
        pd4_ps = psP.tile([40, 64], f32, name="pd4_ps")
        pd2_ps = psP.tile([34, 64], f32, name="pd2_ps")
        pp0 = psR.tile([128, 512], f32, tag="pp0", name="pp0")
        pp1 = psR.tile([128, 512], f32, tag="pp1", name="pp1")

        def pd_matmuls(src, sl):
            # src[j] -> [128, 43] pool partials for this half (j = ch chunk)
            for j in range(2):
                nc.tensor.matmul(pdm_ps[32 * sl: 32 * sl + 32, :],
                                 src(j, 11, 43), psp_wT[:, j, SI[8], :],
                                 start=(j == 0), stop=(j == 1),
                                 skip_group_check=True)
            for j in range(2):
                nc.tensor.matmul(pdm_ps[64:65, :],
                                 src(j, 0, 1), psp_wT[:, j, SI[1], :],
                                 start=(sl == 0 and j == 0),
                                 stop=(sl == 1 and j == 1),
                                 skip_group_check=True)
            for j in range(2):
                nc.tensor.matmul(pd4_ps[32 * sl: 32 * sl + 8, :],
                                 src(j, 3, 11), psp_wT[:, j, SI[4], :],
                                 start=(j == 0), stop=(j == 1),
                                 skip_group_check=True)
            for j in range(2):
                nc.tensor.matmul(pd2_ps[32 * sl: 32 * sl + 2, :],
                                 src(j, 1, 3), psp_wT[:, j, SI[2], :],
                                 start=(j == 0), stop=(j == 1),
                                 skip_group_check=True)

        def pd_copies(sl):
            nc.scalar.copy(pdT_a8[sl][:, :], pdm_ps[32 * sl: 32 * sl + 32, :])
            nc.vector.tensor_copy(pdT_b[sl][:, :],
                                  pd4_ps[32 * sl: 32 * sl + 8, :])
            nc.vector.tensor_copy(pdT_c[sl][:, :],
                                  pd2_ps[32 * sl: 32 * sl + 2, :])

        def upsample(sl, slot_idx):
            nc.tensor.matmul(pp0[64:128, :], pdT_c[sl][:, :],
                             Wup[0:2, bass.ds(slot_idx, 1), 0, :],
                             start=(sl == 0), stop=(sl == 1),
                             tile_position=(0, 64), skip_group_check=True)
            nc.tensor.matmul(pp1[0:64, :], pdT_b[sl][:, :],
                             Wup[0:8, bass.ds(slot_idx, 1), 1, :],
                             start=(sl == 0), stop=(sl == 1),
                             tile_position=(0, 0), skip_group_check=True)
            nc.tensor.matmul(pp1[64:128, :], pdT_a8[sl][:, :],
                             Wup[0:32, bass.ds(slot_idx, 1), 2, :],
                             start=(sl == 0), stop=(sl == 1),
                             tile_position=(0, 64), skip_group_check=True)

        psW3_cm = tc.tile_pool(name="psW3", bufs=1, space="PSUM")
        psW3 = psW3_cm.__enter__()
        # own half: runs while the collective is in flight
        pd_matmuls(lambda j, a, b: pools16[:, j, a:b], 0)
        pd_copies(0)
        upsample(0, pid2)
        # keep the PE clock ramped through the pools AllGather
        spin_pe(psW3, 88)
        # partner half: after the collective
        slp_p = wk.tile([128, 2, 43], f16, name="slp_p")
        nc.sync.dma_start(
            slp_p[:, :, :],
            pools_o[bass.ds(omh, 1)].rearrange("s j p k -> s p j k"))
        pd_matmuls(lambda j, a, b: slp_p[:, j, a:b], 1)
        pd_copies(1)
        nc.scalar.copy(pdT_s1[:, :], pdm_ps[64:65, :])
        upsample(1, omh)
        nc.tensor.matmul(pp0[0:64, :], pdT_s1[:, :], ones_f[0:1, :],
                         start=True, stop=True, skip_group_check=True)
        nc.scalar.copy(pri[0][:, :], pp0[:, :])
        nc.scalar.copy(pri[1][:, :], pp1[:, :])
        psW3_cm.__exit__(None, None, None)
        psR_cm.__exit__(None, None, None)
        psP_cm.__exit__(None, None, None)

        if stage == "dbg":
            nc.sync.dma_start(prm["dbg_pd"][0:32], pdT_a8[0][:])
            nc.sync.dma_start(prm["dbg_pd"][32:64], pdT_a8[1][:])
            nc.sync.dma_start(prm["dbg_pd"][64:65], pdT_s1[:])

        if stage == "dbg":
            nc.sync.dma_start(prm["dbg_pri0"][:], pri[0][:])
            nc.sync.dma_start(prm["dbg_pri1"][:], pri[1][:])

        out_sb = wk.tile([128, 2, 512], f32, name="out_sb")
        for m in range(2):
            po = po_t[m]
            for k in range(2):
                nc.tensor.matmul(po[:, :],
                                 bott_wT[:, k, m * 128: m * 128 + 128],
                                 pri[k][:, :],
                                 start=False, stop=(k == 1),
                                 skip_group_check=True)
            nc.scalar.activation(out_sb[:, m, :], po[:, :], AF.Relu,
                                 bias=bott_b[:, m: m + 1])
        psO_cm.__exit__(None, None, None)
        nc.sync.dma_start(out_prm[:],
                          out_sb.rearrange("p j (r c) -> p j r c", c=32))


# ---------------------------------------------------------------------------
# Runner
# ---------------------------------------------------------------------------

_CACHE = {}


def _get_nc(stage="full"):
    if stage not in _CACHE:
        _CACHE[stage] = build(stage)
    return _CACHE[stage]


def run_cores(inputs, stage="full"):
    nc = _get_nc(stage)
    in_maps = [prep_core_inputs(inputs, c) for c in range(N_CORES)]
    res = run_bass_kernel_spmd(nc, in_maps, list(range(N_CORES)))
    return res.results


def kernel(**inputs):
    results = run_cores(inputs, "full")
    out = np.zeros((B, 1, COUT, H, W), np.float32)
    for c in range(N_CORES):
        b, h = c // 2, c % 2
        o = results[c]["out"]                    # [128, 2, 16, 32]
        out[b, 0, :, 16 * h: 16 * h + 16, :] = (
            o.transpose(1, 0, 2, 3).reshape(COUT, 16, 32))
    return out



# revision 103
# speedup vs baseline: 1.0062x; 1.0062x over previous
"""Trainium2 Bass kernel for nn_BottleneckFusion (STCN memory readout + ResBlock
+ CBAM + PSP + bottleneck), 8-core SPMD.

Sharding: core c -> (batch b = c//2, half h = c%2).
  Phase A (attention): TM split across the pair (4 memory frames each);
    software-pipelined affinity/exp/value loop; flash-style combine of
    (unnormalized value, sumexp) via a pairwise AllGather TRIMMED to the
    21-image-row window the partner actually needs (dynamic partition-id
    driven send/receive offsets).
  Phase B (convs/CBAM/PSP): row-half split with halo recompute. The
    batch-query half of the ResBlock conv accumulation is emitted before the
    value AllGather so it fills the collective window; PE-warming spin
    matmuls keep the tensor clock ramped through the other collectives.
    CBAM sigmoids use 1/(1+exp(-z)) so only the exp act table is ever
    loaded; the spatial-gate channel max uses a gpsimd cross-partition
    reduce (no transposes); PSP 1x1 convs are emitted directly in
    transposed (block-major) layout with the own-half computed from local
    pools during the pools AllGather and dynamic Wup slot slices.

kernel(**inputs) takes the FULL unsharded inputs and returns the FULL output.
"""
import sys

sys.path.insert(0, "/opt/trn_rl_repo")

import numpy as np
import ml_dtypes

import concourse.bass as bass
import concourse.bacc as bacc
import concourse.bass_isa as bass_isa
import concourse.mybir as mybir
import concourse.tile as tile
from concourse.bass_utils import run_bass_kernel_spmd

BF16 = ml_dtypes.bfloat16
F16 = np.float16
bf = mybir.dt.bfloat16
f16 = mybir.dt.float16
f32 = mybir.dt.float32
AF = mybir.ActivationFunctionType
ALU = mybir.AluOpType
AX = mybir.AxisListType

N_CORES = 8
B, TM, CIN, CK, CV, COUT, H, W = 4, 8, 256, 64, 256, 256, 32, 32
EPS = 1e-5

# local row coordinates: l = image_row - (r0 - 5), l in 0..25
XROWS = 26                 # x window rows (image r0-5 .. r0+20)
CROWS = 22                 # xb/xc/comp local rows (image r0-3 .. r0+18)
PIXPAD = 768               # padded xc free size (22*34=748 -> 768)
PAIRS = [[0, 1], [2, 3], [4, 5], [6, 7]]
UPS = (2, 4, 8)            # upsampled PSP scales
# full pool pyramid offsets [s1, s2, s4, s8] and own-partial offsets
FOFF = {1: 0, 2: 1, 4: 5, 8: 21}
POFF = {1: 0, 2: 1, 4: 3, 8: 11}


def interp_matrix(s_in, s_out=32):
    if s_in == 1:
        return np.ones((s_out, 1), np.float32)
    c = np.arange(s_out) * (s_in - 1) / (s_out - 1)
    lo = np.floor(c).astype(np.int64)
    hi = np.minimum(lo + 1, s_in - 1)
    w = (c - lo).astype(np.float32)
    M = np.zeros((s_out, s_in), np.float32)
    M[np.arange(s_out), lo] += 1.0 - w
    M[np.arange(s_out), hi] += w
    return M


# ---------------------------------------------------------------------------
# Host-side input preparation
# ---------------------------------------------------------------------------

def _pad_hw(a):
    out = np.zeros(a.shape[:-2] + (34, 34), a.dtype)
    out[..., 1:33, 1:33] = a
    return out


def _chw_chunks(a):
    """[256, ...] -> [128, 2, ...] (partition, chunk)."""
    return a.reshape(2, 128, *a.shape[1:]).transpose(
        1, 0, *range(2, a.ndim + 1))


def prep_core_inputs(inputs, core):
    b, h = core // 2, core % 2
    r0 = 16 * h
    g = {}

    f16_q = np.asarray(inputs["f16_q"], np.float32)
    f16_m = np.asarray(inputs["f16_m"], np.float32)
    value_m = np.asarray(inputs["value_m"], np.float32)

    # xm: [128, 2, 4, 34, 34] padded memory frames
    src = f16_m[b, 4 * h: 4 * h + 4]                        # [4, 256, 32, 32]
    src = src.reshape(4, 2, 128, 32, 32).transpose(2, 1, 0, 3, 4)
    g["xm"] = _pad_hw(src).astype(F16)

    # xq: [128, 2, 34, 34] padded query
    q = _chw_chunks(f16_q[b, 0])                            # [128, 2, 32, 32]
    g["xq"] = _pad_hw(q).astype(F16)

    # vT: [128, 32, 256] transposed value
    V = value_m[b][:, 4 * h: 4 * h + 4].reshape(CV, 4096)
    g["vT"] = np.ascontiguousarray(
        V.T.reshape(32, 128, CV).transpose(1, 0, 2)).astype(BF16)  # stays bf16 (matches e)

    # x window q-part: [128, 2, 26, 34]
    qw = np.zeros((128, 2, XROWS, 34), np.float32)
    for l in range(XROWS):
        img = r0 - 5 + l
        if 0 <= img <= 31:
            qw[:, :, l, 1:33] = q[:, :, img, :]
    g["xqb_raw"] = qw.astype(F16)
    g["xqb_relu"] = np.maximum(qw, 0.0).astype(F16)

    pk_w = np.asarray(inputs["pk_w"], np.float32)
    g["pk_wT"] = np.ascontiguousarray(
        pk_w.reshape(CK, 2, 128, 3, 3).transpose(2, 1, 3, 4, 0)).astype(F16)
    pk_b = np.asarray(inputs["pk_b"], np.float32)
    g["pkb2"] = np.concatenate([pk_b, pk_b]).reshape(128, 1).astype(np.float32)

    def conv_lhsT(w, kc):
        co = w.shape[0]
        return np.ascontiguousarray(
            w.reshape(co, kc, 128, 3, 3).transpose(2, 1, 3, 4, 0)).astype(F16)

    g["rb1_wT"] = conv_lhsT(np.asarray(inputs["rb1_w"], np.float32), 4)
    g["rb2_wT"] = conv_lhsT(np.asarray(inputs["rb2_w"], np.float32), 2)
    g["rbd_wT"] = conv_lhsT(np.asarray(inputs["rbd_w"], np.float32), 4)
    g["rb1_b"] = np.asarray(inputs["rb1_b"], np.float32).reshape(2, 128).T.copy()
    g["xb_bias"] = (np.asarray(inputs["rb2_b"], np.float32)
                    + np.asarray(inputs["rbd_b"], np.float32)
                    ).reshape(2, 128).T.copy()

    w1 = np.asarray(inputs["mlp_w1"], np.float32)           # [16, 256]
    g["mlp_w1T"] = np.ascontiguousarray(
        w1.reshape(16, 2, 128).transpose(2, 1, 0)).copy()   # [128, 2, 16]
    g["mlp_b1"] = np.asarray(inputs["mlp_b1"], np.float32).reshape(16, 1).copy()
    g["mlp_w2T"] = np.ascontiguousarray(
        np.asarray(inputs["mlp_w2"], np.float32).T).astype(F16)  # [16, 256]
    # sigmoid is computed as 1/(1+exp(-z)) on the exp table, so biases are
    # stored pre-negated for the exp activation
    g["mlp_b2x2n"] = (-2.0 * np.asarray(inputs["mlp_b2"], np.float32)
                      ).reshape(2, 128).T.copy()
    g["mlp_b2row"] = (2.0 * np.asarray(inputs["mlp_b2"], np.float32)
                      ).reshape(1, 256).astype(F16)

    spw = np.asarray(inputs["sp_w"], np.float32)[0]       # [2, 7, 7]
    g["spw_r"] = np.ascontiguousarray(
        spw.reshape(14, 7)).astype(np.float16)                # [(ch,dy), dx]
    bn_scale = float(np.asarray(inputs["sp_g"], np.float32)[0]) / float(
        np.sqrt(1.0 + EPS))
    bn_bias = float(np.asarray(inputs["sp_b"], np.float32)[0])
    g["bn_nsb"] = np.array([[-bn_scale, -bn_bias]], np.float32)

    maskT = np.zeros((128, 6, 1), np.float16)
    mask_mean = np.zeros((1, 768), np.float16)
    for pix in range(CROWS * 34):
        img = r0 - 3 + pix // 34
        if 0 <= img <= 31:
            maskT[pix % 128, pix // 128, 0] = 1.0
            mask_mean[0, pix] = 1.0
    g["comp_maskT"] = maskT
    g["mask_mean"] = mask_mean

    pw = np.zeros((128, 2, 4, 64), np.float32)
    for si, s in enumerate((1, 2, 4, 8)):
        wc = np.asarray(inputs[f"psp_w{s}"], np.float32)[:, :, 0, 0]
        scale = 1.0 / ((32 // s) ** 2)
        pw[:, :, si, :] = (wc.T * scale).reshape(2, 128, 64).transpose(1, 0, 2)
    g["psp_wT"] = pw.astype(F16)

    # folded upsample operators, split per AG slot so every matmul operand
    # sits at partition base 0: Wup[k_local, sl, si, (r*32+c)] with the
    # global block k = jr*s+jc split as sl = k // (s*s/2), k_local = k % ..
    Wup = np.zeros((32, 2, 3, 512), np.float32)
    for si, s in enumerate(UPS):
        M = interp_matrix(s)
        Mrr = M[r0: r0 + 16, :]                 # [16, s]
        half = s * s // 2
        for jr in range(s):
            for jc in range(s):
                k = jr * s + jc
                Wup[k % half, k // half, si, :] = np.outer(
                    Mrr[:, jr], M[:, jc]).reshape(512)
    g["Wup"] = Wup.astype(F16)

    bott_w = np.asarray(inputs["bott_w"], np.float32)[:, :, 0, 0]
    g["bott_wT"] = np.ascontiguousarray(
        bott_w.reshape(COUT, 4, 128).transpose(2, 1, 0)).astype(F16)
    g["bott_b"] = np.asarray(inputs["bott_b"], np.float32).reshape(2, 128).T.copy()

    rmask = np.zeros((1, XROWS, 34), np.float16)
    for l in range(XROWS):
        if 0 <= r0 - 5 + l <= 31:
            rmask[0, l, :] = 1.0
    g["rmask"] = rmask

    g["ident"] = np.eye(128, dtype=F16)
    return g


INPUT_SPECS = [
    ("xm", [128, 2, 4, 34, 34], f16),
    ("xq", [128, 2, 34, 34], f16),
    ("vT", [128, 32, 256], bf),
    ("xqb_raw", [128, 2, XROWS, 34], f16),
    ("xqb_relu", [128, 2, XROWS, 34], f16),
    ("pk_wT", [128, 2, 3, 3, 64], f16),
    ("pkb2", [128, 1], f32),
    ("rb1_wT", [128, 4, 3, 3, 256], f16),
    ("rb2_wT", [128, 2, 3, 3, 256], f16),
    ("rbd_wT", [128, 4, 3, 3, 256], f16),
    ("rb1_b", [128, 2], f32),
    ("xb_bias", [128, 2], f32),
    ("mlp_w1T", [128, 2, 16], f32),
    ("mlp_b1", [16, 1], f32),
    ("mlp_w2T", [16, 256], f16),
    ("mlp_b2x2n", [128, 2], f32),
    ("mlp_b2row", [1, 256], f16),
    ("spw_r", [14, 7], f16),
    ("bn_nsb", [1, 2], f32),
    ("comp_maskT", [128, 6, 1], f16),
    ("mask_mean", [1, 768], f16),
    ("psp_wT", [128, 2, 4, 64], f16),
    ("Wup", [32, 2, 3, 512], f16),
    ("bott_wT", [128, 4, 256], f16),
    ("bott_b", [128, 2], f32),
    ("ident", [128, 128], f16),
    ("rmask", [1, XROWS, 34], f16),
]


# ---------------------------------------------------------------------------
# Device kernel
# ---------------------------------------------------------------------------

def build(stage="full"):
    nc = bacc.Bacc("TRN2", target_bir_lowering=False, debug=False,
                   num_devices=N_CORES)
    prm = {n: nc.declare_dram_parameter(n, sh, dt, isOutput=False)
           for n, sh, dt in INPUT_SPECS}
    if stage == "A":
        out_prm = nc.declare_dram_parameter("out_a", [257, 1024], f32,
                                            isOutput=True)
    else:
        out_prm = nc.declare_dram_parameter("out", [128, 2, 16, 32], f32,
                                            isOutput=True)
    if stage == "dbg":
        for n, sh, dt in [("dbg_xraw", [128, 4, XROWS, 34], f16),
                          ("dbg_xb", [128, 2, CROWS, 34], f16),
                          ("dbg_gate", [128, 2, 1], f32),
                          ("dbg_sig", [1, 512], f16),
                          ("dbg_fused", [128, 2, 16, 32], f16),
                          ("dbg_pd", [65, 64], f16),
                          ("dbg_il", [14, 16, 38], f16),
                          ("dbg_cmp", [1, 768], f16),
                          ("dbg_pri0", [128, 512], f16),
                          ("dbg_pri1", [128, 512], f16)]:
            prm[n] = nc.declare_dram_parameter(n, sh, dt, isOutput=True)
    with tile.TileContext(nc) as tc:
        _emit(tc, nc, prm, stage, out_prm)
    nc.compile()
    return nc


def _emit(tc, nc, prm, stage, out_prm):
    import contextlib
    es = contextlib.ExitStack()
    with es:
        wpool = es.enter_context(tc.tile_pool(name="wpool", bufs=1))
        apool = es.enter_context(tc.tile_pool(name="apool", bufs=1))
        dram = es.enter_context(tc.tile_pool(name="dram", bufs=1, space="DRAM"))
        aonly_cm = tc.tile_pool(name="aonly", bufs=1)
        aonly = aonly_cm.__enter__()

        def load(name, pool=wpool):
            t = pool.tile(list(prm[name].shape), prm[name].dtype,
                          name=f"{name}_sb")
            nc.sync.dma_start(t[:], prm[name][:])
            return t

        pk_wT = load("pk_wT")
        pkb2 = load("pkb2")
        xm_sb = aonly.tile([128, 2, 4, 34, 34], f16, name="xm_sb")
        # frames 0/1 rows 0:19 land first so the mk conv starts early
        for t in range(2):
            nc.sync.dma_start(xm_sb[:, :, t, 0:19, :],
                              prm["xm"][:, :, t, 0:19, :])
        for t in range(2):
            nc.sync.dma_start(xm_sb[:, :, t, 19:34, :],
                              prm["xm"][:, :, t, 19:34, :])
        for t in range(2, 4):
            nc.sync.dma_start(xm_sb[:, :, t, :, :], prm["xm"][:, :, t, :, :])
        xq_sb = load("xq", aonly)
        vT_sb = load("vT", aonly)

        ones_bf = wpool.tile([128, 1], bf)
        nc.vector.memset(ones_bf[:], 1.0)
        # spin sources: tiny constant operands for PE-warming matmuls that
        # keep the tensor clock ramped through collective windows
        spin_w = wpool.tile([1, 1], f16)
        nc.vector.memset(spin_w[:], 0.0)
        spin_src = wpool.tile([1, 512], f16)
        nc.vector.memset(spin_src[:], 0.0)
        zero128 = wpool.tile([1, 128], f16)
        nc.vector.memset(zero128[:], 0.0)

        def spin_pe(pool, n, rows=512):
            for _ in range(n):
                sp = pool.tile([1, rows], f32, tag="spin", name="sp")
                nc.tensor.matmul(sp[0:1, :], spin_w[0:1, 0:1],
                                 spin_src[0:1, 0:rows],
                                 start=True, stop=True,
                                 skip_group_check=True)

        # ================= phase A =================
        mk_sb = aonly.tile([128, 2, 1024], f16)
        qk_sb = aonly.tile([128, 1024], f16)

        with tc.tile_pool(name="psA", bufs=2, space="PSUM") as psA:
            for tp in range(2):
                for n in range(2):
                    pm = psA.tile([128, 512], f32, tag="mkps", name="pm")
                    for par in range(2):
                        t = 2 * tp + par
                        k = 0
                        for j in range(2):
                            for dy in range(3):
                                for dx in range(3):
                                    nc.tensor.matmul(
                                        pm[64 * par: 64 * par + 64, :],
                                        pk_wT[:, j, dy, dx, :],
                                        xm_sb[:, j, t,
                                              n * 16 + dy: n * 16 + dy + 16,
                                              dx: dx + 32],
                                        start=(k == 0), stop=(k == 17),
                                        tile_position=(0, 64 * par),
                                    )
                                    k += 1
                    nc.scalar.activation(
                        mk_sb[:, tp, n * 512: (n + 1) * 512], pm[:, :],
                        AF.Identity, bias=pkb2[:, 0:1])

            for n in range(2):
                pq = psA.tile([64, 512], f32, tag="qkps", name="pq")
                k = 0
                for j in range(2):
                    for dy in range(3):
                        for dx in range(3):
                            nc.tensor.matmul(
                                pq[:, :], pk_wT[:, j, dy, dx, :],
                                xq_sb[:, j, n * 16 + dy: n * 16 + dy + 16,
                                      dx: dx + 32],
                                start=(k == 0), stop=(k == 17))
                            k += 1
                nc.scalar.activation(
                    qk_sb[0:64, n * 512: (n + 1) * 512], pq[:, :],
                    AF.Identity, bias=pkb2[0:64, 0:1])
            # replicate qk to partitions 64..127 so odd-frame mk slices
            # (base partition 64) can stream against it
            nc.sync.dma_start(qk_sb[64:128, :], qk_sb[0:64, :])

        # pair exchange buffers: full [257,1024] for the debug stage, a
        # 21-image-row window (the part the partner actually needs) otherwise
        if stage == "A":
            arv = dram.tile([257, 1024], bf)
            arvg = dram.tile([2, 257, 1024], bf)
        else:
            arv2 = dram.tile([257, 672], bf)
            arvg2 = dram.tile([2, 257, 672], bf)
        pid = nc.partition_id()
        pid2 = pid % 2
        omh = (pid + 1) % 2
        sendoff = omh * 352
        myoff = pid2 * 352
        vstart160 = omh * 160

        with (
            tc.tile_pool(name="psAff", bufs=2, space="PSUM") as psAff,
            tc.tile_pool(name="psV", bufs=1, space="PSUM") as psV,
        ):
            vps = [psV.tile([128, 1024], f32, name=f"vps{j}") for j in range(2)]
            s_acc = aonly.tile([128, 1024], bf, name="s_acc")

            order = [16 * h + o + 8 * par for h in range(2) for o in range(8)
                     for par in range(2)]

            def lhs_aff(i):
                t = i >> 3
                pb = i & 7
                tp, par = t >> 1, t & 1
                return par, mk_sb[64 * par: 64 * par + 64, tp,
                                  pb * 128: pb * 128 + 128]

            # software-pipelined: affinity matmuls + exp run one chunk ahead
            # of the value accumulation so the PE never waits on the exp.
            e_tiles = {}

            def emit_aff(idx):
                i = order[idx]
                par, lhs = lhs_aff(i)
                e_t = aonly.tile([128, 1024], bf, tag="e", name="e_t", bufs=4)
                pa = psAff.tile([128, 1024], f32, tag="affp", name="pa")
                for qn in range(2):
                    nc.tensor.matmul(
                        pa[:, qn * 512: (qn + 1) * 512], lhs,
                        qk_sb[64 * par: 64 * par + 64,
                              qn * 512: (qn + 1) * 512],
                        start=True, stop=True)
                nc.scalar.activation(e_t[:, :], pa[:, :], AF.Exp, scale=0.125)
                e_tiles[idx] = e_t

            emit_aff(0)
            for idx in range(32):
                if idx + 1 < 32:
                    emit_aff(idx + 1)
                i = order[idx]
                e_t = e_tiles.pop(idx)
                for j in range(2):
                    for qn in range(2):
                        nc.tensor.matmul(
                            vps[j][:, qn * 512: (qn + 1) * 512],
                            vT_sb[:, i, j * 128: (j + 1) * 128],
                            e_t[:, qn * 512: (qn + 1) * 512],
                            start=(idx == 0), stop=(idx == 31),
                            skip_group_check=True)
                if idx == 0:
                    nc.vector.tensor_copy(s_acc[:, :], e_t[:, :])
                else:
                    nc.vector.tensor_add(s_acc[:, :], s_acc[:, :], e_t[:, :])

            v_sb = apool.tile([128, 2, 1024], bf, name="v_sb")
            s_sb = apool.tile([1, 1024], bf, name="s_sb")
            # one PSUM->SBUF copy on DVE, one on Act so they run concurrently
            nc.vector.tensor_copy(v_sb[:, 0, :], vps[0][:, :])
            nc.scalar.copy(v_sb[:, 1, :], vps[1][:, :])
            if stage == "A":
                for j in range(2):
                    nc.sync.dma_start(arv[128 * j: 128 * j + 128, :],
                                      v_sb[:, j, :])
            else:
                nc.sync.dma_start(
                    arv2[0:256, :].rearrange("(j p) w -> p j w", j=2),
                    v_sb[:, :, bass.ds(sendoff, 672)])
            # fold the 128-partition sumexp accumulator with a ones matmul
            for qn in range(2):
                sfold = psAff.tile([1, 512], f32, tag="affp", name="sfold")
                nc.tensor.matmul(sfold[0:1, :],
                                 ones_bf[:, 0:1],
                                 s_acc[:, qn * 512: (qn + 1) * 512],
                                 start=True, stop=True)
                nc.vector.tensor_copy(s_sb[:, qn * 512: (qn + 1) * 512],
                                      sfold[0:1, :])
            if stage == "A":
                nc.sync.dma_start(arv[256:257, :], s_sb[:, :])
            else:
                nc.sync.dma_start(arv2[256:257, :],
                                  s_sb[0:1, bass.ds(sendoff, 672)])

        if stage == "A":
            nc.gpsimd.collective_compute(
                "AllGather", ALU.bypass, replica_groups=PAIRS,
                ins=[arv[:].opt()], outs=[arvg[:].opt()])
        else:
            nc.gpsimd.collective_compute(
                "AllGather", ALU.bypass, replica_groups=PAIRS,
                ins=[arv2[:].opt()], outs=[arvg2[:].opt()])


        aonly_cm.__exit__(None, None, None)

        if stage == "A":
            with tc.tile_pool(name="cmb", bufs=1) as cmb:
                cs0 = cmb.tile([1, 1024], bf, name="cs0")
                cs1 = cmb.tile([1, 1024], bf, name="cs1")
                cso = cmb.tile([1, 1024], f32, name="cso")
                nc.sync.dma_start(cs0[:], arvg[0, 256:257, :])
                nc.sync.dma_start(cs1[:], arvg[1, 256:257, :])
                nc.vector.tensor_add(cso[:, :], cs0[:, :], cs1[:, :])
                nc.sync.dma_start(out_prm[256:257, :], cso[:, :])
                for j in range(2):
                    ca = cmb.tile([128, 1024], bf, tag="ca", name="ca")
                    cb = cmb.tile([128, 1024], bf, tag="cb", name="cb")
                    co = cmb.tile([128, 1024], f32, tag="co", name="co")
                    nc.sync.dma_start(ca[:, :], arvg[0, 128 * j: 128 * j + 128, :])
                    nc.sync.dma_start(cb[:, :], arvg[1, 128 * j: 128 * j + 128, :])
                    nc.vector.tensor_add(co[:, :], ca[:, :], cb[:, :])
                    nc.sync.dma_start(out_prm[128 * j: 128 * j + 128, :],
                                      co[:, :])
            return

        # ================= phase B =================
        wk = es.enter_context(tc.tile_pool(name="wk", bufs=1))
        rb1_wT = load("rb1_wT")
        rb2_wT = load("rb2_wT")
        rbd_wT = load("rbd_wT")
        rb1_b = load("rb1_b")
        xb_bias = load("xb_bias")
        mlp_w1T = load("mlp_w1T")
        mlp_b1 = load("mlp_b1")
        mlp_w2T = load("mlp_w2T")
        mlp_b2x2n = load("mlp_b2x2n")
        spw_r = load("spw_r")
        bn_nsb = load("bn_nsb")
        mask_mean = load("mask_mean")
        psp_wT = load("psp_wT")
        Wup = load("Wup")
        bott_wT = load("bott_wT")
        bott_b = load("bott_b")
        ident = load("ident")

        r0v = (nc.vector.partition_id() % 2) * 16

        # ---- val-independent prep: x tiles, query-side loads, masks ----
        x_raw = apool.tile([128, 4, XROWS, 34], f16)
        x_relu = apool.tile([128, 4, XROWS, 34], f16)
        for tt in (x_raw, x_relu):
            nc.vector.memset(tt[:, 2:4, :, 0:1], 0.0)
            nc.vector.memset(tt[:, 2:4, :, 33:34], 0.0)
        nc.sync.dma_start(x_raw[:, 0:2, :, :], prm["xqb_raw"][:])
        nc.sync.dma_start(x_relu[:, 0:2, :, :], prm["xqb_relu"][:])

        r1_relu = apool.tile([128, 2, XROWS, 34], f16)
        nc.vector.memset(r1_relu[:, :, 0:1, :], 0.0)
        nc.vector.memset(r1_relu[:, :, 25:26, :], 0.0)
        nc.vector.memset(r1_relu[:, :, :, 0:1], 0.0)
        nc.vector.memset(r1_relu[:, :, :, 33:34], 0.0)
        rmaskb = apool.tile([128, XROWS, 34], f16)
        nc.sync.dma_start(rmaskb[:], prm["rmask"][:].partition_broadcast(128))
        xb = apool.tile([128, 2, PIXPAD], f16)
        xbv = [xb[:, j, 0: CROWS * 34].rearrange("p (r c) -> p r c", c=34)
               for j in range(2)]
        for j in range(2):
            nc.vector.memset(xbv[j][:, :, 0:1], 0.0)
            nc.vector.memset(xbv[j][:, :, 33:34], 0.0)
        nc.vector.memset(xb[:, :, CROWS * 34:], 0.0)

        # val window tiles in x-window coordinates (26 rows = XROWS); the
        # 21-row valid band sits at dynamic row offset 5*(1-h). Pad rows are
        # zeroed statically (both possible pad bands); the valid-band writes
        # land after and overwrite any overlap.
        val_pad = apool.tile([128, 2, 832], f32)
        nc.vector.memset(val_pad[:, :, 0:160], 0.0)
        nc.vector.memset(val_pad[:, :, 672:832], 0.0)
        # x val-part pad bands (rows outside the 21-row valid window) are
        # zeroed statically; the valid band is written at a dynamic offset
        for tt in (x_raw, x_relu):
            nc.vector.memset(tt[:, 2:4, 0:5, 1:33], 0.0)
            nc.vector.memset(tt[:, 2:4, 21:26, 1:33], 0.0)

        # ---- query-side conv accumulation: fills the PE while the val
        # AllGather is in flight (j=0,1 of x are batch-query channels) ----
        psR1_cm = tc.tile_pool(name="psR1", bufs=1, space="PSUM")
        psR1 = psR1_cm.__enter__()
        psXB_cm = tc.tile_pool(name="psXB", bufs=1, space="PSUM")
        psXB = psXB_cm.__enter__()
        psW1_cm = tc.tile_pool(name="psW1", bufs=1, space="PSUM")
        psW1 = psW1_cm.__enter__()
        R1G = ((0, 1, 16), (0, 17, 8), (1, 1, 16), (1, 17, 8))
        XBG = ((0, 2, 16), (0, 18, 6), (1, 2, 16), (1, 18, 6))
        # the two short row-groups per producer share one PSUM bank
        # (independent column ranges) so all 8 accumulators fit in 6 banks
        pr_t = {}
        px_t = {}
        r1sm = psR1.tile([128, 512], f32, tag="r1s", name="r1sm")
        xbsm = psXB.tile([128, 384], f32, tag="xbs", name="xbsm")
        # a start=True matmul resets the whole PSUM bank, so shared banks are
        # zeroed once up front and every accumulation into them avoids start
        nc.tensor.matmul(r1sm[:, :], zero128[0:1, :], spin_src[0:1, 0:512],
                         start=True, stop=False, skip_group_check=True)
        nc.tensor.matmul(xbsm[:, :], zero128[0:1, :], spin_src[0:1, 0:384],
                         start=True, stop=False, skip_group_check=True)
        for m in range(2):
            pr_t[(m, 1)] = psR1.tile([128, 512], f32, tag=f"r1b{m}",
                                     name="prb")
            pr_t[(m, 17)] = r1sm[:, m * 256: m * 256 + 256]
            px_t[(m, 2)] = psXB.tile([128, 512], f32, tag=f"xbb{m}",
                                     name="pxb")
            px_t[(m, 18)] = xbsm[:, m * 192: m * 192 + 192]
        for (m, l0, nr) in R1G:
            pr = pr_t[(m, l0)]
            k = 0
            for j in range(2):
                for dy in range(3):
                    for dx in range(3):
                        nc.tensor.matmul(
                            pr[:, : nr * 32],
                            rb1_wT[:, j, dy, dx, m * 128: m * 128 + 128],
                            x_relu[:, j, l0 + dy - 1: l0 + dy - 1 + nr,
                                   dx: dx + 32],
                            start=(k == 0 and nr == 16), stop=False,
                            skip_group_check=True)
                        k += 1
        for (m, l0, nr) in XBG:
            px = px_t[(m, l0)]
            k = 0
            for j in range(2):
                for dy in range(3):
                    for dx in range(3):
                        nc.tensor.matmul(
                            px[:, : nr * 32],
                            rbd_wT[:, j, dy, dx, m * 128: m * 128 + 128],
                            x_raw[:, j, l0 + dy - 1: l0 + dy - 1 + nr,
                                  dx: dx + 32],
                            start=(k == 0 and nr == 16), stop=False,
                            skip_group_check=True)
                        k += 1
        # keep the PE clock ramped through the rest of the AllGather window
        spin_pe(psW1, 48)

        # ---- val: own window (SBUF) + partner window (AG slot), normalize,
        # window into x ----
        vs_p = wk.tile([128, 2, 672], bf, name="vs_p")
        nc.sync.dma_start(
            vs_p[:, :, :],
            arvg2[bass.ds(omh, 1), 0:256, :].rearrange(
                "s (j p) w -> s p j w", j=2))
        sp_row = wk.tile([1, 672], bf, name="sp_row")
        nc.gpsimd.dma_start(sp_row[:, :], arvg2[bass.ds(omh, 1), 256:257, :])
        for j in range(2):
            nc.vector.tensor_add(val_pad[:, j, bass.ds(vstart160, 672)],
                                 v_sb[:, j, bass.ds(myoff, 672)],
                                 vs_p[:, j, :])
        s_row = wk.tile([1, 672], f32, name="s_row")
        nc.vector.tensor_add(s_row[:, :], s_sb[0:1, bass.ds(myoff, 672)],
                             sp_row[:, :])
        inv_row = wk.tile([1, 672], f32, name="inv_row")
        nc.vector.reciprocal(inv_row[:, :], s_row[:, :])
        inv_d = dram.tile([1, 672], f32)
        nc.sync.dma_start(inv_d[:], inv_row[:, :])
        inv_b = wk.tile([128, 21, 32], f32, name="inv_b")
        nc.sync.dma_start(inv_b.rearrange("p r c -> p (r c)"),
                          inv_d.partition_broadcast(128))
        vp_v = val_pad.rearrange("p j (r c) -> p j r c", c=32)
        omh5 = omh * 5
        for j in range(2):
            nc.vector.tensor_mul(x_raw[:, 2 + j, bass.ds(omh5, 21), 1:33],
                                 vp_v[:, j, bass.ds(omh5, 21), :],
                                 inv_b[:, :, :])
            nc.scalar.activation(x_relu[:, 2 + j, :, 1:33],
                                 x_raw[:, 2 + j, :, 1:33], AF.Relu)

        # ---- val-side conv accumulation + activations ----
        for (m, l0, nr) in R1G:
            pr = pr_t[(m, l0)]
            k = 0
            for j in (2, 3):
                for dy in range(3):
                    for dx in range(3):
                        nc.tensor.matmul(
                            pr[:, : nr * 32],
                            rb1_wT[:, j, dy, dx, m * 128: m * 128 + 128],
                            x_relu[:, j, l0 + dy - 1: l0 + dy - 1 + nr,
                                   dx: dx + 32],
                            start=False, stop=(k == 17),
                            skip_group_check=True)
                        k += 1
            nc.scalar.activation(
                r1_relu[:, m, l0: l0 + nr, 1:33], pr[:, : nr * 32],
                AF.Relu, bias=rb1_b[:, m: m + 1])
            nc.vector.tensor_mul(r1_relu[:, m, l0: l0 + nr, 1:33],
                                 r1_relu[:, m, l0: l0 + nr, 1:33],
                                 rmaskb[:, l0: l0 + nr, 1:33])
        for (m, l0, nr) in XBG:
            px = px_t[(m, l0)]
            k = 0
            for j in (2, 3):
                for dy in range(3):
                    for dx in range(3):
                        nc.tensor.matmul(
                            px[:, : nr * 32],
                            rbd_wT[:, j, dy, dx, m * 128: m * 128 + 128],
                            x_raw[:, j, l0 + dy - 1: l0 + dy - 1 + nr,
                                  dx: dx + 32],
                            start=False, stop=False,
                            skip_group_check=True)
                        k += 1
            for j in range(2):
                for dy in range(3):
                    for dx in range(3):
                        nc.tensor.matmul(
                            px[:, : nr * 32],
                            rb2_wT[:, j, dy, dx, m * 128: m * 128 + 128],
                            r1_relu[:, j, l0 + dy - 1: l0 + dy - 1 + nr,
                                    dx: dx + 32],
                            start=False, stop=(k == 35),
                            skip_group_check=True)
                        k += 1
            nc.scalar.activation(
                xbv[m][:, l0 - 2: l0 - 2 + nr, 1:33], px[:, : nr * 32],
                AF.Identity, bias=xb_bias[:, m: m + 1])
        psW1_cm.__exit__(None, None, None)
        psXB_cm.__exit__(None, None, None)
        psR1_cm.__exit__(None, None, None)

        if stage == "dbg":
            nc.sync.dma_start(prm["dbg_xraw"][:], x_raw[:])
            for j in range(2):
                nc.sync.dma_start(prm["dbg_xb"][:, j], xbv[j])

        # ---- CBAM channel gate ----
        stats = wk.tile([128, 2, 2], f32, name="stats")
        for j in range(2):
            nc.vector.tensor_reduce(stats[:, j, 0:1], xbv[j][:, 3:19, 1:33],
                                    AX.XY, ALU.add)
            nc.vector.tensor_reduce(stats[:, j, 1:2], xbv[j][:, 3:19, 1:33],
                                    AX.XY, ALU.max)
        stats_d = dram.tile([256, 2], f32)
        stats_o = dram.tile([2, 256, 2], f32)
        nc.sync.dma_start(stats_d.rearrange("(j p) k -> p j k", j=2),
                          stats[:, :, :])
        # zeroed 38-stride comp rows in SBUF (borders give the conv halo);
        # the spatial-conv im2col then gathers straight from SBUF
        comp_sp = wk.tile([1, CROWS * 38], f16, name="comp_sp")
        mean_sp = wk.tile([1, CROWS * 38], f16, name="mean_sp")
        nc.vector.memset(comp_sp[:], 0.0)
        nc.vector.memset(mean_sp[:], 0.0)
        nc.gpsimd.collective_compute(
            "AllGather", ALU.bypass, replica_groups=PAIRS,
            ins=[stats_d[:].opt()], outs=[stats_o[:].opt()])
        slb = wk.tile([128, 2, 2, 2], f32, name="slb")  # [p, slot, j, stat]
        nc.sync.dma_start(slb[:, :, :, :],
                          stats_o.rearrange("s (j p) k -> p s j k", j=2))
        gate_in = wk.tile([128, 2, 2], f32, name="gate_in")
        tsum = wk.tile([128, 2, 1], f32, name="tsum")
        nc.vector.tensor_add(tsum[:, :, :], slb[:, 0, :, 0:1],
                             slb[:, 1, :, 0:1])
        nc.scalar.mul(gate_in[:, :, 0:1], tsum[:, :, :], 1.0 / 1024.0)
        nc.vector.tensor_max(gate_in[:, :, 1:2], slb[:, 0, :, 1:2],
                             slb[:, 1, :, 1:2])

        gate = wk.tile([128, 2, 1], f32, name="gate")
        ones1 = wk.tile([1, 128], f16, name="ones1")
        nc.vector.memset(ones1[:], 1.0)
        with tc.tile_pool(name="psG", bufs=1, space="PSUM") as psG:
            ph1 = psG.tile([16, 2], f32, name="ph1")
            for j in range(2):
                nc.tensor.matmul(ph1[:, :], mlp_w1T[:, j, :], gate_in[:, j, :],
                                 start=(j == 0), stop=(j == 1))
            h1 = wk.tile([16, 2], f16, name="h1")
            nc.scalar.activation(h1[:, :], ph1[:, :], AF.Relu,
                                 bias=mlp_b1[:, 0:1])
            # per-partition gate (sigmoid via the already-loaded exp table)
            for j in range(2):
                ph2 = psG.tile([128, 2], f32, tag="ph2", name="ph2")
                nc.tensor.matmul(ph2[:, :], mlp_w2T[:, j * 128: j * 128 + 128],
                                 h1[:, :], start=True, stop=True)
                h2 = wk.tile([128, 2], f32, tag="h2", name="h2")
                nc.vector.tensor_copy(h2[:, :], ph2[:, :])
                t2 = wk.tile([128, 1], f32, tag="t2", name="t2")
                nc.vector.tensor_add(t2[:, :], h2[:, 0:1], h2[:, 1:2])
                ev = wk.tile([128, 1], f32, tag="ev", name="ev")
                nc.scalar.activation(ev[:, :], t2[:, :], AF.Exp, scale=-1.0,
                                     bias=mlp_b2x2n[:, j: j + 1])
                e1 = wk.tile([128, 1], f32, tag="e1", name="e1")
                nc.scalar.activation(e1[:, :], ev[:, :], AF.Identity,
                                     bias=1.0)
                nc.vector.reciprocal(gate[:, j, :], e1[:, :])

        if stage == "dbg":
            nc.sync.dma_start(prm["dbg_gate"][:], gate[:])

        gate_sc = wk.tile([128, 2, 1], f16, name="gate_sc")
        nc.scalar.mul(gate_sc[:, :, :], gate[:, :, :], 1.0 / 256.0)

        # channel max of xb*gate via a cross-partition gpsimd reduce -- the
        # result lands directly in pixel-major layout, skipping the PE
        # transposes and one DRAM staging hop
        xcj = wk.tile([128, 2, 768], f16, name="xcj")
        for j in range(2):
            nc.vector.tensor_scalar_mul(xcj[:, j, :], xb[:, j, :],
                                        gate[:, j, 0:1])
        cmx = wk.tile([128, 768], f32, name="cmx")
        cmx2 = wk.tile([128, 768], f32, name="cmx2")
        nc.gpsimd.partition_all_reduce(cmx[:, :], xcj[:, 0, :], 128,
                                       bass_isa.ReduceOp.max)
        nc.gpsimd.partition_all_reduce(cmx2[:, :], xcj[:, 1, :], 128,
                                       bass_isa.ReduceOp.max)
        comp_row = wk.tile([1, 768], f16, name="comp_row")
        nc.vector.tensor_max(comp_row[0:1, :], cmx[0:1, :], cmx2[0:1, :])
        nc.vector.tensor_mul(
            comp_sp[0:1, :].rearrange("o (r c) -> o r c", c=38)[:, :, 2:36],
            comp_row[0:1, 0:748].rearrange("o (r c) -> o r c", c=34),
            mask_mean[0:1, 0:748].rearrange("o (r c) -> o r c", c=34))

        # channel mean of xb*gate via gate-weighted ones-matmul; the mean
        # half of comp then flows through its DRAM hops on the Pool queue
        # while the max half (slower DVE path) catches up on the SP queue.
        il = wk.tile([14, 16, 38], f16, name="il")
        mean_sb = wk.tile([1, 748], f16, name="mean_sb")
        psW2_cm = tc.tile_pool(name="psW2", bufs=1, space="PSUM")
        psW2 = psW2_cm.__enter__()
        with tc.tile_pool(name="psM", bufs=1, space="PSUM") as psM:
            pm1 = psM.tile([1, 748], f32, name="pm1")
            for j in range(2):
                for (o0, nn) in ((0, 512), (512, 236)):
                    nc.tensor.matmul(pm1[0:1, o0: o0 + nn],
                                     gate_sc[:, j, :],
                                     xb[:, j, o0: o0 + nn],
                                     start=(j == 0), stop=(j == 1))
            nc.scalar.copy(mean_sb[:, :], pm1[:, :])
        nc.vector.tensor_mul(
            mean_sp[0:1, :].rearrange("o (r c) -> o r c", c=38)[:, :, 2:36],
            mean_sb[0:1, :].rearrange("o (r c) -> o r c", c=34),
            mask_mean[0:1, 0:748].rearrange("o (r c) -> o r c", c=34))
        # keep the PE clock ramped until the spatial-conv operands land
        spin_pe(psW2, 40)

        # tracked guard-reads on the issuing queues order the untracked
        # (manual-AP) im2col gathers after the DVE mask-mul writes
        guard_d = dram.tile([2, CROWS * 38], f16)
        nc.gpsimd.dma_start(guard_d[1:2, :], mean_sp[0:1, :])
        nc.gpsimd.dma_start(
            il[7:14, :, :],
            bass.AP(mean_sp.tensor, 0, [[1, 1], [38, 7], [38, 16], [1, 38]]))
        nc.sync.dma_start(guard_d[0:1, :], comp_sp[0:1, :])
        nc.sync.dma_start(
            il[0:7, :, :],
            bass.AP(comp_sp.tensor, 0, [[1, 1], [38, 7], [38, 16], [1, 38]]))
        sig_row = wk.tile([1, 512], f16, name="sig_row")
        sigb = wk.tile([128, 16, 32], f16, name="sigb")
        with tc.tile_pool(name="psS", bufs=1, space="PSUM") as psS:
            pss = psS.tile([1, 512], f32, name="pss")
            for dx in range(7):
                nc.tensor.matmul(pss[:, :], spw_r[:, dx: dx + 1],
                                 il[:, :, dx: dx + 32],
                                 start=(dx == 0), stop=(dx == 6))
            # sigmoid via 1/(1+exp(-z)) on the exp table (z = bn affine)
            se = wk.tile([1, 512], f32, name="se")
            nc.scalar.activation(se[:, :], pss[:, :], AF.Exp,
                                 scale=bn_nsb[0:1, 0:1], bias=bn_nsb[0:1, 1:2])
            se1 = wk.tile([1, 512], f32, name="se1")
            nc.scalar.activation(se1[:, :], se[:, :], AF.Identity, bias=1.0)
            sgr = wk.tile([1, 512], f32, name="sgr")
            nc.vector.reciprocal(sgr[:, :], se1[:, :])
            nc.vector.tensor_copy(sig_row[:, :], sgr[:, :])
            # broadcast along partitions with a ones-matmul (no DRAM hop)
            sigb_ps = psS.tile([128, 512], f32, tag="sigbps", name="sigb_ps")
            nc.tensor.matmul(sigb_ps[:, :], ones1[0:1, :], sig_row[0:1, :],
                             start=True, stop=True)
            nc.scalar.copy(sigb.rearrange("p r c -> p (r c)"), sigb_ps[:, :])
        psW2_cm.__exit__(None, None, None)

        if stage == "dbg":
            nc.sync.dma_start(prm["dbg_sig"][:], sig_row[:])
            nc.sync.dma_start(prm["dbg_il"][:], il[:])
            nc.sync.dma_start(prm["dbg_cmp"][:], comp_row[:])

        # fused = xb_own + (xb_own * gate) * sigb
        fused = apool.tile([128, 2, 16, 32], f16)
        for j in range(2):
            xc_own = wk.tile([128, 16, 32], f16, tag="xc_own", name="xc_own")
            nc.scalar.mul(xc_own[:, :, :], xbv[j][:, 3:19, 1:33],
                          gate[:, j, 0:1])
            tm = wk.tile([128, 16, 32], f16, tag="tm", name="tm")
            nc.vector.tensor_mul(tm[:, :, :], xc_own[:, :, :], sigb[:, :, :])
            nc.vector.tensor_add(fused[:, j, :, :], xbv[j][:, 3:19, 1:33],
                                 tm[:, :, :])

        if stage == "dbg":
            nc.sync.dma_start(prm["dbg_fused"][:], fused[:])

        # bottleneck conv: accumulate the fused-input chunks now so the PE
        # works during the pools AllGather; priors chunks finish the group
        # after the collective.
        fbv = fused.rearrange("p j r c -> p j (r c)")
        psO_cm = tc.tile_pool(name="psO", bufs=2, space="PSUM")
        psO = psO_cm.__enter__()
        po_t = []
        for m in range(2):
            po = psO.tile([128, 512], f32, tag="po", name="po")
            po_t.append(po)
            for jj in range(2):
                nc.tensor.matmul(po[:, :],
                                 bott_wT[:, 2 + jj, m * 128: m * 128 + 128],
                                 fbv[:, jj, :],
                                 start=(jj == 0), stop=False,
                                 skip_group_check=True)

        # ---- PSP pools (raw block sums over own rows) ----
        pools = wk.tile([128, 2, 43], f32, name="pools")
        for j in range(2):
            f8 = fused[:, j].rearrange("p (rb ri) (cb ci) -> p rb cb ri ci",
                                       ri=4, ci=4)
            p8v = pools[:, j, 11:43].rearrange("p (rb cb) -> p rb cb", cb=8)
            nc.vector.tensor_reduce(p8v, f8, AX.XY, ALU.add)
            p8i = pools[:, j, 11:43].rearrange(
                "p (rb ri cb ci) -> p rb cb ri ci", rb=2, ri=2, cb=4, ci=2)
            p4v = pools[:, j, 3:11].rearrange("p (rb cb) -> p rb cb", cb=4)
            nc.vector.tensor_reduce(p4v, p8i, AX.XY, ALU.add)
        p4i = pools[:, :, 3:11].rearrange(
            "p j (rb cb ci) -> p j cb rb ci", rb=2, cb=2, ci=2)
        nc.vector.tensor_reduce(
            pools[:, :, 1:3].rearrange("p j (a k) -> p j a k", a=2, k=1),
            p4i, AX.XY, ALU.add)
        nc.vector.tensor_reduce(pools[:, :, 0:1], pools[:, :, 1:3], AX.X,
                                ALU.add)

        pools16 = wk.tile([128, 2, 43], f16, name="pools16")
        nc.vector.tensor_copy(pools16[:, :, :], pools[:, :, :])
        pools_d = dram.tile([2, 128, 43], f16)
        pools_o = dram.tile([2, 2, 128, 43], f16)
        nc.sync.dma_start(pools_d.rearrange("j p k -> p j k"),
                          pools16[:, :, :])
        nc.gpsimd.collective_compute(
            "AllGather", ALU.bypass, replica_groups=PAIRS,
            ins=[pools_d[:].opt()], outs=[pools_o[:].opt()])
        # 1x1 convs on pools in TRANSPOSED layout (out partition = pool
        # block, free = psp channel). The OWN half comes straight from
        # pools16 in SBUF and runs DURING the AllGather; only the partner
        # half waits for the collective. Wup's slot dim is indexed
        # dynamically (own = rank-in-pair, partner = the other).
        SI = {1: 0, 2: 1, 4: 2, 8: 3}
        pdT_a8 = [wk.tile([32, 64], f16, tag=f"pdT_a8{s}", name=f"pdT_a8{s}")
                  for s in range(2)]
        pdT_s1 = wk.tile([1, 64], f16, name="pdT_s1")
        pdT_b = [wk.tile([8, 64], f16, tag=f"pdT_b{s}", name=f"pdT_b{s}")
                 for s in range(2)]
        pdT_c = [wk.tile([2, 64], f16, tag=f"pdT_c{s}", name=f"pdT_c{s}")
                 for s in range(2)]
        pri = [wk.tile([128, 512], f16, tag=f"pri{i}", name=f"pri{i}")
               for i in range(2)]
        ones_f = wk.tile([128, 512], f16, name="ones_f")
        nc.vector.memset(ones_f[:], 1.0)
        psP_cm = tc.tile_pool(name="psP", bufs=1, space="PSUM")
        psP = psP_cm.__enter__()
        psR_cm = tc.tile_pool(name="psR", bufs=1, space="PSUM")
        psR = psR_cm.__enter__()
        pdm_ps = psP.tile([65, 64], f32, name="pdm_ps")
        pd4_ps = psP.tile([40, 64], f32, name="pd4_ps")
        pd2_ps = psP.tile([34, 64], f32, name="pd2_ps")
        pp0 = psR.tile([128, 512], f32, tag="pp0", name="pp0")
        pp1 = psR.tile([128, 512], f32, tag="pp1", name="pp1")

        def pd_matmuls(src, sl):
            # src[j] -> [128, 43] pool partials for this half (j = ch chunk)
            for j in range(2):
                nc.tensor.matmul(pdm_ps[32 * sl: 32 * sl + 32, :],
                                 src(j, 11, 43), psp_wT[:, j, SI[8], :],
                                 start=(j == 0), stop=(j == 1),
                                 skip_group_check=True)
            for j in range(2):
                nc.tensor.matmul(pdm_ps[64:65, :],
                                 src(j, 0, 1), psp_wT[:, j, SI[1], :],
                                 start=(sl == 0 and j == 0),
                                 stop=(sl == 1 and j == 1),
                                 skip_group_check=True)
            for j in range(2):
                nc.tensor.matmul(pd4_ps[32 * sl: 32 * sl + 8, :],
                                 src(j, 3, 11), psp_wT[:, j, SI[4], :],
                                 start=(j == 0), stop=(j == 1),
                                 skip_group_check=True)
            for j in range(2):
                nc.tensor.matmul(pd2_ps[32 * sl: 32 * sl + 2, :],
                                 src(j, 1, 3), psp_wT[:, j, SI[2], :],
                                 start=(j == 0), stop=(j == 1),
                                 skip_group_check=True)

        def pd_copies(sl):
            nc.scalar.copy(pdT_a8[sl][:, :], pdm_ps[32 * sl: 32 * sl + 32, :])
            nc.vector.tensor_copy(pdT_b[sl][:, :],
                                  pd4_ps[32 * sl: 32 * sl + 8, :])
            nc.vector.tensor_copy(pdT_c[sl][:, :],
                                  pd2_ps[32 * sl: 32 * sl + 2, :])

        def upsample(sl, slot_idx):
            nc.tensor.matmul(pp0[64:128, :], pdT_c[sl][:, :],
                             Wup[0:2, bass.ds(slot_idx, 1), 0, :],
                             start=(sl == 0), stop=(sl == 1),
                             tile_position=(0, 64), skip_group_check=True)
            nc.tensor.matmul(pp1[0:64, :], pdT_b[sl][:, :],
                             Wup[0:8, bass.ds(slot_idx, 1), 1, :],
                             start=(sl == 0), stop=(sl == 1),
                             tile_position=(0, 0), skip_group_check=True)
            nc.tensor.matmul(pp1[64:128, :], pdT_a8[sl][:, :],
                             Wup[0:32, bass.ds(slot_idx, 1), 2, :],
                             start=(sl == 0), stop=(sl == 1),
                             tile_position=(0, 64), skip_group_check=True)

        psW3_cm = tc.tile_pool(name="psW3", bufs=1, space="PSUM")
        psW3 = psW3_cm.__enter__()
        # own half: runs while the collective is in flight
        pd_matmuls(lambda j, a, b: pools16[:, j, a:b], 0)
        pd_copies(0)
        upsample(0, pid2)
        # keep the PE clock ramped through the pools AllGather
        spin_pe(psW3, 88)
        # partner half: after the collective
        slp_p = wk.tile([128, 2, 43], f16, name="slp_p")
        nc.sync.dma_start(
            slp_p[:, :, :],
            pools_o[bass.ds(omh, 1)].rearrange("s j p k -> s p j k"))
        pd_matmuls(lambda j, a, b: slp_p[:, j, a:b], 1)
        pd_copies(1)
        nc.scalar.copy(pdT_s1[:, :], pdm_ps[64:65, :])
        upsample(1, omh)
        nc.tensor.matmul(pp0[0:64, :], pdT_s1[:, :], ones_f[0:1, :],
                         start=True, stop=True, skip_group_check=True)
        nc.scalar.copy(pri[0][:, :], pp0[:, :])
        nc.scalar.copy(pri[1][:, :], pp1[:, :])
        psW3_cm.__exit__(None, None, None)
        psR_cm.__exit__(None, None, None)
        psP_cm.__exit__(None, None, None)

        if stage == "dbg":
            nc.sync.dma_start(prm["dbg_pd"][0:32], pdT_a8[0][:])
            nc.sync.dma_start(prm["dbg_pd"][32:64], pdT_a8[1][:])
            nc.sync.dma_start(prm["dbg_pd"][64:65], pdT_s1[:])

        if stage == "dbg":
            nc.sync.dma_start(prm["dbg_pri0"][:], pri[0][:])
            nc.sync.dma_start(prm["dbg_pri1"][:], pri[1][:])

        out_sb = wk.tile([128, 2, 512], f32, name="out_sb")
        for m in range(2):
            po = po_t[m]
            for k in range(2):
                nc.tensor.matmul(po[:, :],
                                 bott_wT[:, k, m * 128: m * 128 + 128],
                                 pri[k][:, :],
                                 start=False, stop=(k == 1),
                                 skip_group_check=True)
            nc.scalar.activation(out_sb[:, m, :], po[:, :], AF.Relu,
                                 bias=bott_b[:, m: m + 1])
        psO_cm.__exit__(None, None, None)
        nc.sync.dma_start(out_prm[:],
                          out_sb.rearrange("p j (r c) -> p j r c", c=32))


# ---------------------------------------------------------------------------
# Runner
# ---------------------------------------------------------------------------

_CACHE = {}


def _get_nc(stage="full"):
    if stage not in _CACHE:
        _CACHE[stage] = build(stage)
    return _CACHE[stage]


def run_cores(inputs, stage="full"):
    nc = _get_nc(stage)
    in_maps = [prep_core_inputs(inputs, c) for c in range(N_CORES)]
    res = run_bass_kernel_spmd(nc, in_maps, list(range(N_CORES)))
    return res.results


def kernel(**inputs):
    results = run_cores(inputs, "full")
    out = np.zeros((B, 1, COUT, H, W), np.float32)
    for c in range(N_CORES):
        b, h = c // 2, c % 2
        o = results[c]["out"]                    # [128, 2, 16, 32]
        out[b, 0, :, 16 * h: 16 * h + 16, :] = (
            o.transpose(1, 0, 2, 3).reshape(COUT, 16, 32))
    return out



# revision 104
# speedup vs baseline: 1.0089x; 1.0027x over previous
"""Trainium2 Bass kernel for nn_BottleneckFusion (STCN memory readout + ResBlock
+ CBAM + PSP + bottleneck), 8-core SPMD.

Sharding: core c -> (batch b = c//2, half h = c%2).
  Phase A (attention): TM split across the pair (4 memory frames each);
    software-pipelined affinity/exp/value loop; flash-style combine of
    (unnormalized value, sumexp) via a pairwise AllGather TRIMMED to the
    21-image-row window the partner actually needs (dynamic partition-id
    driven send/receive offsets).
  Phase B (convs/CBAM/PSP): row-half split with halo recompute. The
    batch-query half of the ResBlock conv accumulation is emitted before the
    value AllGather so it fills the collective window; PE-warming spin
    matmuls keep the tensor clock ramped through the other collectives.
    CBAM sigmoids use 1/(1+exp(-z)) so only the exp act table is ever
    loaded; the spatial-gate channel max uses a gpsimd cross-partition
    reduce (no transposes); PSP 1x1 convs are emitted directly in
    transposed (block-major) layout with the own-half computed from local
    pools during the pools AllGather and dynamic Wup slot slices.

kernel(**inputs) takes the FULL unsharded inputs and returns the FULL output.
"""
import sys

sys.path.insert(0, "/opt/trn_rl_repo")

import numpy as np
import ml_dtypes

import concourse.bass as bass
import concourse.bacc as bacc
import concourse.bass_isa as bass_isa
import concourse.mybir as mybir
import concourse.tile as tile
from concourse.bass_utils import run_bass_kernel_spmd

BF16 = ml_dtypes.bfloat16
F16 = np.float16
bf = mybir.dt.bfloat16
f16 = mybir.dt.float16
f32 = mybir.dt.float32
AF = mybir.ActivationFunctionType
ALU = mybir.AluOpType
AX = mybir.AxisListType

N_CORES = 8
B, TM, CIN, CK, CV, COUT, H, W = 4, 8, 256, 64, 256, 256, 32, 32
EPS = 1e-5

# local row coordinates: l = image_row - (r0 - 5), l in 0..25
XROWS = 26                 # x window rows (image r0-5 .. r0+20)
CROWS = 22                 # xb/xc/comp local rows (image r0-3 .. r0+18)
PIXPAD = 768               # padded xc free size (22*34=748 -> 768)
PAIRS = [[0, 1], [2, 3], [4, 5], [6, 7]]
UPS = (2, 4, 8)            # upsampled PSP scales
# full pool pyramid offsets [s1, s2, s4, s8] and own-partial offsets
FOFF = {1: 0, 2: 1, 4: 5, 8: 21}
POFF = {1: 0, 2: 1, 4: 3, 8: 11}


def interp_matrix(s_in, s_out=32):
    if s_in == 1:
        return np.ones((s_out, 1), np.float32)
    c = np.arange(s_out) * (s_in - 1) / (s_out - 1)
    lo = np.floor(c).astype(np.int64)
    hi = np.minimum(lo + 1, s_in - 1)
    w = (c - lo).astype(np.float32)
    M = np.zeros((s_out, s_in), np.float32)
    M[np.arange(s_out), lo] += 1.0 - w
    M[np.arange(s_out), hi] += w
    return M


# ---------------------------------------------------------------------------
# Host-side input preparation
# ---------------------------------------------------------------------------

def _pad_hw(a):
    out = np.zeros(a.shape[:-2] + (34, 34), a.dtype)
    out[..., 1:33, 1:33] = a
    return out


def _chw_chunks(a):
    """[256, ...] -> [128, 2, ...] (partition, chunk)."""
    return a.reshape(2, 128, *a.shape[1:]).transpose(
        1, 0, *range(2, a.ndim + 1))


def prep_core_inputs(inputs, core):
    b, h = core // 2, core % 2
    r0 = 16 * h
    g = {}

    f16_q = np.asarray(inputs["f16_q"], np.float32)
    f16_m = np.asarray(inputs["f16_m"], np.float32)
    value_m = np.asarray(inputs["value_m"], np.float32)

    # xm: [128, 2, 4, 34, 34] padded memory frames
    src = f16_m[b, 4 * h: 4 * h + 4]                        # [4, 256, 32, 32]
    src = src.reshape(4, 2, 128, 32, 32).transpose(2, 1, 0, 3, 4)
    g["xm"] = _pad_hw(src).astype(F16)

    # xq: [128, 2, 34, 34] padded query
    q = _chw_chunks(f16_q[b, 0])                            # [128, 2, 32, 32]
    g["xq"] = _pad_hw(q).astype(F16)

    # vT: [128, 32, 256] transposed value
    V = value_m[b][:, 4 * h: 4 * h + 4].reshape(CV, 4096)
    g["vT"] = np.ascontiguousarray(
        V.T.reshape(32, 128, CV).transpose(1, 0, 2)).astype(BF16)  # stays bf16 (matches e)

    # x window q-part: [128, 2, 26, 34]
    qw = np.zeros((128, 2, XROWS, 34), np.float32)
    for l in range(XROWS):
        img = r0 - 5 + l
        if 0 <= img <= 31:
            qw[:, :, l, 1:33] = q[:, :, img, :]
    g["xqb_raw"] = qw.astype(F16)
    g["xqb_relu"] = np.maximum(qw, 0.0).astype(F16)

    pk_w = np.asarray(inputs["pk_w"], np.float32)
    g["pk_wT"] = np.ascontiguousarray(
        pk_w.reshape(CK, 2, 128, 3, 3).transpose(2, 1, 3, 4, 0)).astype(F16)
    pk_b = np.asarray(inputs["pk_b"], np.float32)
    g["pkb2"] = np.concatenate([pk_b, pk_b]).reshape(128, 1).astype(np.float32)

    def conv_lhsT(w, kc):
        co = w.shape[0]
        return np.ascontiguousarray(
            w.reshape(co, kc, 128, 3, 3).transpose(2, 1, 3, 4, 0)).astype(F16)

    g["rb1_wT"] = conv_lhsT(np.asarray(inputs["rb1_w"], np.float32), 4)
    g["rb2_wT"] = conv_lhsT(np.asarray(inputs["rb2_w"], np.float32), 2)
    g["rbd_wT"] = conv_lhsT(np.asarray(inputs["rbd_w"], np.float32), 4)
    g["rb1_b"] = np.asarray(inputs["rb1_b"], np.float32).reshape(2, 128).T.copy()
    g["xb_bias"] = (np.asarray(inputs["rb2_b"], np.float32)
                    + np.asarray(inputs["rbd_b"], np.float32)
                    ).reshape(2, 128).T.copy()

    w1 = np.asarray(inputs["mlp_w1"], np.float32)           # [16, 256]
    g["mlp_w1T"] = np.ascontiguousarray(
        w1.reshape(16, 2, 128).transpose(2, 1, 0)).copy()   # [128, 2, 16]
    g["mlp_b1"] = np.asarray(inputs["mlp_b1"], np.float32).reshape(16, 1).copy()
    g["mlp_w2T"] = np.ascontiguousarray(
        np.asarray(inputs["mlp_w2"], np.float32).T).astype(F16)  # [16, 256]
    # sigmoid is computed as 1/(1+exp(-z)) on the exp table, so biases are
    # stored pre-negated for the exp activation
    g["mlp_b2x2n"] = (-2.0 * np.asarray(inputs["mlp_b2"], np.float32)
                      ).reshape(2, 128).T.copy()
    g["mlp_b2row"] = (2.0 * np.asarray(inputs["mlp_b2"], np.float32)
                      ).reshape(1, 256).astype(F16)

    spw = np.asarray(inputs["sp_w"], np.float32)[0]       # [2, 7, 7]
    g["spw_r"] = np.ascontiguousarray(
        spw.reshape(14, 7)).astype(np.float16)                # [(ch,dy), dx]
    bn_scale = float(np.asarray(inputs["sp_g"], np.float32)[0]) / float(
        np.sqrt(1.0 + EPS))
    bn_bias = float(np.asarray(inputs["sp_b"], np.float32)[0])
    g["bn_nsb"] = np.array([[-bn_scale, -bn_bias]], np.float32)

    maskT = np.zeros((128, 6, 1), np.float16)
    mask_mean = np.zeros((1, 768), np.float16)
    for pix in range(CROWS * 34):
        img = r0 - 3 + pix // 34
        if 0 <= img <= 31:
            maskT[pix % 128, pix // 128, 0] = 1.0
            mask_mean[0, pix] = 1.0
    g["comp_maskT"] = maskT
    g["mask_mean"] = mask_mean

    pw = np.zeros((128, 2, 4, 64), np.float32)
    for si, s in enumerate((1, 2, 4, 8)):
        wc = np.asarray(inputs[f"psp_w{s}"], np.float32)[:, :, 0, 0]
        scale = 1.0 / ((32 // s) ** 2)
        pw[:, :, si, :] = (wc.T * scale).reshape(2, 128, 64).transpose(1, 0, 2)
    g["psp_wT"] = pw.astype(F16)

    # folded upsample operators, split per AG slot so every matmul operand
    # sits at partition base 0: Wup[k_local, sl, si, (r*32+c)] with the
    # global block k = jr*s+jc split as sl = k // (s*s/2), k_local = k % ..
    Wup = np.zeros((32, 2, 3, 512), np.float32)
    for si, s in enumerate(UPS):
        M = interp_matrix(s)
        Mrr = M[r0: r0 + 16, :]                 # [16, s]
        half = s * s // 2
        for jr in range(s):
            for jc in range(s):
                k = jr * s + jc
                Wup[k % half, k // half, si, :] = np.outer(
                    Mrr[:, jr], M[:, jc]).reshape(512)
    g["Wup"] = Wup.astype(F16)

    bott_w = np.asarray(inputs["bott_w"], np.float32)[:, :, 0, 0]
    g["bott_wT"] = np.ascontiguousarray(
        bott_w.reshape(COUT, 4, 128).transpose(2, 1, 0)).astype(F16)
    g["bott_b"] = np.asarray(inputs["bott_b"], np.float32).reshape(2, 128).T.copy()

    rmask = np.zeros((1, XROWS, 34), np.float16)
    for l in range(XROWS):
        if 0 <= r0 - 5 + l <= 31:
            rmask[0, l, :] = 1.0
    g["rmask"] = rmask

    g["ident"] = np.eye(128, dtype=F16)
    return g


INPUT_SPECS = [
    ("xm", [128, 2, 4, 34, 34], f16),
    ("xq", [128, 2, 34, 34], f16),
    ("vT", [128, 32, 256], bf),
    ("xqb_raw", [128, 2, XROWS, 34], f16),
    ("xqb_relu", [128, 2, XROWS, 34], f16),
    ("pk_wT", [128, 2, 3, 3, 64], f16),
    ("pkb2", [128, 1], f32),
    ("rb1_wT", [128, 4, 3, 3, 256], f16),
    ("rb2_wT", [128, 2, 3, 3, 256], f16),
    ("rbd_wT", [128, 4, 3, 3, 256], f16),
    ("rb1_b", [128, 2], f32),
    ("xb_bias", [128, 2], f32),
    ("mlp_w1T", [128, 2, 16], f32),
    ("mlp_b1", [16, 1], f32),
    ("mlp_w2T", [16, 256], f16),
    ("mlp_b2x2n", [128, 2], f32),
    ("mlp_b2row", [1, 256], f16),
    ("spw_r", [14, 7], f16),
    ("bn_nsb", [1, 2], f32),
    ("comp_maskT", [128, 6, 1], f16),
    ("mask_mean", [1, 768], f16),
    ("psp_wT", [128, 2, 4, 64], f16),
    ("Wup", [32, 2, 3, 512], f16),
    ("bott_wT", [128, 4, 256], f16),
    ("bott_b", [128, 2], f32),
    ("ident", [128, 128], f16),
    ("rmask", [1, XROWS, 34], f16),
]


# ---------------------------------------------------------------------------
# Device kernel
# ---------------------------------------------------------------------------

def build(stage="full"):
    nc = bacc.Bacc("TRN2", target_bir_lowering=False, debug=False,
                   num_devices=N_CORES)
    prm = {n: nc.declare_dram_parameter(n, sh, dt, isOutput=False)
           for n, sh, dt in INPUT_SPECS}
    if stage == "A":
        out_prm = nc.declare_dram_parameter("out_a", [257, 1024], f32,
                                            isOutput=True)
    else:
        out_prm = nc.declare_dram_parameter("out", [128, 2, 16, 32], f32,
                                            isOutput=True)
    if stage == "dbg":
        for n, sh, dt in [("dbg_xraw", [128, 4, XROWS, 34], f16),
                          ("dbg_xb", [128, 2, CROWS, 34], f16),
                          ("dbg_gate", [128, 2, 1], f32),
                          ("dbg_sig", [1, 512], f16),
                          ("dbg_fused", [128, 2, 16, 32], f16),
                          ("dbg_pd", [65, 64], f16),
                          ("dbg_il", [14, 16, 38], f16),
                          ("dbg_cmp", [1, 768], f16),
                          ("dbg_pri0", [128, 512], f16),
                          ("dbg_pri1", [128, 512], f16)]:
            prm[n] = nc.declare_dram_parameter(n, sh, dt, isOutput=True)
    with tile.TileContext(nc) as tc:
        _emit(tc, nc, prm, stage, out_prm)
    nc.compile()
    return nc


def _emit(tc, nc, prm, stage, out_prm):
    import contextlib
    es = contextlib.ExitStack()
    with es:
        wpool = es.enter_context(tc.tile_pool(name="wpool", bufs=1))
        apool = es.enter_context(tc.tile_pool(name="apool", bufs=1))
        dram = es.enter_context(tc.tile_pool(name="dram", bufs=1, space="DRAM"))
        aonly_cm = tc.tile_pool(name="aonly", bufs=1)
        aonly = aonly_cm.__enter__()

        def load(name, pool=wpool):
            t = pool.tile(list(prm[name].shape), prm[name].dtype,
                          name=f"{name}_sb")
            nc.sync.dma_start(t[:], prm[name][:])
            return t

        pk_wT = load("pk_wT")
        pkb2 = load("pkb2")
        xm_sb = aonly.tile([128, 2, 4, 34, 34], f16, name="xm_sb")
        # frames 0/1 rows 0:19 land first so the mk conv starts early
        for t in range(2):
            nc.sync.dma_start(xm_sb[:, :, t, 0:19, :],
                              prm["xm"][:, :, t, 0:19, :])
        for t in range(2):
            nc.sync.dma_start(xm_sb[:, :, t, 19:34, :],
                              prm["xm"][:, :, t, 19:34, :])
        for t in range(2, 4):
            nc.sync.dma_start(xm_sb[:, :, t, :, :], prm["xm"][:, :, t, :, :])
        xq_sb = load("xq", aonly)
        vT_sb = load("vT", aonly)

        ones_bf = wpool.tile([128, 1], bf)
        nc.vector.memset(ones_bf[:], 1.0)
        # spin sources: tiny constant operands for PE-warming matmuls that
        # keep the tensor clock ramped through collective windows
        spin_w = wpool.tile([1, 1], f16)
        nc.vector.memset(spin_w[:], 0.0)
        spin_src = wpool.tile([1, 512], f16)
        nc.vector.memset(spin_src[:], 0.0)
        zero128 = wpool.tile([1, 128], f16)
        nc.vector.memset(zero128[:], 0.0)

        def spin_pe(pool, n, rows=512):
            for _ in range(n):
                sp = pool.tile([1, rows], f32, tag="spin", name="sp")
                nc.tensor.matmul(sp[0:1, :], spin_w[0:1, 0:1],
                                 spin_src[0:1, 0:rows],
                                 start=True, stop=True,
                                 skip_group_check=True)

        # ================= phase A =================
        mk_sb = aonly.tile([128, 2, 1024], f16)
        qk_sb = aonly.tile([128, 1024], f16)

        with tc.tile_pool(name="psA", bufs=2, space="PSUM") as psA:
            for tp in range(2):
                for n in range(2):
                    pm = psA.tile([128, 512], f32, tag="mkps", name="pm")
                    for par in range(2):
                        t = 2 * tp + par
                        k = 0
                        for j in range(2):
                            for dy in range(3):
                                for dx in range(3):
                                    nc.tensor.matmul(
                                        pm[64 * par: 64 * par + 64, :],
                                        pk_wT[:, j, dy, dx, :],
                                        xm_sb[:, j, t,
                                              n * 16 + dy: n * 16 + dy + 16,
                                              dx: dx + 32],
                                        start=(k == 0), stop=(k == 17),
                                        tile_position=(0, 64 * par),
                                    )
                                    k += 1
                    nc.scalar.activation(
                        mk_sb[:, tp, n * 512: (n + 1) * 512], pm[:, :],
                        AF.Identity, bias=pkb2[:, 0:1])

            for n in range(2):
                pq = psA.tile([64, 512], f32, tag="qkps", name="pq")
                k = 0
                for j in range(2):
                    for dy in range(3):
                        for dx in range(3):
                            nc.tensor.matmul(
                                pq[:, :], pk_wT[:, j, dy, dx, :],
                                xq_sb[:, j, n * 16 + dy: n * 16 + dy + 16,
                                      dx: dx + 32],
                                start=(k == 0), stop=(k == 17))
                            k += 1
                nc.scalar.activation(
                    qk_sb[0:64, n * 512: (n + 1) * 512], pq[:, :],
                    AF.Identity, bias=pkb2[0:64, 0:1])
            # replicate qk to partitions 64..127 so odd-frame mk slices
            # (base partition 64) can stream against it
            nc.sync.dma_start(qk_sb[64:128, :], qk_sb[0:64, :])

        # pair exchange buffers: full [257,1024] for the debug stage, a
        # 21-image-row window (the part the partner actually needs) otherwise
        if stage == "A":
            arv = dram.tile([257, 1024], bf)
            arvg = dram.tile([2, 257, 1024], bf)
        else:
            arv2 = dram.tile([257, 672], bf)
            arvg2 = dram.tile([2, 257, 672], bf)
        pid = nc.partition_id()
        pid2 = pid % 2
        omh = (pid + 1) % 2
        sendoff = omh * 352
        myoff = pid2 * 352
        vstart160 = omh * 160

        with (
            tc.tile_pool(name="psAff", bufs=2, space="PSUM") as psAff,
            tc.tile_pool(name="psV", bufs=1, space="PSUM") as psV,
        ):
            vps = [psV.tile([128, 1024], f32, name=f"vps{j}") for j in range(2)]
            s_acc = aonly.tile([128, 1024], bf, name="s_acc")

            order = [16 * h + o + 8 * par for h in range(2) for o in range(8)
                     for par in range(2)]

            def lhs_aff(i):
                t = i >> 3
                pb = i & 7
                tp, par = t >> 1, t & 1
                return par, mk_sb[64 * par: 64 * par + 64, tp,
                                  pb * 128: pb * 128 + 128]

            # software-pipelined: affinity matmuls + exp run one chunk ahead
            # of the value accumulation so the PE never waits on the exp.
            e_tiles = {}

            def emit_aff(idx):
                i = order[idx]
                par, lhs = lhs_aff(i)
                e_t = aonly.tile([128, 1024], bf, tag="e", name="e_t", bufs=4)
                pa = psAff.tile([128, 1024], f32, tag="affp", name="pa")
                for qn in range(2):
                    nc.tensor.matmul(
                        pa[:, qn * 512: (qn + 1) * 512], lhs,
                        qk_sb[64 * par: 64 * par + 64,
                              qn * 512: (qn + 1) * 512],
                        start=True, stop=True)
                nc.scalar.activation(e_t[:, :], pa[:, :], AF.Exp, scale=0.125)
                e_tiles[idx] = e_t

            emit_aff(0)
            for idx in range(32):
                if idx + 1 < 32:
                    emit_aff(idx + 1)
                i = order[idx]
                e_t = e_tiles.pop(idx)
                for j in range(2):
                    for qn in range(2):
                        nc.tensor.matmul(
                            vps[j][:, qn * 512: (qn + 1) * 512],
                            vT_sb[:, i, j * 128: (j + 1) * 128],
                            e_t[:, qn * 512: (qn + 1) * 512],
                            start=(idx == 0), stop=(idx == 31),
                            skip_group_check=True)
                if idx == 0:
                    nc.vector.tensor_copy(s_acc[:, :], e_t[:, :])
                else:
                    nc.vector.tensor_add(s_acc[:, :], s_acc[:, :], e_t[:, :])

            v_sb = apool.tile([128, 2, 1024], bf, name="v_sb")
            s_sb = apool.tile([1, 1024], bf, name="s_sb")
            # one PSUM->SBUF copy on DVE, one on Act so they run concurrently
            nc.vector.tensor_copy(v_sb[:, 0, :], vps[0][:, :])
            nc.scalar.copy(v_sb[:, 1, :], vps[1][:, :])
            if stage == "A":
                for j in range(2):
                    nc.sync.dma_start(arv[128 * j: 128 * j + 128, :],
                                      v_sb[:, j, :])
            else:
                nc.sync.dma_start(
                    arv2[0:256, :].rearrange("(j p) w -> p j w", j=2),
                    v_sb[:, :, bass.ds(sendoff, 672)])
            # fold the 128-partition sumexp accumulator with a ones matmul
            for qn in range(2):
                sfold = psAff.tile([1, 512], f32, tag="affp", name="sfold")
                nc.tensor.matmul(sfold[0:1, :],
                                 ones_bf[:, 0:1],
                                 s_acc[:, qn * 512: (qn + 1) * 512],
                                 start=True, stop=True)
                nc.vector.tensor_copy(s_sb[:, qn * 512: (qn + 1) * 512],
                                      sfold[0:1, :])
            if stage == "A":
                nc.sync.dma_start(arv[256:257, :], s_sb[:, :])
            else:
                nc.sync.dma_start(arv2[256:257, :],
                                  s_sb[0:1, bass.ds(sendoff, 672)])

        if stage == "A":
            nc.gpsimd.collective_compute(
                "AllGather", ALU.bypass, replica_groups=PAIRS,
                ins=[arv[:].opt()], outs=[arvg[:].opt()])
        else:
            nc.gpsimd.collective_compute(
                "AllGather", ALU.bypass, replica_groups=PAIRS,
                ins=[arv2[:].opt()], outs=[arvg2[:].opt()])


        aonly_cm.__exit__(None, None, None)

        if stage == "A":
            with tc.tile_pool(name="cmb", bufs=1) as cmb:
                cs0 = cmb.tile([1, 1024], bf, name="cs0")
                cs1 = cmb.tile([1, 1024], bf, name="cs1")
                cso = cmb.tile([1, 1024], f32, name="cso")
                nc.sync.dma_start(cs0[:], arvg[0, 256:257, :])
                nc.sync.dma_start(cs1[:], arvg[1, 256:257, :])
                nc.vector.tensor_add(cso[:, :], cs0[:, :], cs1[:, :])
                nc.sync.dma_start(out_prm[256:257, :], cso[:, :])
                for j in range(2):
                    ca = cmb.tile([128, 1024], bf, tag="ca", name="ca")
                    cb = cmb.tile([128, 1024], bf, tag="cb", name="cb")
                    co = cmb.tile([128, 1024], f32, tag="co", name="co")
                    nc.sync.dma_start(ca[:, :], arvg[0, 128 * j: 128 * j + 128, :])
                    nc.sync.dma_start(cb[:, :], arvg[1, 128 * j: 128 * j + 128, :])
                    nc.vector.tensor_add(co[:, :], ca[:, :], cb[:, :])
                    nc.sync.dma_start(out_prm[128 * j: 128 * j + 128, :],
                                      co[:, :])
            return

        # ================= phase B =================
        wk = es.enter_context(tc.tile_pool(name="wk", bufs=1))
        rb1_wT = load("rb1_wT")
        rb2_wT = load("rb2_wT")
        rbd_wT = load("rbd_wT")
        rb1_b = load("rb1_b")
        xb_bias = load("xb_bias")
        mlp_w1T = load("mlp_w1T")
        mlp_b1 = load("mlp_b1")
        mlp_w2T = load("mlp_w2T")
        mlp_b2x2n = load("mlp_b2x2n")
        spw_r = load("spw_r")
        bn_nsb = load("bn_nsb")
        mask_mean = load("mask_mean")
        psp_wT = load("psp_wT")
        Wup = load("Wup")
        bott_wT = load("bott_wT")
        bott_b = load("bott_b")
        ident = load("ident")

        r0v = (nc.vector.partition_id() % 2) * 16

        # ---- val-independent prep: x tiles, query-side loads, masks ----
        x_raw = apool.tile([128, 4, XROWS, 34], f16)
        x_relu = apool.tile([128, 4, XROWS, 34], f16)
        for tt in (x_raw, x_relu):
            nc.vector.memset(tt[:, 2:4, :, 0:1], 0.0)
            nc.vector.memset(tt[:, 2:4, :, 33:34], 0.0)
        nc.sync.dma_start(x_raw[:, 0:2, :, :], prm["xqb_raw"][:])
        nc.sync.dma_start(x_relu[:, 0:2, :, :], prm["xqb_relu"][:])

        r1_relu = apool.tile([128, 2, XROWS, 34], f16)
        nc.vector.memset(r1_relu[:, :, 0:1, :], 0.0)
        nc.vector.memset(r1_relu[:, :, 25:26, :], 0.0)
        nc.vector.memset(r1_relu[:, :, :, 0:1], 0.0)
        nc.vector.memset(r1_relu[:, :, :, 33:34], 0.0)
        rmaskb = apool.tile([128, XROWS, 34], f16)
        nc.sync.dma_start(rmaskb[:], prm["rmask"][:].partition_broadcast(128))
        xb = apool.tile([128, 2, PIXPAD], f16)
        xbv = [xb[:, j, 0: CROWS * 34].rearrange("p (r c) -> p r c", c=34)
               for j in range(2)]
        for j in range(2):
            nc.vector.memset(xbv[j][:, :, 0:1], 0.0)
            nc.vector.memset(xbv[j][:, :, 33:34], 0.0)
        nc.vector.memset(xb[:, :, CROWS * 34:], 0.0)

        # val window tiles in x-window coordinates (26 rows = XROWS); the
        # 21-row valid band sits at dynamic row offset 5*(1-h). Pad rows are
        # zeroed statically (both possible pad bands); the valid-band writes
        # land after and overwrite any overlap.
        val_pad = apool.tile([128, 2, 832], f32)
        nc.vector.memset(val_pad[:, :, 0:160], 0.0)
        nc.vector.memset(val_pad[:, :, 672:832], 0.0)
        # x val-part pad bands (rows outside the 21-row valid window) are
        # zeroed statically; the valid band is written at a dynamic offset
        for tt in (x_raw, x_relu):
            nc.vector.memset(tt[:, 2:4, 0:5, 1:33], 0.0)
            nc.vector.memset(tt[:, 2:4, 21:26, 1:33], 0.0)

        # ---- query-side conv accumulation: fills the PE while the val
        # AllGather is in flight (j=0,1 of x are batch-query channels) ----
        psR1_cm = tc.tile_pool(name="psR1", bufs=1, space="PSUM")
        psR1 = psR1_cm.__enter__()
        psXB_cm = tc.tile_pool(name="psXB", bufs=1, space="PSUM")
        psXB = psXB_cm.__enter__()
        psW1_cm = tc.tile_pool(name="psW1", bufs=1, space="PSUM")
        psW1 = psW1_cm.__enter__()
        R1G = ((0, 1, 16), (0, 17, 8), (1, 1, 16), (1, 17, 8))
        XBG = ((0, 2, 16), (0, 18, 6), (1, 2, 16), (1, 18, 6))
        # the two short row-groups per producer share one PSUM bank
        # (independent column ranges) so all 8 accumulators fit in 6 banks
        pr_t = {}
        px_t = {}
        r1sm = psR1.tile([128, 512], f32, tag="r1s", name="r1sm")
        xbsm = psXB.tile([128, 384], f32, tag="xbs", name="xbsm")
        # a start=True matmul resets the whole PSUM bank, so shared banks are
        # zeroed once up front and every accumulation into them avoids start
        nc.tensor.matmul(r1sm[:, :], zero128[0:1, :], spin_src[0:1, 0:512],
                         start=True, stop=False, skip_group_check=True)
        nc.tensor.matmul(xbsm[:, :], zero128[0:1, :], spin_src[0:1, 0:384],
                         start=True, stop=False, skip_group_check=True)
        for m in range(2):
            pr_t[(m, 1)] = psR1.tile([128, 512], f32, tag=f"r1b{m}",
                                     name="prb")
            pr_t[(m, 17)] = r1sm[:, m * 256: m * 256 + 256]
            px_t[(m, 2)] = psXB.tile([128, 512], f32, tag=f"xbb{m}",
                                     name="pxb")
            px_t[(m, 18)] = xbsm[:, m * 192: m * 192 + 192]
        for (m, l0, nr) in R1G:
            pr = pr_t[(m, l0)]
            k = 0
            for j in range(2):
                for dy in range(3):
                    for dx in range(3):
                        nc.tensor.matmul(
                            pr[:, : nr * 32],
                            rb1_wT[:, j, dy, dx, m * 128: m * 128 + 128],
                            x_relu[:, j, l0 + dy - 1: l0 + dy - 1 + nr,
                                   dx: dx + 32],
                            start=(k == 0 and nr == 16), stop=False,
                            skip_group_check=True)
                        k += 1
        for (m, l0, nr) in XBG:
            px = px_t[(m, l0)]
            k = 0
            for j in range(2):
                for dy in range(3):
                    for dx in range(3):
                        nc.tensor.matmul(
                            px[:, : nr * 32],
                            rbd_wT[:, j, dy, dx, m * 128: m * 128 + 128],
                            x_raw[:, j, l0 + dy - 1: l0 + dy - 1 + nr,
                                  dx: dx + 32],
                            start=(k == 0 and nr == 16), stop=False,
                            skip_group_check=True)
                        k += 1
        # keep the PE clock ramped through the rest of the AllGather window
        spin_pe(psW1, 48)

        # ---- val: own window (SBUF) + partner window (AG slot), normalize,
        # window into x ----
        vs_p = wk.tile([128, 2, 672], bf, name="vs_p")
        nc.sync.dma_start(
            vs_p[:, :, :],
            arvg2[bass.ds(omh, 1), 0:256, :].rearrange(
                "s (j p) w -> s p j w", j=2))
        sp_row = wk.tile([1, 672], bf, name="sp_row")
        nc.gpsimd.dma_start(sp_row[:, :], arvg2[bass.ds(omh, 1), 256:257, :])
        for j in range(2):
            nc.vector.tensor_add(val_pad[:, j, bass.ds(vstart160, 672)],
                                 v_sb[:, j, bass.ds(myoff, 672)],
                                 vs_p[:, j, :])
        s_row = wk.tile([1, 672], f32, name="s_row")
        nc.vector.tensor_add(s_row[:, :], s_sb[0:1, bass.ds(myoff, 672)],
                             sp_row[:, :])
        inv_row = wk.tile([1, 672], f32, name="inv_row")
        nc.vector.reciprocal(inv_row[:, :], s_row[:, :])
        inv_d = dram.tile([1, 672], f32)
        nc.sync.dma_start(inv_d[:], inv_row[:, :])
        inv_b = wk.tile([128, 21, 32], f32, name="inv_b")
        nc.sync.dma_start(inv_b.rearrange("p r c -> p (r c)"),
                          inv_d.partition_broadcast(128))
        vp_v = val_pad.rearrange("p j (r c) -> p j r c", c=32)
        omh5 = omh * 5
        for j in range(2):
            nc.vector.tensor_mul(x_raw[:, 2 + j, bass.ds(omh5, 21), 1:33],
                                 vp_v[:, j, bass.ds(omh5, 21), :],
                                 inv_b[:, :, :])
            nc.scalar.activation(x_relu[:, 2 + j, :, 1:33],
                                 x_raw[:, 2 + j, :, 1:33], AF.Relu)

        # ---- val-side conv accumulation + activations ----
        for (m, l0, nr) in R1G:
            pr = pr_t[(m, l0)]
            k = 0
            for j in (2, 3):
                for dy in range(3):
                    for dx in range(3):
                        nc.tensor.matmul(
                            pr[:, : nr * 32],
                            rb1_wT[:, j, dy, dx, m * 128: m * 128 + 128],
                            x_relu[:, j, l0 + dy - 1: l0 + dy - 1 + nr,
                                   dx: dx + 32],
                            start=False, stop=(k == 17),
                            skip_group_check=True)
                        k += 1
            nc.scalar.activation(
                r1_relu[:, m, l0: l0 + nr, 1:33], pr[:, : nr * 32],
                AF.Relu, bias=rb1_b[:, m: m + 1])
            nc.vector.tensor_mul(r1_relu[:, m, l0: l0 + nr, 1:33],
                                 r1_relu[:, m, l0: l0 + nr, 1:33],
                                 rmaskb[:, l0: l0 + nr, 1:33])
        for (m, l0, nr) in XBG:
            px = px_t[(m, l0)]
            k = 0
            for j in (2, 3):
                for dy in range(3):
                    for dx in range(3):
                        nc.tensor.matmul(
                            px[:, : nr * 32],
                            rbd_wT[:, j, dy, dx, m * 128: m * 128 + 128],
                            x_raw[:, j, l0 + dy - 1: l0 + dy - 1 + nr,
                                  dx: dx + 32],
                            start=False, stop=False,
                            skip_group_check=True)
                        k += 1
            for j in range(2):
                for dy in range(3):
                    for dx in range(3):
                        nc.tensor.matmul(
                            px[:, : nr * 32],
                            rb2_wT[:, j, dy, dx, m * 128: m * 128 + 128],
                            r1_relu[:, j, l0 + dy - 1: l0 + dy - 1 + nr,
                                    dx: dx + 32],
                            start=False, stop=(k == 35),
                            skip_group_check=True)
                        k += 1
            nc.scalar.activation(
                xbv[m][:, l0 - 2: l0 - 2 + nr, 1:33], px[:, : nr * 32],
                AF.Identity, bias=xb_bias[:, m: m + 1])
        psW1_cm.__exit__(None, None, None)
        psXB_cm.__exit__(None, None, None)
        psR1_cm.__exit__(None, None, None)

        if stage == "dbg":
            nc.sync.dma_start(prm["dbg_xraw"][:], x_raw[:])
            for j in range(2):
                nc.sync.dma_start(prm["dbg_xb"][:, j], xbv[j])

        # ---- CBAM channel gate ----
        stats = wk.tile([128, 2, 2], f32, name="stats")
        for j in range(2):
            nc.vector.tensor_reduce(stats[:, j, 0:1], xbv[j][:, 3:19, 1:33],
                                    AX.XY, ALU.add)
            nc.vector.tensor_reduce(stats[:, j, 1:2], xbv[j][:, 3:19, 1:33],
                                    AX.XY, ALU.max)
        stats_d = dram.tile([256, 2], f32)
        stats_o = dram.tile([2, 256, 2], f32)
        nc.sync.dma_start(stats_d.rearrange("(j p) k -> p j k", j=2),
                          stats[:, :, :])
        # zeroed 38-stride comp rows in SBUF (borders give the conv halo);
        # the spatial-conv im2col then gathers straight from SBUF
        comp_sp = wk.tile([1, CROWS * 38], f16, name="comp_sp")
        mean_sp = wk.tile([1, CROWS * 38], f16, name="mean_sp")
        nc.vector.memset(comp_sp[:], 0.0)
        nc.vector.memset(mean_sp[:], 0.0)
        nc.gpsimd.collective_compute(
            "AllGather", ALU.bypass, replica_groups=PAIRS,
            ins=[stats_d[:].opt()], outs=[stats_o[:].opt()])
        slb = wk.tile([128, 2, 2, 2], f32, name="slb")  # [p, slot, j, stat]
        nc.sync.dma_start(slb[:, :, :, :],
                          stats_o.rearrange("s (j p) k -> p s j k", j=2))
        gate_in = wk.tile([128, 2, 2], f32, name="gate_in")
        tsum = wk.tile([128, 2, 1], f32, name="tsum")
        nc.vector.tensor_add(tsum[:, :, :], slb[:, 0, :, 0:1],
                             slb[:, 1, :, 0:1])
        nc.scalar.mul(gate_in[:, :, 0:1], tsum[:, :, :], 1.0 / 1024.0)
        nc.vector.tensor_max(gate_in[:, :, 1:2], slb[:, 0, :, 1:2],
                             slb[:, 1, :, 1:2])

        gate = wk.tile([128, 2, 1], f32, name="gate")
        ones1 = wk.tile([1, 128], f16, name="ones1")
        nc.vector.memset(ones1[:], 1.0)
        with tc.tile_pool(name="psG", bufs=1, space="PSUM") as psG:
            ph1 = psG.tile([16, 2], f32, name="ph1")
            for j in range(2):
                nc.tensor.matmul(ph1[:, :], mlp_w1T[:, j, :], gate_in[:, j, :],
                                 start=(j == 0), stop=(j == 1))
            h1 = wk.tile([16, 2], f16, name="h1")
            nc.scalar.activation(h1[:, :], ph1[:, :], AF.Relu,
                                 bias=mlp_b1[:, 0:1])
            # per-partition gate (sigmoid via the already-loaded exp table)
            for j in range(2):
                ph2 = psG.tile([128, 2], f32, tag="ph2", name="ph2")
                nc.tensor.matmul(ph2[:, :], mlp_w2T[:, j * 128: j * 128 + 128],
                                 h1[:, :], start=True, stop=True)
                h2 = wk.tile([128, 2], f32, tag="h2", name="h2")
                nc.vector.tensor_copy(h2[:, :], ph2[:, :])
                t2 = wk.tile([128, 1], f32, tag="t2", name="t2")
                nc.vector.tensor_add(t2[:, :], h2[:, 0:1], h2[:, 1:2])
                ev = wk.tile([128, 1], f32, tag="ev", name="ev")
                nc.scalar.activation(ev[:, :], t2[:, :], AF.Exp, scale=-1.0,
                                     bias=mlp_b2x2n[:, j: j + 1])
                e1 = wk.tile([128, 1], f32, tag="e1", name="e1")
                nc.scalar.activation(e1[:, :], ev[:, :], AF.Identity,
                                     bias=1.0)
                nc.vector.reciprocal(gate[:, j, :], e1[:, :])

        if stage == "dbg":
            nc.sync.dma_start(prm["dbg_gate"][:], gate[:])

        gate_sc = wk.tile([128, 2, 1], f16, name="gate_sc")
        nc.scalar.mul(gate_sc[:, :, :], gate[:, :, :], 1.0 / 256.0)

        # channel max of xb*gate via a cross-partition gpsimd reduce -- the
        # result lands directly in pixel-major layout, skipping the PE
        # transposes and one DRAM staging hop
        xcj = wk.tile([128, 2, 768], f16, name="xcj")
        for j in range(2):
            nc.vector.tensor_scalar_mul(xcj[:, j, :], xb[:, j, :],
                                        gate[:, j, 0:1])
        cmx = wk.tile([128, 768], f32, name="cmx")
        cmx2 = wk.tile([128, 768], f32, name="cmx2")
        nc.gpsimd.partition_all_reduce(cmx[:, :], xcj[:, 0, :], 128,
                                       bass_isa.ReduceOp.max)
        nc.gpsimd.partition_all_reduce(cmx2[:, :], xcj[:, 1, :], 128,
                                       bass_isa.ReduceOp.max)
        comp_row = wk.tile([1, 768], f16, name="comp_row")
        nc.vector.tensor_max(comp_row[0:1, :], cmx[0:1, :], cmx2[0:1, :])
        nc.vector.tensor_mul(
            comp_sp[0:1, :].rearrange("o (r c) -> o r c", c=38)[:, :, 2:36],
            comp_row[0:1, 0:748].rearrange("o (r c) -> o r c", c=34),
            mask_mean[0:1, 0:748].rearrange("o (r c) -> o r c", c=34))

        # channel mean of xb*gate via gate-weighted ones-matmul; the mean
        # half of comp then flows through its DRAM hops on the Pool queue
        # while the max half (slower DVE path) catches up on the SP queue.
        il = wk.tile([14, 16, 38], f16, name="il")
        mean_sb = wk.tile([1, 748], f16, name="mean_sb")
        psW2_cm = tc.tile_pool(name="psW2", bufs=1, space="PSUM")
        psW2 = psW2_cm.__enter__()
        with tc.tile_pool(name="psM", bufs=1, space="PSUM") as psM:
            pm1 = psM.tile([1, 748], f32, name="pm1")
            for j in range(2):
                for (o0, nn) in ((0, 512), (512, 236)):
                    nc.tensor.matmul(pm1[0:1, o0: o0 + nn],
                                     gate_sc[:, j, :],
                                     xb[:, j, o0: o0 + nn],
                                     start=(j == 0), stop=(j == 1))
            nc.scalar.copy(mean_sb[:, :], pm1[:, :])
        nc.vector.tensor_mul(
            mean_sp[0:1, :].rearrange("o (r c) -> o r c", c=38)[:, :, 2:36],
            mean_sb[0:1, :].rearrange("o (r c) -> o r c", c=34),
            mask_mean[0:1, 0:748].rearrange("o (r c) -> o r c", c=34))
        # keep the PE clock ramped until the spatial-conv operands land
        spin_pe(psW2, 40)

        # tracked guard-reads on the issuing queues order the untracked
        # (manual-AP) im2col gathers after the DVE mask-mul writes
        guard_d = dram.tile([2, CROWS * 38], f16)
        nc.gpsimd.dma_start(guard_d[1:2, :], mean_sp[0:1, :])
        nc.gpsimd.dma_start(
            il[7:14, :, :],
            bass.AP(mean_sp.tensor, 0, [[1, 1], [38, 7], [38, 16], [1, 38]]))
        nc.sync.dma_start(guard_d[0:1, :], comp_sp[0:1, :])
        nc.sync.dma_start(
            il[0:7, :, :],
            bass.AP(comp_sp.tensor, 0, [[1, 1], [38, 7], [38, 16], [1, 38]]))
        sig_row = wk.tile([1, 512], f16, name="sig_row")
        sigb = wk.tile([128, 16, 32], f16, name="sigb")
        with tc.tile_pool(name="psS", bufs=1, space="PSUM") as psS:
            pss = psS.tile([1, 512], f32, name="pss")
            for dx in range(7):
                nc.tensor.matmul(pss[:, :], spw_r[:, dx: dx + 1],
                                 il[:, :, dx: dx + 32],
                                 start=(dx == 0), stop=(dx == 6))
            # sigmoid via 1/(1+exp(-z)) on the exp table (z = bn affine)
            se = wk.tile([1, 512], f32, name="se")
            nc.scalar.activation(se[:, :], pss[:, :], AF.Exp,
                                 scale=bn_nsb[0:1, 0:1], bias=bn_nsb[0:1, 1:2])
            se1 = wk.tile([1, 512], f32, name="se1")
            nc.scalar.activation(se1[:, :], se[:, :], AF.Identity, bias=1.0)
            sgr = wk.tile([1, 512], f32, name="sgr")
            nc.vector.reciprocal(sgr[:, :], se1[:, :])
            nc.vector.tensor_copy(sig_row[:, :], sgr[:, :])
            # broadcast along partitions with a ones-matmul (no DRAM hop)
            sigb_ps = psS.tile([128, 512], f32, tag="sigbps", name="sigb_ps")
            nc.tensor.matmul(sigb_ps[:, :], ones1[0:1, :], sig_row[0:1, :],
                             start=True, stop=True)
            nc.scalar.copy(sigb.rearrange("p r c -> p (r c)"), sigb_ps[:, :])
        psW2_cm.__exit__(None, None, None)

        if stage == "dbg":
            nc.sync.dma_start(prm["dbg_sig"][:], sig_row[:])
            nc.sync.dma_start(prm["dbg_il"][:], il[:])
            nc.sync.dma_start(prm["dbg_cmp"][:], comp_row[:])

        # fused = xb_own + (xb_own * gate) * sigb
        fused = apool.tile([128, 2, 16, 32], f16)
        for j in range(2):
            xc_own = wk.tile([128, 16, 32], f16, tag="xc_own", name="xc_own")
            nc.scalar.mul(xc_own[:, :, :], xbv[j][:, 3:19, 1:33],
                          gate[:, j, 0:1])
            tm = wk.tile([128, 16, 32], f16, tag="tm", name="tm")
            nc.vector.tensor_mul(tm[:, :, :], xc_own[:, :, :], sigb[:, :, :])
            nc.vector.tensor_add(fused[:, j, :, :], xbv[j][:, 3:19, 1:33],
                                 tm[:, :, :])

        if stage == "dbg":
            nc.sync.dma_start(prm["dbg_fused"][:], fused[:])

        # bottleneck conv: accumulate the fused-input chunks now so the PE
        # works during the pools AllGather; priors chunks finish the group
        # after the collective.
        fbv = fused.rearrange("p j r c -> p j (r c)")
        psO_cm = tc.tile_pool(name="psO", bufs=2, space="PSUM")
        psO = psO_cm.__enter__()
        po_t = []
        for m in range(2):
            po = psO.tile([128, 512], f32, tag="po", name="po")
            po_t.append(po)
            for jj in range(2):
                nc.tensor.matmul(po[:, :],
                                 bott_wT[:, 2 + jj, m * 128: m * 128 + 128],
                                 fbv[:, jj, :],
                                 start=(jj == 0), stop=False,
                                 skip_group_check=True)

        # ---- PSP pools (raw block sums over own rows) ----
        pools = wk.tile([128, 2, 43], f32, name="pools")
        for j in range(2):
            f8 = fused[:, j].rearrange("p (rb ri) (cb ci) -> p rb cb ri ci",
                                       ri=4, ci=4)
            p8v = pools[:, j, 11:43].rearrange("p (rb cb) -> p rb cb", cb=8)
            nc.vector.tensor_reduce(p8v, f8, AX.XY, ALU.add)
            p8i = pools[:, j, 11:43].rearrange(
                "p (rb ri cb ci) -> p rb cb ri ci", rb=2, ri=2, cb=4, ci=2)
            p4v = pools[:, j, 3:11].rearrange("p (rb cb) -> p rb cb", cb=4)
            nc.vector.tensor_reduce(p4v, p8i, AX.XY, ALU.add)
        p4i = pools[:, :, 3:11].rearrange(
            "p j (rb cb ci) -> p j cb rb ci", rb=2, cb=2, ci=2)
        nc.vector.tensor_reduce(
            pools[:, :, 1:3].rearrange("p j (a k) -> p j a k", a=2, k=1),
            p4i, AX.XY, ALU.add)
        nc.vector.tensor_reduce(pools[:, :, 0:1], pools[:, :, 1:3], AX.X,
                                ALU.add)

        pools16 = wk.tile([128, 2, 43], f16, name="pools16")
        nc.vector.tensor_copy(pools16[:, :, :], pools[:, :, :])
        pools_d = dram.tile([2, 128, 43], f16)
        pools_o = dram.tile([2, 2, 128, 43], f16)
        nc.sync.dma_start(pools_d.rearrange("j p k -> p j k"),
                          pools16[:, :, :])
        nc.gpsimd.collective_compute(
            "AllGather", ALU.bypass, replica_groups=PAIRS,
            ins=[pools_d[:].opt()], outs=[pools_o[:].opt()])
        # 1x1 convs on pools in TRANSPOSED layout (out partition = pool
        # block, free = psp channel). The OWN half comes straight from
        # pools16 in SBUF and runs DURING the AllGather; only the partner
        # half waits for the collective. Wup's slot dim is indexed
        # dynamically (own = rank-in-pair, partner = the other).
        SI = {1: 0, 2: 1, 4: 2, 8: 3}
        pdT_a8 = [wk.tile([32, 64], f16, tag=f"pdT_a8{s}", name=f"pdT_a8{s}")
                  for s in range(2)]
        pdT_s1 = wk.tile([1, 64], f16, name="pdT_s1")
        pdT_b = [wk.tile([8, 64], f16, tag=f"pdT_b{s}", name=f"pdT_b{s}")
                 for s in range(2)]
        pdT_c = [wk.tile([2, 64], f16, tag=f"pdT_c{s}", name=f"pdT_c{s}")
                 for s in range(2)]
        pri = [wk.tile([128, 512], f16, tag=f"pri{i}", name=f"pri{i}")
               for i in range(2)]
        ones_f = wk.tile([128, 512], f16, name="ones_f")
        nc.vector.memset(ones_f[:], 1.0)
        psP_cm = tc.tile_pool(name="psP", bufs=1, space="PSUM")
        psP = psP_cm.__enter__()
        psR_cm = tc.tile_pool(name="psR", bufs=1, space="PSUM")
        psR = psR_cm.__enter__()
        pdm_ps = psP.tile([65, 64], f32, name="pdm_ps")
        pd4_ps = psP.tile([40, 64], f32, name="pd4_ps")
        pd2_ps = psP.tile([34, 64], f32, name="pd2_ps")
        pp0 = psR.tile([128, 512], f32, tag="pp0", name="pp0")
        pp1 = psR.tile([128, 512], f32, tag="pp1", name="pp1")

        def pd_matmuls(src, sl):
            # src[j] -> [128, 43] pool partials for this half (j = ch chunk)
            for j in range(2):
                nc.tensor.matmul(pdm_ps[32 * sl: 32 * sl + 32, :],
                                 src(j, 11, 43), psp_wT[:, j, SI[8], :],
                                 start=(j == 0), stop=(j == 1),
                                 skip_group_check=True)
            for j in range(2):
                nc.tensor.matmul(pdm_ps[64:65, :],
                                 src(j, 0, 1), psp_wT[:, j, SI[1], :],
                                 start=(sl == 0 and j == 0),
                                 stop=(sl == 1 and j == 1),
                                 skip_group_check=True)
            for j in range(2):
                nc.tensor.matmul(pd4_ps[32 * sl: 32 * sl + 8, :],
                                 src(j, 3, 11), psp_wT[:, j, SI[4], :],
                                 start=(j == 0), stop=(j == 1),
                                 skip_group_check=True)
            for j in range(2):
                nc.tensor.matmul(pd2_ps[32 * sl: 32 * sl + 2, :],
                                 src(j, 1, 3), psp_wT[:, j, SI[2], :],
                                 start=(j == 0), stop=(j == 1),
                                 skip_group_check=True)

        def pd_copies(sl):
            nc.scalar.copy(pdT_a8[sl][:, :], pdm_ps[32 * sl: 32 * sl + 32, :])
            nc.vector.tensor_copy(pdT_b[sl][:, :],
                                  pd4_ps[32 * sl: 32 * sl + 8, :])
            nc.vector.tensor_copy(pdT_c[sl][:, :],
                                  pd2_ps[32 * sl: 32 * sl + 2, :])

        def upsample(sl, slot_idx):
            nc.tensor.matmul(pp0[64:128, :], pdT_c[sl][:, :],
                             Wup[0:2, bass.ds(slot_idx, 1), 0, :],
                             start=(sl == 0), stop=(sl == 1),
                             tile_position=(0, 64), skip_group_check=True)
            nc.tensor.matmul(pp1[0:64, :], pdT_b[sl][:, :],
                             Wup[0:8, bass.ds(slot_idx, 1), 1, :],
                             start=(sl == 0), stop=(sl == 1),
                             tile_position=(0, 0), skip_group_check=True)
            nc.tensor.matmul(pp1[64:128, :], pdT_a8[sl][:, :],
                             Wup[0:32, bass.ds(slot_idx, 1), 2, :],
                             start=(sl == 0), stop=(sl == 1),
                             tile_position=(0, 64), skip_group_check=True)

        psW3_cm = tc.tile_pool(name="psW3", bufs=1, space="PSUM")
        psW3 = psW3_cm.__enter__()
        # own half: runs while the collective is in flight
        pd_matmuls(lambda j, a, b: pools16[:, j, a:b], 0)
        pd_copies(0)
        upsample(0, pid2)
        # keep the PE clock ramped through the pools AllGather
        spin_pe(psW3, 88)
        # partner half: after the collective
        slp_p = wk.tile([128, 2, 43], f16, name="slp_p")
        nc.sync.dma_start(
            slp_p[:, :, :],
            pools_o[bass.ds(omh, 1)].rearrange("s j p k -> s p j k"))
        pd_matmuls(lambda j, a, b: slp_p[:, j, a:b], 1)
        pd_copies(1)
        nc.scalar.copy(pdT_s1[:, :], pdm_ps[64:65, :])
        upsample(1, omh)
        nc.tensor.matmul(pp0[0:64, :], pdT_s1[:, :], ones_f[0:1, :],
                         start=True, stop=True, skip_group_check=True)
        nc.scalar.copy(pri[0][:, :], pp0[:, :])
        nc.scalar.copy(pri[1][:, :], pp1[:, :])
        psW3_cm.__exit__(None, None, None)
        psR_cm.__exit__(None, None, None)
        psP_cm.__exit__(None, None, None)

        if stage == "dbg":
            nc.sync.dma_start(prm["dbg_pd"][0:32], pdT_a8[0][:])
            nc.sync.dma_start(prm["dbg_pd"][32:64], pdT_a8[1][:])
            nc.sync.dma_start(prm["dbg_pd"][64:65], pdT_s1[:])

        if stage == "dbg":
            nc.sync.dma_start(prm["dbg_pri0"][:], pri[0][:])
            nc.sync.dma_start(prm["dbg_pri1"][:], pri[1][:])

        out_sb = wk.tile([128, 2, 512], f32, name="out_sb")
        for m in range(2):
            po = po_t[m]
            for k in range(2):
                nc.tensor.matmul(po[:, :],
                                 bott_wT[:, k, m * 128: m * 128 + 128],
                                 pri[k][:, :],
                                 start=False, stop=(k == 1),
                                 skip_group_check=True)
            nc.scalar.activation(out_sb[:, m, :], po[:, :], AF.Relu,
                                 bias=bott_b[:, m: m + 1])
            # per-half output DMA overlaps the other half's epilogue
            nc.sync.dma_start(
                out_prm[:, m: m + 1, :, :],
                out_sb[:, m: m + 1, :].rearrange("p j (r c) -> p j r c",
                                                 c=32))
        psO_cm.__exit__(None, None, None)


# ---------------------------------------------------------------------------
# Runner
# ---------------------------------------------------------------------------

_CACHE = {}


def _get_nc(stage="full"):
    if stage not in _CACHE:
        _CACHE[stage] = build(stage)
    return _CACHE[stage]


def run_cores(inputs, stage="full"):
    nc = _get_nc(stage)
    in_maps = [prep_core_inputs(inputs, c) for c in range(N_CORES)]
    res = run_bass_kernel_spmd(nc, in_maps, list(range(N_CORES)))
    return res.results


def kernel(**inputs):
    results = run_cores(inputs, "full")
    out = np.zeros((B, 1, COUT, H, W), np.float32)
    for c in range(N_CORES):
        b, h = c // 2, c % 2
        o = results[c]["out"]                    # [128, 2, 16, 32]
        out[b, 0, :, 16 * h: 16 * h + 16, :] = (
            o.transpose(1, 0, 2, 3).reshape(COUT, 16, 32))
    return out



# revision 106
# speedup vs baseline: 1.0139x; 1.0050x over previous
"""Trainium2 Bass kernel for nn_BottleneckFusion (STCN memory readout + ResBlock
+ CBAM + PSP + bottleneck), 8-core SPMD.

Sharding: core c -> (batch b = c//2, half h = c%2).
  Phase A (attention): TM split across the pair (4 memory frames each);
    software-pipelined affinity/exp/value loop; flash-style combine of
    (unnormalized value, sumexp) via a pairwise AllGather TRIMMED to the
    21-image-row window the partner actually needs (dynamic partition-id
    driven send/receive offsets).
  Phase B (convs/CBAM/PSP): row-half split with halo recompute. The
    batch-query half of the ResBlock conv accumulation is emitted before the
    value AllGather so it fills the collective window; PE-warming spin
    matmuls keep the tensor clock ramped through the other collectives.
    CBAM sigmoids use 1/(1+exp(-z)) so only the exp act table is ever
    loaded; the spatial-gate channel max uses a gpsimd cross-partition
    reduce (no transposes); PSP 1x1 convs are emitted directly in
    transposed (block-major) layout with the own-half computed from local
    pools during the pools AllGather and dynamic Wup slot slices.

kernel(**inputs) takes the FULL unsharded inputs and returns the FULL output.
"""
import sys

sys.path.insert(0, "/opt/trn_rl_repo")

import numpy as np
import ml_dtypes

import concourse.bass as bass
import concourse.bacc as bacc
import concourse.bass_isa as bass_isa
import concourse.mybir as mybir
import concourse.tile as tile
from concourse.bass_utils import run_bass_kernel_spmd

BF16 = ml_dtypes.bfloat16
F16 = np.float16
bf = mybir.dt.bfloat16
f16 = mybir.dt.float16
f32 = mybir.dt.float32
AF = mybir.ActivationFunctionType
ALU = mybir.AluOpType
AX = mybir.AxisListType

N_CORES = 8
B, TM, CIN, CK, CV, COUT, H, W = 4, 8, 256, 64, 256, 256, 32, 32
EPS = 1e-5

# local row coordinates: l = image_row - (r0 - 5), l in 0..25
XROWS = 26                 # x window rows (image r0-5 .. r0+20)
CROWS = 22                 # xb/xc/comp local rows (image r0-3 .. r0+18)
PIXPAD = 768               # padded xc free size (22*34=748 -> 768)
PAIRS = [[0, 1], [2, 3], [4, 5], [6, 7]]
UPS = (2, 4, 8)            # upsampled PSP scales
# full pool pyramid offsets [s1, s2, s4, s8] and own-partial offsets
FOFF = {1: 0, 2: 1, 4: 5, 8: 21}
POFF = {1: 0, 2: 1, 4: 3, 8: 11}


def interp_matrix(s_in, s_out=32):
    if s_in == 1:
        return np.ones((s_out, 1), np.float32)
    c = np.arange(s_out) * (s_in - 1) / (s_out - 1)
    lo = np.floor(c).astype(np.int64)
    hi = np.minimum(lo + 1, s_in - 1)
    w = (c - lo).astype(np.float32)
    M = np.zeros((s_out, s_in), np.float32)
    M[np.arange(s_out), lo] += 1.0 - w
    M[np.arange(s_out), hi] += w
    return M


# ---------------------------------------------------------------------------
# Host-side input preparation
# ---------------------------------------------------------------------------

def _pad_hw(a):
    out = np.zeros(a.shape[:-2] + (34, 34), a.dtype)
    out[..., 1:33, 1:33] = a
    return out


def _chw_chunks(a):
    """[256, ...] -> [128, 2, ...] (partition, chunk)."""
    return a.reshape(2, 128, *a.shape[1:]).transpose(
        1, 0, *range(2, a.ndim + 1))


def prep_core_inputs(inputs, core):
    b, h = core // 2, core % 2
    r0 = 16 * h
    g = {}

    f16_q = np.asarray(inputs["f16_q"], np.float32)
    f16_m = np.asarray(inputs["f16_m"], np.float32)
    value_m = np.asarray(inputs["value_m"], np.float32)

    # xm: [128, 2, 4, 34, 34] padded memory frames
    src = f16_m[b, 4 * h: 4 * h + 4]                        # [4, 256, 32, 32]
    src = src.reshape(4, 2, 128, 32, 32).transpose(2, 1, 0, 3, 4)
    g["xm"] = _pad_hw(src).astype(F16)

    # xq: [128, 2, 34, 34] padded query
    q = _chw_chunks(f16_q[b, 0])                            # [128, 2, 32, 32]
    g["xq"] = _pad_hw(q).astype(F16)

    # vT: [128, 32, 256] transposed value
    V = value_m[b][:, 4 * h: 4 * h + 4].reshape(CV, 4096)
    g["vT"] = np.ascontiguousarray(
        V.T.reshape(32, 128, CV).transpose(1, 0, 2)).astype(BF16)  # stays bf16 (matches e)

    # x window q-part: [128, 2, 26, 34]
    qw = np.zeros((128, 2, XROWS, 34), np.float32)
    for l in range(XROWS):
        img = r0 - 5 + l
        if 0 <= img <= 31:
            qw[:, :, l, 1:33] = q[:, :, img, :]
    g["xqb_raw"] = qw.astype(F16)
    g["xqb_relu"] = np.maximum(qw, 0.0).astype(F16)

    pk_w = np.asarray(inputs["pk_w"], np.float32)
    g["pk_wT"] = np.ascontiguousarray(
        pk_w.reshape(CK, 2, 128, 3, 3).transpose(2, 1, 3, 4, 0)).astype(F16)
    pk_b = np.asarray(inputs["pk_b"], np.float32)
    g["pkb2"] = np.concatenate([pk_b, pk_b]).reshape(128, 1).astype(np.float32)

    def conv_lhsT(w, kc):
        co = w.shape[0]
        return np.ascontiguousarray(
            w.reshape(co, kc, 128, 3, 3).transpose(2, 1, 3, 4, 0)).astype(F16)

    g["rb1_wT"] = conv_lhsT(np.asarray(inputs["rb1_w"], np.float32), 4)
    g["rb2_wT"] = conv_lhsT(np.asarray(inputs["rb2_w"], np.float32), 2)
    g["rbd_wT"] = conv_lhsT(np.asarray(inputs["rbd_w"], np.float32), 4)
    g["rb1_b"] = np.asarray(inputs["rb1_b"], np.float32).reshape(2, 128).T.copy()
    g["xb_bias"] = (np.asarray(inputs["rb2_b"], np.float32)
                    + np.asarray(inputs["rbd_b"], np.float32)
                    ).reshape(2, 128).T.copy()

    w1 = np.asarray(inputs["mlp_w1"], np.float32)           # [16, 256]
    g["mlp_w1T"] = np.ascontiguousarray(
        w1.reshape(16, 2, 128).transpose(2, 1, 0)).copy()   # [128, 2, 16]
    g["mlp_b1"] = np.asarray(inputs["mlp_b1"], np.float32).reshape(16, 1).copy()
    g["mlp_w2T"] = np.ascontiguousarray(
        np.asarray(inputs["mlp_w2"], np.float32).T).astype(F16)  # [16, 256]
    # sigmoid is computed as 1/(1+exp(-z)) on the exp table, so biases are
    # stored pre-negated for the exp activation
    g["mlp_b2x2n"] = (-2.0 * np.asarray(inputs["mlp_b2"], np.float32)
                      ).reshape(2, 128).T.copy()
    g["mlp_b2row"] = (2.0 * np.asarray(inputs["mlp_b2"], np.float32)
                      ).reshape(1, 256).astype(F16)

    spw = np.asarray(inputs["sp_w"], np.float32)[0]       # [2, 7, 7]
    g["spw_r"] = np.ascontiguousarray(
        spw.reshape(14, 7)).astype(np.float16)                # [(ch,dy), dx]
    bn_scale = float(np.asarray(inputs["sp_g"], np.float32)[0]) / float(
        np.sqrt(1.0 + EPS))
    bn_bias = float(np.asarray(inputs["sp_b"], np.float32)[0])
    g["bn_nsb"] = np.array([[-bn_scale, -bn_bias]], np.float32)

    maskT = np.zeros((128, 6, 1), np.float16)
    mask_mean = np.zeros((1, 768), np.float16)
    for pix in range(CROWS * 34):
        img = r0 - 3 + pix // 34
        if 0 <= img <= 31:
            maskT[pix % 128, pix // 128, 0] = 1.0
            mask_mean[0, pix] = 1.0
    g["comp_maskT"] = maskT
    g["mask_mean"] = mask_mean

    pw = np.zeros((128, 2, 4, 64), np.float32)
    for si, s in enumerate((1, 2, 4, 8)):
        wc = np.asarray(inputs[f"psp_w{s}"], np.float32)[:, :, 0, 0]
        scale = 1.0 / ((32 // s) ** 2)
        pw[:, :, si, :] = (wc.T * scale).reshape(2, 128, 64).transpose(1, 0, 2)
    g["psp_wT"] = pw.astype(F16)

    # folded upsample operators, split per AG slot so every matmul operand
    # sits at partition base 0: Wup[k_local, sl, si, (r*32+c)] with the
    # global block k = jr*s+jc split as sl = k // (s*s/2), k_local = k % ..
    Wup = np.zeros((32, 2, 3, 512), np.float32)
    for si, s in enumerate(UPS):
        M = interp_matrix(s)
        Mrr = M[r0: r0 + 16, :]                 # [16, s]
        half = s * s // 2
        for jr in range(s):
            for jc in range(s):
                k = jr * s + jc
                Wup[k % half, k // half, si, :] = np.outer(
                    Mrr[:, jr], M[:, jc]).reshape(512)
    g["Wup"] = Wup.astype(F16)

    bott_w = np.asarray(inputs["bott_w"], np.float32)[:, :, 0, 0]
    g["bott_wT"] = np.ascontiguousarray(
        bott_w.reshape(COUT, 4, 128).transpose(2, 1, 0)).astype(F16)
    g["bott_b"] = np.asarray(inputs["bott_b"], np.float32).reshape(2, 128).T.copy()

    rmask = np.zeros((1, XROWS, 34), np.float16)
    for l in range(XROWS):
        if 0 <= r0 - 5 + l <= 31:
            rmask[0, l, :] = 1.0
    g["rmask"] = rmask

    g["ident"] = np.eye(128, dtype=F16)
    return g


INPUT_SPECS = [
    ("xm", [128, 2, 4, 34, 34], f16),
    ("xq", [128, 2, 34, 34], f16),
    ("vT", [128, 32, 256], bf),
    ("xqb_raw", [128, 2, XROWS, 34], f16),
    ("xqb_relu", [128, 2, XROWS, 34], f16),
    ("pk_wT", [128, 2, 3, 3, 64], f16),
    ("pkb2", [128, 1], f32),
    ("rb1_wT", [128, 4, 3, 3, 256], f16),
    ("rb2_wT", [128, 2, 3, 3, 256], f16),
    ("rbd_wT", [128, 4, 3, 3, 256], f16),
    ("rb1_b", [128, 2], f32),
    ("xb_bias", [128, 2], f32),
    ("mlp_w1T", [128, 2, 16], f32),
    ("mlp_b1", [16, 1], f32),
    ("mlp_w2T", [16, 256], f16),
    ("mlp_b2x2n", [128, 2], f32),
    ("mlp_b2row", [1, 256], f16),
    ("spw_r", [14, 7], f16),
    ("bn_nsb", [1, 2], f32),
    ("comp_maskT", [128, 6, 1], f16),
    ("mask_mean", [1, 768], f16),
    ("psp_wT", [128, 2, 4, 64], f16),
    ("Wup", [32, 2, 3, 512], f16),
    ("bott_wT", [128, 4, 256], f16),
    ("bott_b", [128, 2], f32),
    ("ident", [128, 128], f16),
    ("rmask", [1, XROWS, 34], f16),
]


# ---------------------------------------------------------------------------
# Device kernel
# ---------------------------------------------------------------------------

def build(stage="full"):
    nc = bacc.Bacc("TRN2", target_bir_lowering=False, debug=False,
                   num_devices=N_CORES)
    prm = {n: nc.declare_dram_parameter(n, sh, dt, isOutput=False)
           for n, sh, dt in INPUT_SPECS}
    if stage == "A":
        out_prm = nc.declare_dram_parameter("out_a", [257, 1024], f32,
                                            isOutput=True)
    else:
        out_prm = nc.declare_dram_parameter("out", [128, 2, 16, 32], f32,
                                            isOutput=True)
    if stage == "dbg":
        for n, sh, dt in [("dbg_xraw", [128, 4, XROWS, 34], f16),
                          ("dbg_xb", [128, 2, CROWS, 34], f16),
                          ("dbg_gate", [128, 2, 1], f32),
                          ("dbg_sig", [1, 512], f16),
                          ("dbg_fused", [128, 2, 16, 32], f16),
                          ("dbg_pd", [65, 64], f16),
                          ("dbg_il", [14, 16, 38], f16),
                          ("dbg_cmp", [1, 768], f16),
                          ("dbg_pri0", [128, 512], f16),
                          ("dbg_pri1", [128, 512], f16)]:
            prm[n] = nc.declare_dram_parameter(n, sh, dt, isOutput=True)
    with tile.TileContext(nc) as tc:
        _emit(tc, nc, prm, stage, out_prm)
    nc.compile()
    return nc


def _emit(tc, nc, prm, stage, out_prm):
    import contextlib
    es = contextlib.ExitStack()
    with es:
        wpool = es.enter_context(tc.tile_pool(name="wpool", bufs=1))
        apool = es.enter_context(tc.tile_pool(name="apool", bufs=1))
        dram = es.enter_context(tc.tile_pool(name="dram", bufs=1, space="DRAM"))
        aonly_cm = tc.tile_pool(name="aonly", bufs=1)
        aonly = aonly_cm.__enter__()

        def load(name, pool=wpool):
            t = pool.tile(list(prm[name].shape), prm[name].dtype,
                          name=f"{name}_sb")
            nc.sync.dma_start(t[:], prm[name][:])
            return t

        pk_wT = load("pk_wT")
        pkb2 = load("pkb2")
        xm_sb = aonly.tile([128, 2, 4, 34, 34], f16, name="xm_sb")
        # frames 0/1 rows 0:19 land first so the mk conv starts early
        for t in range(2):
            nc.sync.dma_start(xm_sb[:, :, t, 0:19, :],
                              prm["xm"][:, :, t, 0:19, :])
        for t in range(2):
            nc.sync.dma_start(xm_sb[:, :, t, 19:34, :],
                              prm["xm"][:, :, t, 19:34, :])
        for t in range(2, 4):
            nc.sync.dma_start(xm_sb[:, :, t, :, :], prm["xm"][:, :, t, :, :])
        xq_sb = load("xq", aonly)
        vT_sb = load("vT", aonly)

        ones_bf = wpool.tile([128, 1], bf)
        nc.vector.memset(ones_bf[:], 1.0)
        # spin sources: tiny constant operands for PE-warming matmuls that
        # keep the tensor clock ramped through collective windows
        spin_w = wpool.tile([1, 1], f16)
        nc.vector.memset(spin_w[:], 0.0)
        spin_src = wpool.tile([1, 512], f16)
        nc.vector.memset(spin_src[:], 0.0)
        zero128 = wpool.tile([1, 128], f16)
        nc.vector.memset(zero128[:], 0.0)

        def spin_pe(pool, n, rows=512):
            for _ in range(n):
                sp = pool.tile([1, rows], f32, tag="spin", name="sp")
                nc.tensor.matmul(sp[0:1, :], spin_w[0:1, 0:1],
                                 spin_src[0:1, 0:rows],
                                 start=True, stop=True,
                                 skip_group_check=True)

        # ================= phase A =================
        mk_sb = aonly.tile([128, 2, 1024], f16)
        qk_sb = aonly.tile([128, 1024], f16)

        with tc.tile_pool(name="psA", bufs=2, space="PSUM") as psA:
            for tp in range(2):
                for n in range(2):
                    pm = psA.tile([128, 512], f32, tag="mkps", name="pm")
                    for par in range(2):
                        t = 2 * tp + par
                        k = 0
                        for j in range(2):
                            for dy in range(3):
                                for dx in range(3):
                                    nc.tensor.matmul(
                                        pm[64 * par: 64 * par + 64, :],
                                        pk_wT[:, j, dy, dx, :],
                                        xm_sb[:, j, t,
                                              n * 16 + dy: n * 16 + dy + 16,
                                              dx: dx + 32],
                                        start=(k == 0), stop=(k == 17),
                                        tile_position=(0, 64 * par),
                                    )
                                    k += 1
                    nc.scalar.activation(
                        mk_sb[:, tp, n * 512: (n + 1) * 512], pm[:, :],
                        AF.Identity, bias=pkb2[:, 0:1])

            for n in range(2):
                pq = psA.tile([64, 512], f32, tag="qkps", name="pq")
                k = 0
                for j in range(2):
                    for dy in range(3):
                        for dx in range(3):
                            nc.tensor.matmul(
                                pq[:, :], pk_wT[:, j, dy, dx, :],
                                xq_sb[:, j, n * 16 + dy: n * 16 + dy + 16,
                                      dx: dx + 32],
                                start=(k == 0), stop=(k == 17))
                            k += 1
                nc.scalar.activation(
                    qk_sb[0:64, n * 512: (n + 1) * 512], pq[:, :],
                    AF.Identity, bias=pkb2[0:64, 0:1])
            # replicate qk to partitions 64..127 so odd-frame mk slices
            # (base partition 64) can stream against it
            nc.sync.dma_start(qk_sb[64:128, :], qk_sb[0:64, :])

        # pair exchange buffers: full [257,1024] for the debug stage, a
        # 21-image-row window (the part the partner actually needs) otherwise
        if stage == "A":
            arv = dram.tile([257, 1024], bf)
            arvg = dram.tile([2, 257, 1024], bf)
        else:
            arv2 = dram.tile([257, 672], bf)
            arvg2 = dram.tile([2, 257, 672], bf)
        pid = nc.partition_id()
        pid2 = pid % 2
        omh = (pid + 1) % 2
        sendoff = omh * 352
        myoff = pid2 * 352
        vstart160 = omh * 160

        with (
            tc.tile_pool(name="psAff", bufs=2, space="PSUM") as psAff,
            tc.tile_pool(name="psV", bufs=1, space="PSUM") as psV,
        ):
            vps = [psV.tile([128, 1024], f32, name=f"vps{j}") for j in range(2)]
            s_acc = aonly.tile([128, 1024], bf, name="s_acc")

            order = [16 * h + o + 8 * par for h in range(2) for o in range(8)
                     for par in range(2)]

            def lhs_aff(i):
                t = i >> 3
                pb = i & 7
                tp, par = t >> 1, t & 1
                return par, mk_sb[64 * par: 64 * par + 64, tp,
                                  pb * 128: pb * 128 + 128]

            # software-pipelined: affinity matmuls + exp run one chunk ahead
            # of the value accumulation so the PE never waits on the exp.
            e_tiles = {}

            def emit_aff(idx):
                i = order[idx]
                par, lhs = lhs_aff(i)
                e_t = aonly.tile([128, 1024], bf, tag="e", name="e_t", bufs=4)
                pa = psAff.tile([128, 1024], f32, tag="affp", name="pa")
                for qn in range(2):
                    nc.tensor.matmul(
                        pa[:, qn * 512: (qn + 1) * 512], lhs,
                        qk_sb[64 * par: 64 * par + 64,
                              qn * 512: (qn + 1) * 512],
                        start=True, stop=True)
                nc.scalar.activation(e_t[:, :], pa[:, :], AF.Exp, scale=0.125)
                e_tiles[idx] = e_t

            emit_aff(0)
            for idx in range(32):
                if idx + 1 < 32:
                    emit_aff(idx + 1)
                i = order[idx]
                e_t = e_tiles.pop(idx)
                for j in range(2):
                    for qn in range(2):
                        nc.tensor.matmul(
                            vps[j][:, qn * 512: (qn + 1) * 512],
                            vT_sb[:, i, j * 128: (j + 1) * 128],
                            e_t[:, qn * 512: (qn + 1) * 512],
                            start=(idx == 0), stop=(idx == 31),
                            skip_group_check=True)
                if idx == 0:
                    nc.vector.tensor_copy(s_acc[:, :], e_t[:, :])
                else:
                    nc.vector.tensor_add(s_acc[:, :], s_acc[:, :], e_t[:, :])

            v_sb = apool.tile([128, 2, 1024], bf, name="v_sb")
            s_sb = apool.tile([1, 1024], bf, name="s_sb")
            # one PSUM->SBUF copy on DVE, one on Act so they run concurrently
            nc.vector.tensor_copy(v_sb[:, 0, :], vps[0][:, :])
            nc.scalar.copy(v_sb[:, 1, :], vps[1][:, :])
            if stage == "A":
                for j in range(2):
                    nc.sync.dma_start(arv[128 * j: 128 * j + 128, :],
                                      v_sb[:, j, :])
            else:
                nc.sync.dma_start(
                    arv2[0:256, :].rearrange("(j p) w -> p j w", j=2),
                    v_sb[:, :, bass.ds(sendoff, 672)])
            # fold the 128-partition sumexp accumulator with a ones matmul
            for qn in range(2):
                sfold = psAff.tile([1, 512], f32, tag="affp", name="sfold")
                nc.tensor.matmul(sfold[0:1, :],
                                 ones_bf[:, 0:1],
                                 s_acc[:, qn * 512: (qn + 1) * 512],
                                 start=True, stop=True)
                nc.vector.tensor_copy(s_sb[:, qn * 512: (qn + 1) * 512],
                                      sfold[0:1, :])
            if stage == "A":
                nc.sync.dma_start(arv[256:257, :], s_sb[:, :])
            else:
                nc.sync.dma_start(arv2[256:257, :],
                                  s_sb[0:1, bass.ds(sendoff, 672)])

        if stage == "A":
            nc.gpsimd.collective_compute(
                "AllGather", ALU.bypass, replica_groups=PAIRS,
                ins=[arv[:].opt()], outs=[arvg[:].opt()])
        else:
            nc.gpsimd.collective_compute(
                "AllGather", ALU.bypass, replica_groups=PAIRS,
                ins=[arv2[:].opt()], outs=[arvg2[:].opt()])


        aonly_cm.__exit__(None, None, None)

        if stage == "A":
            with tc.tile_pool(name="cmb", bufs=1) as cmb:
                cs0 = cmb.tile([1, 1024], bf, name="cs0")
                cs1 = cmb.tile([1, 1024], bf, name="cs1")
                cso = cmb.tile([1, 1024], f32, name="cso")
                nc.sync.dma_start(cs0[:], arvg[0, 256:257, :])
                nc.sync.dma_start(cs1[:], arvg[1, 256:257, :])
                nc.vector.tensor_add(cso[:, :], cs0[:, :], cs1[:, :])
                nc.sync.dma_start(out_prm[256:257, :], cso[:, :])
                for j in range(2):
                    ca = cmb.tile([128, 1024], bf, tag="ca", name="ca")
                    cb = cmb.tile([128, 1024], bf, tag="cb", name="cb")
                    co = cmb.tile([128, 1024], f32, tag="co", name="co")
                    nc.sync.dma_start(ca[:, :], arvg[0, 128 * j: 128 * j + 128, :])
                    nc.sync.dma_start(cb[:, :], arvg[1, 128 * j: 128 * j + 128, :])
                    nc.vector.tensor_add(co[:, :], ca[:, :], cb[:, :])
                    nc.sync.dma_start(out_prm[128 * j: 128 * j + 128, :],
                                      co[:, :])
            return

        # ================= phase B =================
        wk = es.enter_context(tc.tile_pool(name="wk", bufs=1))
        rb1_wT = load("rb1_wT")
        rb2_wT = load("rb2_wT")
        rbd_wT = load("rbd_wT")
        rb1_b = load("rb1_b")
        xb_bias = load("xb_bias")
        mlp_w1T = load("mlp_w1T")
        mlp_b1 = load("mlp_b1")
        mlp_w2T = load("mlp_w2T")
        mlp_b2x2n = load("mlp_b2x2n")
        spw_r = load("spw_r")
        bn_nsb = load("bn_nsb")
        mask_mean = load("mask_mean")
        psp_wT = load("psp_wT")
        Wup = load("Wup")
        bott_wT = load("bott_wT")
        bott_b = load("bott_b")
        ident = load("ident")

        r0v = (nc.vector.partition_id() % 2) * 16

        # ---- val-independent prep: x tiles, query-side loads, masks ----
        x_raw = apool.tile([128, 4, XROWS, 34], f16)
        x_relu = apool.tile([128, 4, XROWS, 34], f16)
        for tt in (x_raw, x_relu):
            nc.vector.memset(tt[:, 2:4, :, 0:1], 0.0)
            nc.vector.memset(tt[:, 2:4, :, 33:34], 0.0)
        nc.sync.dma_start(x_raw[:, 0:2, :, :], prm["xqb_raw"][:])
        nc.sync.dma_start(x_relu[:, 0:2, :, :], prm["xqb_relu"][:])

        r1_relu = apool.tile([128, 2, XROWS, 34], f16)
        nc.vector.memset(r1_relu[:, :, 0:1, :], 0.0)
        nc.vector.memset(r1_relu[:, :, 25:26, :], 0.0)
        nc.vector.memset(r1_relu[:, :, :, 0:1], 0.0)
        nc.vector.memset(r1_relu[:, :, :, 33:34], 0.0)
        rmaskb = apool.tile([128, XROWS, 34], f16)
        nc.sync.dma_start(rmaskb[:], prm["rmask"][:].partition_broadcast(128))
        xb = apool.tile([128, 2, PIXPAD], f16)
        xbv = [xb[:, j, 0: CROWS * 34].rearrange("p (r c) -> p r c", c=34)
               for j in range(2)]
        for j in range(2):
            nc.vector.memset(xbv[j][:, :, 0:1], 0.0)
            nc.vector.memset(xbv[j][:, :, 33:34], 0.0)
        nc.vector.memset(xb[:, :, CROWS * 34:], 0.0)

        # val window tiles in x-window coordinates (26 rows = XROWS); the
        # 21-row valid band sits at dynamic row offset 5*(1-h). Pad rows are
        # zeroed statically (both possible pad bands); the valid-band writes
        # land after and overwrite any overlap.
        val_pad = apool.tile([128, 2, 832], f32)
        nc.vector.memset(val_pad[:, :, 0:160], 0.0)
        nc.vector.memset(val_pad[:, :, 672:832], 0.0)
        # x val-part pad bands (rows outside the 21-row valid window) are
        # zeroed statically; the valid band is written at a dynamic offset
        for tt in (x_raw, x_relu):
            nc.vector.memset(tt[:, 2:4, 0:5, 1:33], 0.0)
            nc.vector.memset(tt[:, 2:4, 21:26, 1:33], 0.0)

        # ---- query-side conv accumulation: fills the PE while the val
        # AllGather is in flight (j=0,1 of x are batch-query channels) ----
        psR1_cm = tc.tile_pool(name="psR1", bufs=1, space="PSUM")
        psR1 = psR1_cm.__enter__()
        psXB_cm = tc.tile_pool(name="psXB", bufs=1, space="PSUM")
        psXB = psXB_cm.__enter__()
        psW1_cm = tc.tile_pool(name="psW1", bufs=1, space="PSUM")
        psW1 = psW1_cm.__enter__()
        R1G = ((0, 1, 16), (0, 17, 8), (1, 1, 16), (1, 17, 8))
        XBG = ((0, 2, 16), (0, 18, 6), (1, 2, 16), (1, 18, 6))
        # the two short row-groups per producer share one PSUM bank
        # (independent column ranges) so all 8 accumulators fit in 6 banks
        pr_t = {}
        px_t = {}
        r1sm = psR1.tile([128, 512], f32, tag="r1s", name="r1sm")
        xbsm = psXB.tile([128, 384], f32, tag="xbs", name="xbsm")
        # a start=True matmul resets the whole PSUM bank, so shared banks are
        # zeroed once up front and every accumulation into them avoids start
        nc.tensor.matmul(r1sm[:, :], zero128[0:1, :], spin_src[0:1, 0:512],
                         start=True, stop=False, skip_group_check=True)
        nc.tensor.matmul(xbsm[:, :], zero128[0:1, :], spin_src[0:1, 0:384],
                         start=True, stop=False, skip_group_check=True)
        for m in range(2):
            pr_t[(m, 1)] = psR1.tile([128, 512], f32, tag=f"r1b{m}",
                                     name="prb")
            pr_t[(m, 17)] = r1sm[:, m * 256: m * 256 + 256]
            px_t[(m, 2)] = psXB.tile([128, 512], f32, tag=f"xbb{m}",
                                     name="pxb")
            px_t[(m, 18)] = xbsm[:, m * 192: m * 192 + 192]
        for (m, l0, nr) in R1G:
            pr = pr_t[(m, l0)]
            k = 0
            for j in range(2):
                for dy in range(3):
                    for dx in range(3):
                        nc.tensor.matmul(
                            pr[:, : nr * 32],
                            rb1_wT[:, j, dy, dx, m * 128: m * 128 + 128],
                            x_relu[:, j, l0 + dy - 1: l0 + dy - 1 + nr,
                                   dx: dx + 32],
                            start=(k == 0 and nr == 16), stop=False,
                            skip_group_check=True)
                        k += 1
        for (m, l0, nr) in XBG:
            px = px_t[(m, l0)]
            k = 0
            for j in range(2):
                for dy in range(3):
                    for dx in range(3):
                        nc.tensor.matmul(
                            px[:, : nr * 32],
                            rbd_wT[:, j, dy, dx, m * 128: m * 128 + 128],
                            x_raw[:, j, l0 + dy - 1: l0 + dy - 1 + nr,
                                  dx: dx + 32],
                            start=(k == 0 and nr == 16), stop=False,
                            skip_group_check=True)
                        k += 1
        # keep the PE clock ramped through the rest of the AllGather window
        spin_pe(psW1, 48)

        # ---- val: own window (SBUF) + partner window (AG slot), normalize,
        # window into x ----
        vs_p = wk.tile([128, 2, 672], bf, name="vs_p")
        nc.sync.dma_start(
            vs_p[:, :, :],
            arvg2[bass.ds(omh, 1), 0:256, :].rearrange(
                "s (j p) w -> s p j w", j=2))
        sp_row = wk.tile([1, 672], bf, name="sp_row")
        nc.gpsimd.dma_start(sp_row[:, :], arvg2[bass.ds(omh, 1), 256:257, :])
        for j in range(2):
            nc.vector.tensor_add(val_pad[:, j, bass.ds(vstart160, 672)],
                                 v_sb[:, j, bass.ds(myoff, 672)],
                                 vs_p[:, j, :])
        s_row = wk.tile([1, 672], f32, name="s_row")
        nc.vector.tensor_add(s_row[:, :], s_sb[0:1, bass.ds(myoff, 672)],
                             sp_row[:, :])
        inv_row = wk.tile([1, 672], f32, name="inv_row")
        nc.vector.reciprocal(inv_row[:, :], s_row[:, :])
        inv_d = dram.tile([1, 672], f32)
        nc.sync.dma_start(inv_d[:], inv_row[:, :])
        inv_b = wk.tile([128, 21, 32], f32, name="inv_b")
        nc.sync.dma_start(inv_b.rearrange("p r c -> p (r c)"),
                          inv_d.partition_broadcast(128))
        vp_v = val_pad.rearrange("p j (r c) -> p j r c", c=32)
        omh5 = omh * 5
        for j in range(2):
            nc.vector.tensor_mul(x_raw[:, 2 + j, bass.ds(omh5, 21), 1:33],
                                 vp_v[:, j, bass.ds(omh5, 21), :],
                                 inv_b[:, :, :])
            nc.scalar.activation(x_relu[:, 2 + j, :, 1:33],
                                 x_raw[:, 2 + j, :, 1:33], AF.Relu)

        # ---- val-side conv accumulation + activations ----
        for (m, l0, nr) in R1G:
            pr = pr_t[(m, l0)]
            k = 0
            for j in (2, 3):
                for dy in range(3):
                    for dx in range(3):
                        nc.tensor.matmul(
                            pr[:, : nr * 32],
                            rb1_wT[:, j, dy, dx, m * 128: m * 128 + 128],
                            x_relu[:, j, l0 + dy - 1: l0 + dy - 1 + nr,
                                   dx: dx + 32],
                            start=False, stop=(k == 17),
                            skip_group_check=True)
                        k += 1
            nc.scalar.activation(
                r1_relu[:, m, l0: l0 + nr, 1:33], pr[:, : nr * 32],
                AF.Relu, bias=rb1_b[:, m: m + 1])
            nc.vector.tensor_mul(r1_relu[:, m, l0: l0 + nr, 1:33],
                                 r1_relu[:, m, l0: l0 + nr, 1:33],
                                 rmaskb[:, l0: l0 + nr, 1:33])
        for (m, l0, nr) in XBG:
            px = px_t[(m, l0)]
            k = 0
            for j in (2, 3):
                for dy in range(3):
                    for dx in range(3):
                        nc.tensor.matmul(
                            px[:, : nr * 32],
                            rbd_wT[:, j, dy, dx, m * 128: m * 128 + 128],
                            x_raw[:, j, l0 + dy - 1: l0 + dy - 1 + nr,
                                  dx: dx + 32],
                            start=False, stop=False,
                            skip_group_check=True)
                        k += 1
            for j in range(2):
                for dy in range(3):
                    for dx in range(3):
                        nc.tensor.matmul(
                            px[:, : nr * 32],
                            rb2_wT[:, j, dy, dx, m * 128: m * 128 + 128],
                            r1_relu[:, j, l0 + dy - 1: l0 + dy - 1 + nr,
                                    dx: dx + 32],
                            start=False, stop=(k == 35),
                            skip_group_check=True)
                        k += 1
            nc.scalar.activation(
                xbv[m][:, l0 - 2: l0 - 2 + nr, 1:33], px[:, : nr * 32],
                AF.Identity, bias=xb_bias[:, m: m + 1])
        psW1_cm.__exit__(None, None, None)
        psXB_cm.__exit__(None, None, None)
        psR1_cm.__exit__(None, None, None)

        if stage == "dbg":
            nc.sync.dma_start(prm["dbg_xraw"][:], x_raw[:])
            for j in range(2):
                nc.sync.dma_start(prm["dbg_xb"][:, j], xbv[j])

        # ---- CBAM channel gate ----
        stats = wk.tile([128, 2, 2], f32, name="stats")
        for j in range(2):
            nc.vector.tensor_reduce(stats[:, j, 0:1], xbv[j][:, 3:19, 1:33],
                                    AX.XY, ALU.add)
            nc.vector.tensor_reduce(stats[:, j, 1:2], xbv[j][:, 3:19, 1:33],
                                    AX.XY, ALU.max)
        stats_d = dram.tile([256, 2], f32)
        stats_o = dram.tile([2, 256, 2], f32)
        nc.sync.dma_start(stats_d.rearrange("(j p) k -> p j k", j=2),
                          stats[:, :, :])
        # zeroed 38-stride comp rows in SBUF (borders give the conv halo);
        # the spatial-conv im2col then gathers straight from SBUF
        comp_sp = wk.tile([1, CROWS * 38], f16, name="comp_sp")
        mean_sp = wk.tile([1, CROWS * 38], f16, name="mean_sp")
        nc.vector.memset(comp_sp[:], 0.0)
        nc.vector.memset(mean_sp[:], 0.0)
        nc.gpsimd.collective_compute(
            "AllGather", ALU.bypass, replica_groups=PAIRS,
            ins=[stats_d[:].opt()], outs=[stats_o[:].opt()])
        slb = wk.tile([128, 2, 2, 2], f32, name="slb")  # [p, slot, j, stat]
        nc.sync.dma_start(slb[:, :, :, :],
                          stats_o.rearrange("s (j p) k -> p s j k", j=2))
        gate_in = wk.tile([128, 2, 2], f32, name="gate_in")
        tsum = wk.tile([128, 2, 1], f32, name="tsum")
        nc.vector.tensor_add(tsum[:, :, :], slb[:, 0, :, 0:1],
                             slb[:, 1, :, 0:1])
        nc.scalar.mul(gate_in[:, :, 0:1], tsum[:, :, :], 1.0 / 1024.0)
        nc.vector.tensor_max(gate_in[:, :, 1:2], slb[:, 0, :, 1:2],
                             slb[:, 1, :, 1:2])

        gate = wk.tile([128, 2, 1], f32, name="gate")
        ones1 = wk.tile([1, 128], f16, name="ones1")
        nc.vector.memset(ones1[:], 1.0)
        with tc.tile_pool(name="psG", bufs=1, space="PSUM") as psG:
            ph1 = psG.tile([16, 2], f32, name="ph1")
            for j in range(2):
                nc.tensor.matmul(ph1[:, :], mlp_w1T[:, j, :], gate_in[:, j, :],
                                 start=(j == 0), stop=(j == 1))
            h1 = wk.tile([16, 2], f16, name="h1")
            nc.scalar.activation(h1[:, :], ph1[:, :], AF.Relu,
                                 bias=mlp_b1[:, 0:1])
            # per-partition gate (sigmoid via the already-loaded exp table)
            for j in range(2):
                ph2 = psG.tile([128, 2], f32, tag="ph2", name="ph2")
                nc.tensor.matmul(ph2[:, :], mlp_w2T[:, j * 128: j * 128 + 128],
                                 h1[:, :], start=True, stop=True)
                h2 = wk.tile([128, 2], f32, tag="h2", name="h2")
                nc.vector.tensor_copy(h2[:, :], ph2[:, :])
                t2 = wk.tile([128, 1], f32, tag="t2", name="t2")
                nc.vector.tensor_add(t2[:, :], h2[:, 0:1], h2[:, 1:2])
                ev = wk.tile([128, 1], f32, tag="ev", name="ev")
                nc.scalar.activation(ev[:, :], t2[:, :], AF.Exp, scale=-1.0,
                                     bias=mlp_b2x2n[:, j: j + 1])
                e1 = wk.tile([128, 1], f32, tag="e1", name="e1")
                nc.scalar.activation(e1[:, :], ev[:, :], AF.Identity,
                                     bias=1.0)
                nc.vector.reciprocal(gate[:, j, :], e1[:, :])

        if stage == "dbg":
            nc.sync.dma_start(prm["dbg_gate"][:], gate[:])

        gate_sc = wk.tile([128, 2, 1], f16, name="gate_sc")
        nc.scalar.mul(gate_sc[:, :, :], gate[:, :, :], 1.0 / 256.0)

        # channel max of xb*gate via a cross-partition gpsimd reduce -- the
        # result lands directly in pixel-major layout, skipping the PE
        # transposes and one DRAM staging hop
        xcj = wk.tile([128, 2, 768], f16, name="xcj")
        for j in range(2):
            nc.vector.tensor_scalar_mul(xcj[:, j, :], xb[:, j, :],
                                        gate[:, j, 0:1])
        cmx = wk.tile([128, 768], f32, name="cmx")
        cmx2 = wk.tile([128, 768], f32, name="cmx2")
        nc.gpsimd.partition_all_reduce(cmx[:, :], xcj[:, 0, :], 128,
                                       bass_isa.ReduceOp.max)
        nc.gpsimd.partition_all_reduce(cmx2[:, :], xcj[:, 1, :], 128,
                                       bass_isa.ReduceOp.max)
        comp_row = wk.tile([1, 768], f16, name="comp_row")
        nc.vector.tensor_max(comp_row[0:1, :], cmx[0:1, :], cmx2[0:1, :])
        nc.vector.tensor_mul(
            comp_sp[0:1, :].rearrange("o (r c) -> o r c", c=38)[:, :, 2:36],
            comp_row[0:1, 0:748].rearrange("o (r c) -> o r c", c=34),
            mask_mean[0:1, 0:748].rearrange("o (r c) -> o r c", c=34))

        # channel mean of xb*gate via gate-weighted ones-matmul; the mean
        # half of comp then flows through its DRAM hops on the Pool queue
        # while the max half (slower DVE path) catches up on the SP queue.
        il = wk.tile([14, 16, 38], f16, name="il")
        mean_sb = wk.tile([1, 748], f16, name="mean_sb")
        psW2_cm = tc.tile_pool(name="psW2", bufs=1, space="PSUM")
        psW2 = psW2_cm.__enter__()
        with tc.tile_pool(name="psM", bufs=1, space="PSUM") as psM:
            pm1 = psM.tile([1, 748], f32, name="pm1")
            for j in range(2):
                for (o0, nn) in ((0, 512), (512, 236)):
                    nc.tensor.matmul(pm1[0:1, o0: o0 + nn],
                                     gate_sc[:, j, :],
                                     xb[:, j, o0: o0 + nn],
                                     start=(j == 0), stop=(j == 1))
            nc.scalar.copy(mean_sb[:, :], pm1[:, :])
        nc.vector.tensor_mul(
            mean_sp[0:1, :].rearrange("o (r c) -> o r c", c=38)[:, :, 2:36],
            mean_sb[0:1, :].rearrange("o (r c) -> o r c", c=34),
            mask_mean[0:1, 0:748].rearrange("o (r c) -> o r c", c=34))
        # keep the PE clock ramped until the spatial-conv operands land
        spin_pe(psW2, 40)

        # tracked guard-reads on the issuing queues order the untracked
        # (manual-AP) im2col gathers after the DVE mask-mul writes
        guard_d = dram.tile([2, CROWS * 38], f16)
        nc.gpsimd.dma_start(guard_d[1:2, :], mean_sp[0:1, :])
        nc.gpsimd.dma_start(
            il[7:14, :, :],
            bass.AP(mean_sp.tensor, 0, [[1, 1], [38, 7], [38, 16], [1, 38]]))
        nc.sync.dma_start(guard_d[0:1, :], comp_sp[0:1, :])
        nc.sync.dma_start(
            il[0:7, :, :],
            bass.AP(comp_sp.tensor, 0, [[1, 1], [38, 7], [38, 16], [1, 38]]))
        sigb = wk.tile([128, 16, 32], f16, name="sigb")
        with tc.tile_pool(name="psS", bufs=1, space="PSUM") as psS:
            pss = psS.tile([1, 512], f32, name="pss")
            for dx in range(7):
                nc.tensor.matmul(pss[:, :], spw_r[:, dx: dx + 1],
                                 il[:, :, dx: dx + 32],
                                 start=(dx == 0), stop=(dx == 6))
            # sigmoid via 1/(1+exp(-z)): broadcast exp(-z) down the
            # partitions first, then finish wide on the DVE straight into
            # the f16 sigb tile (no narrow row round-trips)
            se = wk.tile([1, 512], f16, name="se")
            nc.scalar.activation(se[:, :], pss[:, :], AF.Exp,
                                 scale=bn_nsb[0:1, 0:1], bias=bn_nsb[0:1, 1:2])
            sigb_ps = psS.tile([128, 512], f32, tag="sigbps", name="sigb_ps")
            nc.tensor.matmul(sigb_ps[:, :], ones1[0:1, :], se[0:1, :],
                             start=True, stop=True)
            t1 = wk.tile([128, 512], f32, name="t1")
            nc.vector.tensor_scalar_add(t1[:, :], sigb_ps[:, :], 1.0)
            with nc.allow_low_precision(reason="sigmoid output in [0,1]"):
                nc.vector.reciprocal(sigb.rearrange("p r c -> p (r c)"),
                                     t1[:, :])
        psW2_cm.__exit__(None, None, None)

        if stage == "dbg":
            nc.sync.dma_start(prm["dbg_sig"][:], se[:])
            nc.sync.dma_start(prm["dbg_il"][:], il[:])
            nc.sync.dma_start(prm["dbg_cmp"][:], comp_row[:])

        # fused = xb_own + (xb_own * gate) * sigb
        fused = apool.tile([128, 2, 16, 32], f16)
        for j in range(2):
            xc_own = wk.tile([128, 16, 32], f16, tag="xc_own", name="xc_own")
            nc.scalar.mul(xc_own[:, :, :], xbv[j][:, 3:19, 1:33],
                          gate[:, j, 0:1])
            tm = wk.tile([128, 16, 32], f16, tag="tm", name="tm")
            nc.vector.tensor_mul(tm[:, :, :], xc_own[:, :, :], sigb[:, :, :])
            nc.vector.tensor_add(fused[:, j, :, :], xbv[j][:, 3:19, 1:33],
                                 tm[:, :, :])

        if stage == "dbg":
            nc.sync.dma_start(prm["dbg_fused"][:], fused[:])

        # bottleneck conv: accumulate the fused-input chunks now so the PE
        # works during the pools AllGather; priors chunks finish the group
        # after the collective.
        fbv = fused.rearrange("p j r c -> p j (r c)")
        psO_cm = tc.tile_pool(name="psO", bufs=2, space="PSUM")
        psO = psO_cm.__enter__()
        po_t = []
        for m in range(2):
            po = psO.tile([128, 512], f32, tag="po", name="po")
            po_t.append(po)
            for jj in range(2):
                nc.tensor.matmul(po[:, :],
                                 bott_wT[:, 2 + jj, m * 128: m * 128 + 128],
                                 fbv[:, jj, :],
                                 start=(jj == 0), stop=False,
                                 skip_group_check=True)

        # ---- PSP pools (raw block sums over own rows) ----
        pools = wk.tile([128, 2, 43], f32, name="pools")
        for j in range(2):
            f8 = fused[:, j].rearrange("p (rb ri) (cb ci) -> p rb cb ri ci",
                                       ri=4, ci=4)
            p8v = pools[:, j, 11:43].rearrange("p (rb cb) -> p rb cb", cb=8)
            nc.vector.tensor_reduce(p8v, f8, AX.XY, ALU.add)
            p8i = pools[:, j, 11:43].rearrange(
                "p (rb ri cb ci) -> p rb cb ri ci", rb=2, ri=2, cb=4, ci=2)
            p4v = pools[:, j, 3:11].rearrange("p (rb cb) -> p rb cb", cb=4)
            nc.vector.tensor_reduce(p4v, p8i, AX.XY, ALU.add)
        p4i = pools[:, :, 3:11].rearrange(
            "p j (rb cb ci) -> p j cb rb ci", rb=2, cb=2, ci=2)
        nc.vector.tensor_reduce(
            pools[:, :, 1:3].rearrange("p j (a k) -> p j a k", a=2, k=1),
            p4i, AX.XY, ALU.add)
        nc.vector.tensor_reduce(pools[:, :, 0:1], pools[:, :, 1:3], AX.X,
                                ALU.add)

        pools16 = wk.tile([128, 2, 43], f16, name="pools16")
        nc.vector.tensor_copy(pools16[:, :, :], pools[:, :, :])
        pools_d = dram.tile([2, 128, 43], f16)
        pools_o = dram.tile([2, 2, 128, 43], f16)
        nc.sync.dma_start(pools_d.rearrange("j p k -> p j k"),
                          pools16[:, :, :])
        nc.gpsimd.collective_compute(
            "AllGather", ALU.bypass, replica_groups=PAIRS,
            ins=[pools_d[:].opt()], outs=[pools_o[:].opt()])
        # 1x1 convs on pools in TRANSPOSED layout (out partition = pool
        # block, free = psp channel). The OWN half comes straight from
        # pools16 in SBUF and runs DURING the AllGather; only the partner
        # half waits for the collective. Wup's slot dim is indexed
        # dynamically (own = rank-in-pair, partner = the other).
        SI = {1: 0, 2: 1, 4: 2, 8: 3}
        pdT_a8 = [wk.tile([32, 64], f16, tag=f"pdT_a8{s}", name=f"pdT_a8{s}")
                  for s in range(2)]
        pdT_s1 = wk.tile([1, 64], f16, name="pdT_s1")
        pdT_b = [wk.tile([8, 64], f16, tag=f"pdT_b{s}", name=f"pdT_b{s}")
                 for s in range(2)]
        pdT_c = [wk.tile([2, 64], f16, tag=f"pdT_c{s}", name=f"pdT_c{s}")
                 for s in range(2)]
        pri = [wk.tile([128, 512], f16, tag=f"pri{i}", name=f"pri{i}")
               for i in range(2)]
        ones_f = wk.tile([128, 512], f16, name="ones_f")
        nc.vector.memset(ones_f[:], 1.0)
        psP_cm = tc.tile_pool(name="psP", bufs=1, space="PSUM")
        psP = psP_cm.__enter__()
        psR_cm = tc.tile_pool(name="psR", bufs=1, space="PSUM")
        psR = psR_cm.__enter__()
        pdm_ps = psP.tile([65, 64], f32, name="pdm_ps")
        pd4_ps = psP.tile([40, 64], f32, name="pd4_ps")
        pd2_ps = psP.tile([34, 64], f32, name="pd2_ps")
        pp0 = psR.tile([128, 512], f32, tag="pp0", name="pp0")
        pp1 = psR.tile([128, 512], f32, tag="pp1", name="pp1")

        def pd_matmuls(src, sl):
            # src[j] -> [128, 43] pool partials for this half (j = ch chunk)
            for j in range(2):
                nc.tensor.matmul(pdm_ps[32 * sl: 32 * sl + 32, :],
                                 src(j, 11, 43), psp_wT[:, j, SI[8], :],
                                 start=(j == 0), stop=(j == 1),
                                 skip_group_check=True)
            for j in range(2):
                nc.tensor.matmul(pdm_ps[64:65, :],
                                 src(j, 0, 1), psp_wT[:, j, SI[1], :],
                                 start=(sl == 0 and j == 0),
                                 stop=(sl == 1 and j == 1),
                                 skip_group_check=True)
            for j in range(2):
                nc.tensor.matmul(pd4_ps[32 * sl: 32 * sl + 8, :],
                                 src(j, 3, 11), psp_wT[:, j, SI[4], :],
                                 start=(j == 0), stop=(j == 1),
                                 skip_group_check=True)
            for j in range(2):
                nc.tensor.matmul(pd2_ps[32 * sl: 32 * sl + 2, :],
                                 src(j, 1, 3), psp_wT[:, j, SI[2], :],
                                 start=(j == 0), stop=(j == 1),
                                 skip_group_check=True)

        def pd_copies(sl):
            nc.scalar.copy(pdT_a8[sl][:, :], pdm_ps[32 * sl: 32 * sl + 32, :])
            nc.vector.tensor_copy(pdT_b[sl][:, :],
                                  pd4_ps[32 * sl: 32 * sl + 8, :])
            nc.vector.tensor_copy(pdT_c[sl][:, :],
                                  pd2_ps[32 * sl: 32 * sl + 2, :])

        def upsample(sl, slot_idx):
            nc.tensor.matmul(pp0[64:128, :], pdT_c[sl][:, :],
                             Wup[0:2, bass.ds(slot_idx, 1), 0, :],
                             start=(sl == 0), stop=(sl == 1),
                             tile_position=(0, 64), skip_group_check=True)
            nc.tensor.matmul(pp1[0:64, :], pdT_b[sl][:, :],
                             Wup[0:8, bass.ds(slot_idx, 1), 1, :],
                             start=(sl == 0), stop=(sl == 1),
                             tile_position=(0, 0), skip_group_check=True)
            nc.tensor.matmul(pp1[64:128, :], pdT_a8[sl][:, :],
                             Wup[0:32, bass.ds(slot_idx, 1), 2, :],
                             start=(sl == 0), stop=(sl == 1),
                             tile_position=(0, 64), skip_group_check=True)

        psW3_cm = tc.tile_pool(name="psW3", bufs=1, space="PSUM")
        psW3 = psW3_cm.__enter__()
        # own half: runs while the collective is in flight
        pd_matmuls(lambda j, a, b: pools16[:, j, a:b], 0)
        pd_copies(0)
        upsample(0, pid2)
        # keep the PE clock ramped through the pools AllGather
        spin_pe(psW3, 88)
        # partner half: after the collective
        slp_p = wk.tile([128, 2, 43], f16, name="slp_p")
        nc.sync.dma_start(
            slp_p[:, :, :],
            pools_o[bass.ds(omh, 1)].rearrange("s j p k -> s p j k"))
        pd_matmuls(lambda j, a, b: slp_p[:, j, a:b], 1)
        pd_copies(1)
        nc.scalar.copy(pdT_s1[:, :], pdm_ps[64:65, :])
        upsample(1, omh)
        nc.tensor.matmul(pp0[0:64, :], pdT_s1[:, :], ones_f[0:1, :],
                         start=True, stop=True, skip_group_check=True)
        nc.scalar.copy(pri[0][:, :], pp0[:, :])
        nc.scalar.copy(pri[1][:, :], pp1[:, :])
        psW3_cm.__exit__(None, None, None)
        psR_cm.__exit__(None, None, None)
        psP_cm.__exit__(None, None, None)

        if stage == "dbg":
            nc.sync.dma_start(prm["dbg_pd"][0:32], pdT_a8[0][:])
            nc.sync.dma_start(prm["dbg_pd"][32:64], pdT_a8[1][:])
            nc.sync.dma_start(prm["dbg_pd"][64:65], pdT_s1[:])

        if stage == "dbg":
            nc.sync.dma_start(prm["dbg_pri0"][:], pri[0][:])
            nc.sync.dma_start(prm["dbg_pri1"][:], pri[1][:])

        out_sb = wk.tile([128, 2, 512], f32, name="out_sb")
        for m in range(2):
            po = po_t[m]
            for k in range(2):
                nc.tensor.matmul(po[:, :],
                                 bott_wT[:, k, m * 128: m * 128 + 128],
                                 pri[k][:, :],
                                 start=False, stop=(k == 1),
                                 skip_group_check=True)
            nc.scalar.activation(out_sb[:, m, :], po[:, :], AF.Relu,
                                 bias=bott_b[:, m: m + 1])
            # per-half output DMA overlaps the other half's epilogue
            nc.sync.dma_start(
                out_prm[:, m: m + 1, :, :],
                out_sb[:, m: m + 1, :].rearrange("p j (r c) -> p j r c",
                                                 c=32))
        psO_cm.__exit__(None, None, None)


# ---------------------------------------------------------------------------
# Runner
# ---------------------------------------------------------------------------

_CACHE = {}


def _get_nc(stage="full"):
    if stage not in _CACHE:
        _CACHE[stage] = build(stage)
    return _CACHE[stage]


def run_cores(inputs, stage="full"):
    nc = _get_nc(stage)
    in_maps = [prep_core_inputs(inputs, c) for c in range(N_CORES)]
    res = run_bass_kernel_spmd(nc, in_maps, list(range(N_CORES)))
    return res.results


def kernel(**inputs):
    results = run_cores(inputs, "full")
    out = np.zeros((B, 1, COUT, H, W), np.float32)
    for c in range(N_CORES):
        b, h = c // 2, c % 2
        o = results[c]["out"]                    # [128, 2, 16, 32]
        out[b, 0, :, 16 * h: 16 * h + 16, :] = (
            o.transpose(1, 0, 2, 3).reshape(COUT, 16, 32))
    return out



# revision 107
# speedup vs baseline: 1.0156x; 1.0017x over previous
"""Trainium2 Bass kernel for nn_BottleneckFusion (STCN memory readout + ResBlock
+ CBAM + PSP + bottleneck), 8-core SPMD.

Sharding: core c -> (batch b = c//2, half h = c%2).
  Phase A (attention): TM split across the pair (4 memory frames each);
    software-pipelined affinity/exp/value loop; flash-style combine of
    (unnormalized value, sumexp) via a pairwise AllGather TRIMMED to the
    21-image-row window the partner actually needs (dynamic partition-id
    driven send/receive offsets).
  Phase B (convs/CBAM/PSP): row-half split with halo recompute. The
    batch-query half of the ResBlock conv accumulation is emitted before the
    value AllGather so it fills the collective window; PE-warming spin
    matmuls keep the tensor clock ramped through the other collectives.
    CBAM sigmoids use 1/(1+exp(-z)) so only the exp act table is ever
    loaded; the spatial-gate channel max uses a gpsimd cross-partition
    reduce (no transposes); PSP 1x1 convs are emitted directly in
    transposed (block-major) layout with the own-half computed from local
    pools during the pools AllGather and dynamic Wup slot slices.

kernel(**inputs) takes the FULL unsharded inputs and returns the FULL output.
"""
import sys

sys.path.insert(0, "/opt/trn_rl_repo")

import numpy as np
import ml_dtypes

import concourse.bass as bass
import concourse.bacc as bacc
import concourse.bass_isa as bass_isa
import concourse.mybir as mybir
import concourse.tile as tile
from concourse.bass_utils import run_bass_kernel_spmd

BF16 = ml_dtypes.bfloat16
F16 = np.float16
bf = mybir.dt.bfloat16
f16 = mybir.dt.float16
f32 = mybir.dt.float32
AF = mybir.ActivationFunctionType
ALU = mybir.AluOpType
AX = mybir.AxisListType

N_CORES = 8
B, TM, CIN, CK, CV, COUT, H, W = 4, 8, 256, 64, 256, 256, 32, 32
EPS = 1e-5

# local row coordinates: l = image_row - (r0 - 5), l in 0..25
XROWS = 26                 # x window rows (image r0-5 .. r0+20)
CROWS = 22                 # xb/xc/comp local rows (image r0-3 .. r0+18)
PIXPAD = 768               # padded xc free size (22*34=748 -> 768)
PAIRS = [[0, 1], [2, 3], [4, 5], [6, 7]]
UPS = (2, 4, 8)            # upsampled PSP scales
# full pool pyramid offsets [s1, s2, s4, s8] and own-partial offsets
FOFF = {1: 0, 2: 1, 4: 5, 8: 21}
POFF = {1: 0, 2: 1, 4: 3, 8: 11}


def interp_matrix(s_in, s_out=32):
    if s_in == 1:
        return np.ones((s_out, 1), np.float32)
    c = np.arange(s_out) * (s_in - 1) / (s_out - 1)
    lo = np.floor(c).astype(np.int64)
    hi = np.minimum(lo + 1, s_in - 1)
    w = (c - lo).astype(np.float32)
    M = np.zeros((s_out, s_in), np.float32)
    M[np.arange(s_out), lo] += 1.0 - w
    M[np.arange(s_out), hi] += w
    return M


# ---------------------------------------------------------------------------
# Host-side input preparation
# ---------------------------------------------------------------------------

def _pad_hw(a):
    out = np.zeros(a.shape[:-2] + (34, 34), a.dtype)
    out[..., 1:33, 1:33] = a
    return out


def _chw_chunks(a):
    """[256, ...] -> [128, 2, ...] (partition, chunk)."""
    return a.reshape(2, 128, *a.shape[1:]).transpose(
        1, 0, *range(2, a.ndim + 1))


def prep_core_inputs(inputs, core):
    b, h = core // 2, core % 2
    r0 = 16 * h
    g = {}

    f16_q = np.asarray(inputs["f16_q"], np.float32)
    f16_m = np.asarray(inputs["f16_m"], np.float32)
    value_m = np.asarray(inputs["value_m"], np.float32)

    # xm: [128, 2, 4, 34, 34] padded memory frames
    src = f16_m[b, 4 * h: 4 * h + 4]                        # [4, 256, 32, 32]
    src = src.reshape(4, 2, 128, 32, 32).transpose(2, 1, 0, 3, 4)
    g["xm"] = _pad_hw(src).astype(F16)

    # xq: [128, 2, 34, 34] padded query
    q = _chw_chunks(f16_q[b, 0])                            # [128, 2, 32, 32]
    g["xq"] = _pad_hw(q).astype(F16)

    # vT: [128, 32, 256] transposed value
    V = value_m[b][:, 4 * h: 4 * h + 4].reshape(CV, 4096)
    g["vT"] = np.ascontiguousarray(
        V.T.reshape(32, 128, CV).transpose(1, 0, 2)).astype(BF16)  # stays bf16 (matches e)

    # x window q-part: [128, 2, 26, 34]
    qw = np.zeros((128, 2, XROWS, 34), np.float32)
    for l in range(XROWS):
        img = r0 - 5 + l
        if 0 <= img <= 31:
            qw[:, :, l, 1:33] = q[:, :, img, :]
    g["xqb_raw"] = qw.astype(F16)
    g["xqb_relu"] = np.maximum(qw, 0.0).astype(F16)

    pk_w = np.asarray(inputs["pk_w"], np.float32)
    g["pk_wT"] = np.ascontiguousarray(
        pk_w.reshape(CK, 2, 128, 3, 3).transpose(2, 1, 3, 4, 0)).astype(F16)
    pk_b = np.asarray(inputs["pk_b"], np.float32)
    g["pkb2"] = np.concatenate([pk_b, pk_b]).reshape(128, 1).astype(np.float32)

    def conv_lhsT(w, kc):
        co = w.shape[0]
        return np.ascontiguousarray(
            w.reshape(co, kc, 128, 3, 3).transpose(2, 1, 3, 4, 0)).astype(F16)

    g["rb1_wT"] = conv_lhsT(np.asarray(inputs["rb1_w"], np.float32), 4)
    g["rb2_wT"] = conv_lhsT(np.asarray(inputs["rb2_w"], np.float32), 2)
    g["rbd_wT"] = conv_lhsT(np.asarray(inputs["rbd_w"], np.float32), 4)
    g["rb1_b"] = np.asarray(inputs["rb1_b"], np.float32).reshape(2, 128).T.copy()
    g["xb_bias"] = (np.asarray(inputs["rb2_b"], np.float32)
                    + np.asarray(inputs["rbd_b"], np.float32)
                    ).reshape(2, 128).T.copy()

    w1 = np.asarray(inputs["mlp_w1"], np.float32)           # [16, 256]
    g["mlp_w1T"] = np.ascontiguousarray(
        w1.reshape(16, 2, 128).transpose(2, 1, 0)).copy()   # [128, 2, 16]
    g["mlp_b1"] = np.asarray(inputs["mlp_b1"], np.float32).reshape(16, 1).copy()
    g["mlp_w2T"] = np.ascontiguousarray(
        np.asarray(inputs["mlp_w2"], np.float32).T).astype(F16)  # [16, 256]
    # sigmoid is computed as 1/(1+exp(-z)) on the exp table, so biases are
    # stored pre-negated for the exp activation
    g["mlp_b2x2n"] = (-2.0 * np.asarray(inputs["mlp_b2"], np.float32)
                      ).reshape(2, 128).T.copy()
    g["mlp_b2row"] = (2.0 * np.asarray(inputs["mlp_b2"], np.float32)
                      ).reshape(1, 256).astype(F16)

    spw = np.asarray(inputs["sp_w"], np.float32)[0]       # [2, 7, 7]
    g["spw_r"] = np.ascontiguousarray(
        spw.reshape(14, 7)).astype(np.float16)                # [(ch,dy), dx]
    bn_scale = float(np.asarray(inputs["sp_g"], np.float32)[0]) / float(
        np.sqrt(1.0 + EPS))
    bn_bias = float(np.asarray(inputs["sp_b"], np.float32)[0])
    g["bn_nsb"] = np.array([[-bn_scale, -bn_bias]], np.float32)

    maskT = np.zeros((128, 6, 1), np.float16)
    mask_mean = np.zeros((1, 768), np.float16)
    for pix in range(CROWS * 34):
        img = r0 - 3 + pix // 34
        if 0 <= img <= 31:
            maskT[pix % 128, pix // 128, 0] = 1.0
            mask_mean[0, pix] = 1.0
    g["comp_maskT"] = maskT
    g["mask_mean"] = mask_mean

    pw = np.zeros((128, 2, 4, 64), np.float32)
    for si, s in enumerate((1, 2, 4, 8)):
        wc = np.asarray(inputs[f"psp_w{s}"], np.float32)[:, :, 0, 0]
        scale = 1.0 / ((32 // s) ** 2)
        pw[:, :, si, :] = (wc.T * scale).reshape(2, 128, 64).transpose(1, 0, 2)
    g["psp_wT"] = pw.astype(F16)

    # folded upsample operators, split per AG slot so every matmul operand
    # sits at partition base 0: Wup[k_local, sl, si, (r*32+c)] with the
    # global block k = jr*s+jc split as sl = k // (s*s/2), k_local = k % ..
    Wup = np.zeros((32, 2, 3, 512), np.float32)
    for si, s in enumerate(UPS):
        M = interp_matrix(s)
        Mrr = M[r0: r0 + 16, :]                 # [16, s]
        half = s * s // 2
        for jr in range(s):
            for jc in range(s):
                k = jr * s + jc
                Wup[k % half, k // half, si, :] = np.outer(
                    Mrr[:, jr], M[:, jc]).reshape(512)
    g["Wup"] = Wup.astype(F16)

    bott_w = np.asarray(inputs["bott_w"], np.float32)[:, :, 0, 0]
    g["bott_wT"] = np.ascontiguousarray(
        bott_w.reshape(COUT, 4, 128).transpose(2, 1, 0)).astype(F16)
    g["bott_b"] = np.asarray(inputs["bott_b"], np.float32).reshape(2, 128).T.copy()

    rmask = np.zeros((1, XROWS, 34), np.float16)
    for l in range(XROWS):
        if 0 <= r0 - 5 + l <= 31:
            rmask[0, l, :] = 1.0
    g["rmask"] = rmask

    g["ident"] = np.eye(128, dtype=F16)
    return g


INPUT_SPECS = [
    ("xm", [128, 2, 4, 34, 34], f16),
    ("xq", [128, 2, 34, 34], f16),
    ("vT", [128, 32, 256], bf),
    ("xqb_raw", [128, 2, XROWS, 34], f16),
    ("xqb_relu", [128, 2, XROWS, 34], f16),
    ("pk_wT", [128, 2, 3, 3, 64], f16),
    ("pkb2", [128, 1], f32),
    ("rb1_wT", [128, 4, 3, 3, 256], f16),
    ("rb2_wT", [128, 2, 3, 3, 256], f16),
    ("rbd_wT", [128, 4, 3, 3, 256], f16),
    ("rb1_b", [128, 2], f32),
    ("xb_bias", [128, 2], f32),
    ("mlp_w1T", [128, 2, 16], f32),
    ("mlp_b1", [16, 1], f32),
    ("mlp_w2T", [16, 256], f16),
    ("mlp_b2x2n", [128, 2], f32),
    ("mlp_b2row", [1, 256], f16),
    ("spw_r", [14, 7], f16),
    ("bn_nsb", [1, 2], f32),
    ("comp_maskT", [128, 6, 1], f16),
    ("mask_mean", [1, 768], f16),
    ("psp_wT", [128, 2, 4, 64], f16),
    ("Wup", [32, 2, 3, 512], f16),
    ("bott_wT", [128, 4, 256], f16),
    ("bott_b", [128, 2], f32),
    ("ident", [128, 128], f16),
    ("rmask", [1, XROWS, 34], f16),
]


# ---------------------------------------------------------------------------
# Device kernel
# ---------------------------------------------------------------------------

def build(stage="full"):
    nc = bacc.Bacc("TRN2", target_bir_lowering=False, debug=False,
                   num_devices=N_CORES)
    prm = {n: nc.declare_dram_parameter(n, sh, dt, isOutput=False)
           for n, sh, dt in INPUT_SPECS}
    if stage == "A":
        out_prm = nc.declare_dram_parameter("out_a", [257, 1024], f32,
                                            isOutput=True)
    else:
        out_prm = nc.declare_dram_parameter("out", [128, 2, 16, 32], f32,
                                            isOutput=True)
    if stage == "dbg":
        for n, sh, dt in [("dbg_xraw", [128, 4, XROWS, 34], f16),
                          ("dbg_xb", [128, 2, CROWS, 34], f16),
                          ("dbg_gate", [128, 2, 1], f32),
                          ("dbg_sig", [1, 512], f16),
                          ("dbg_fused", [128, 2, 16, 32], f16),
                          ("dbg_pd", [65, 64], f16),
                          ("dbg_il", [14, 16, 38], f16),
                          ("dbg_cmp", [1, 768], f16),
                          ("dbg_pri0", [128, 512], f16),
                          ("dbg_pri1", [128, 512], f16)]:
            prm[n] = nc.declare_dram_parameter(n, sh, dt, isOutput=True)
    with tile.TileContext(nc) as tc:
        _emit(tc, nc, prm, stage, out_prm)
    nc.compile()
    return nc


def _emit(tc, nc, prm, stage, out_prm):
    import contextlib
    es = contextlib.ExitStack()
    with es:
        wpool = es.enter_context(tc.tile_pool(name="wpool", bufs=1))
        apool = es.enter_context(tc.tile_pool(name="apool", bufs=1))
        dram = es.enter_context(tc.tile_pool(name="dram", bufs=1, space="DRAM"))
        aonly_cm = tc.tile_pool(name="aonly", bufs=1)
        aonly = aonly_cm.__enter__()

        def load(name, pool=wpool):
            t = pool.tile(list(prm[name].shape), prm[name].dtype,
                          name=f"{name}_sb")
            nc.sync.dma_start(t[:], prm[name][:])
            return t

        pk_wT = load("pk_wT")
        pkb2 = load("pkb2")
        xm_sb = aonly.tile([128, 2, 4, 34, 34], f16, name="xm_sb")
        # frames 0/1 rows 0:19 land first so the mk conv starts early
        for t in range(2):
            nc.sync.dma_start(xm_sb[:, :, t, 0:19, :],
                              prm["xm"][:, :, t, 0:19, :])
        for t in range(2):
            nc.sync.dma_start(xm_sb[:, :, t, 19:34, :],
                              prm["xm"][:, :, t, 19:34, :])
        for t in range(2, 4):
            nc.sync.dma_start(xm_sb[:, :, t, :, :], prm["xm"][:, :, t, :, :])
        xq_sb = load("xq", aonly)
        vT_sb = load("vT", aonly)

        ones_bf = wpool.tile([128, 1], bf)
        nc.vector.memset(ones_bf[:], 1.0)
        # spin sources: tiny constant operands for PE-warming matmuls that
        # keep the tensor clock ramped through collective windows
        spin_w = wpool.tile([1, 1], f16)
        nc.vector.memset(spin_w[:], 0.0)
        spin_src = wpool.tile([1, 512], f16)
        nc.vector.memset(spin_src[:], 0.0)
        zero128 = wpool.tile([1, 128], f16)
        nc.vector.memset(zero128[:], 0.0)

        def spin_pe(pool, n, rows=512):
            for _ in range(n):
                sp = pool.tile([1, rows], f32, tag="spin", name="sp")
                nc.tensor.matmul(sp[0:1, :], spin_w[0:1, 0:1],
                                 spin_src[0:1, 0:rows],
                                 start=True, stop=True,
                                 skip_group_check=True)

        # ================= phase A =================
        mk_sb = aonly.tile([128, 2, 1024], f16)
        qk_sb = aonly.tile([128, 1024], f16)

        with tc.tile_pool(name="psA", bufs=2, space="PSUM") as psA:
            for tp in range(2):
                for n in range(2):
                    pm = psA.tile([128, 512], f32, tag="mkps", name="pm")
                    for par in range(2):
                        t = 2 * tp + par
                        k = 0
                        for j in range(2):
                            for dy in range(3):
                                for dx in range(3):
                                    nc.tensor.matmul(
                                        pm[64 * par: 64 * par + 64, :],
                                        pk_wT[:, j, dy, dx, :],
                                        xm_sb[:, j, t,
                                              n * 16 + dy: n * 16 + dy + 16,
                                              dx: dx + 32],
                                        start=(k == 0), stop=(k == 17),
                                        tile_position=(0, 64 * par),
                                    )
                                    k += 1
                    nc.scalar.activation(
                        mk_sb[:, tp, n * 512: (n + 1) * 512], pm[:, :],
                        AF.Identity, bias=pkb2[:, 0:1])

            for n in range(2):
                pq = psA.tile([64, 512], f32, tag="qkps", name="pq")
                k = 0
                for j in range(2):
                    for dy in range(3):
                        for dx in range(3):
                            nc.tensor.matmul(
                                pq[:, :], pk_wT[:, j, dy, dx, :],
                                xq_sb[:, j, n * 16 + dy: n * 16 + dy + 16,
                                      dx: dx + 32],
                                start=(k == 0), stop=(k == 17))
                            k += 1
                nc.scalar.activation(
                    qk_sb[0:64, n * 512: (n + 1) * 512], pq[:, :],
                    AF.Identity, bias=pkb2[0:64, 0:1])
            # replicate qk to partitions 64..127 so odd-frame mk slices
            # (base partition 64) can stream against it
            nc.sync.dma_start(qk_sb[64:128, :], qk_sb[0:64, :])

        # pair exchange buffers: full [257,1024] for the debug stage, a
        # 21-image-row window (the part the partner actually needs) otherwise
        if stage == "A":
            arv = dram.tile([257, 1024], bf)
            arvg = dram.tile([2, 257, 1024], bf)
        else:
            arv2 = dram.tile([257, 672], bf)
            arvg2 = dram.tile([2, 257, 672], bf)
        pid = nc.partition_id()
        pid2 = pid % 2
        omh = (pid + 1) % 2
        sendoff = omh * 352
        myoff = pid2 * 352
        vstart160 = omh * 160

        with (
            tc.tile_pool(name="psAff", bufs=2, space="PSUM") as psAff,
            tc.tile_pool(name="psV", bufs=1, space="PSUM") as psV,
        ):
            vps = [psV.tile([128, 1024], f32, name=f"vps{j}") for j in range(2)]
            s_acc = aonly.tile([128, 1024], bf, name="s_acc")

            order = [16 * h + o + 8 * par for h in range(2) for o in range(8)
                     for par in range(2)]

            def lhs_aff(i):
                t = i >> 3
                pb = i & 7
                tp, par = t >> 1, t & 1
                return par, mk_sb[64 * par: 64 * par + 64, tp,
                                  pb * 128: pb * 128 + 128]

            # software-pipelined: affinity matmuls + exp run one chunk ahead
            # of the value accumulation so the PE never waits on the exp.
            e_tiles = {}

            def emit_aff(idx):
                i = order[idx]
                par, lhs = lhs_aff(i)
                e_t = aonly.tile([128, 1024], bf, tag="e", name="e_t", bufs=4)
                pa = psAff.tile([128, 1024], f32, tag="affp", name="pa")
                for qn in range(2):
                    nc.tensor.matmul(
                        pa[:, qn * 512: (qn + 1) * 512], lhs,
                        qk_sb[64 * par: 64 * par + 64,
                              qn * 512: (qn + 1) * 512],
                        start=True, stop=True)
                nc.scalar.activation(e_t[:, :], pa[:, :], AF.Exp, scale=0.125)
                e_tiles[idx] = e_t

            emit_aff(0)
            for idx in range(32):
                if idx + 1 < 32:
                    emit_aff(idx + 1)
                i = order[idx]
                e_t = e_tiles.pop(idx)
                for j in range(2):
                    for qn in range(2):
                        nc.tensor.matmul(
                            vps[j][:, qn * 512: (qn + 1) * 512],
                            vT_sb[:, i, j * 128: (j + 1) * 128],
                            e_t[:, qn * 512: (qn + 1) * 512],
                            start=(idx == 0), stop=(idx == 31),
                            skip_group_check=True)
                if idx == 0:
                    nc.vector.tensor_copy(s_acc[:, :], e_t[:, :])
                else:
                    nc.vector.tensor_add(s_acc[:, :], s_acc[:, :], e_t[:, :])

            v_sb = apool.tile([128, 2, 1024], bf, name="v_sb")
            s_sb = apool.tile([1, 1024], bf, name="s_sb")
            # one PSUM->SBUF copy on DVE, one on Act so they run concurrently
            nc.vector.tensor_copy(v_sb[:, 0, :], vps[0][:, :])
            nc.scalar.copy(v_sb[:, 1, :], vps[1][:, :])
            if stage == "A":
                for j in range(2):
                    nc.sync.dma_start(arv[128 * j: 128 * j + 128, :],
                                      v_sb[:, j, :])
            else:
                nc.sync.dma_start(
                    arv2[0:256, :].rearrange("(j p) w -> p j w", j=2),
                    v_sb[:, :, bass.ds(sendoff, 672)])
            # fold the 128-partition sumexp accumulator with a ones matmul
            for qn in range(2):
                sfold = psAff.tile([1, 512], f32, tag="affp", name="sfold")
                nc.tensor.matmul(sfold[0:1, :],
                                 ones_bf[:, 0:1],
                                 s_acc[:, qn * 512: (qn + 1) * 512],
                                 start=True, stop=True)
                nc.vector.tensor_copy(s_sb[:, qn * 512: (qn + 1) * 512],
                                      sfold[0:1, :])
            if stage == "A":
                nc.sync.dma_start(arv[256:257, :], s_sb[:, :])
            else:
                nc.sync.dma_start(arv2[256:257, :],
                                  s_sb[0:1, bass.ds(sendoff, 672)])

        if stage == "A":
            nc.gpsimd.collective_compute(
                "AllGather", ALU.bypass, replica_groups=PAIRS,
                ins=[arv[:].opt()], outs=[arvg[:].opt()])
        else:
            nc.gpsimd.collective_compute(
                "AllGather", ALU.bypass, replica_groups=PAIRS,
                ins=[arv2[:].opt()], outs=[arvg2[:].opt()])


        aonly_cm.__exit__(None, None, None)

        if stage == "A":
            with tc.tile_pool(name="cmb", bufs=1) as cmb:
                cs0 = cmb.tile([1, 1024], bf, name="cs0")
                cs1 = cmb.tile([1, 1024], bf, name="cs1")
                cso = cmb.tile([1, 1024], f32, name="cso")
                nc.sync.dma_start(cs0[:], arvg[0, 256:257, :])
                nc.sync.dma_start(cs1[:], arvg[1, 256:257, :])
                nc.vector.tensor_add(cso[:, :], cs0[:, :], cs1[:, :])
                nc.sync.dma_start(out_prm[256:257, :], cso[:, :])
                for j in range(2):
                    ca = cmb.tile([128, 1024], bf, tag="ca", name="ca")
                    cb = cmb.tile([128, 1024], bf, tag="cb", name="cb")
                    co = cmb.tile([128, 1024], f32, tag="co", name="co")
                    nc.sync.dma_start(ca[:, :], arvg[0, 128 * j: 128 * j + 128, :])
                    nc.sync.dma_start(cb[:, :], arvg[1, 128 * j: 128 * j + 128, :])
                    nc.vector.tensor_add(co[:, :], ca[:, :], cb[:, :])
                    nc.sync.dma_start(out_prm[128 * j: 128 * j + 128, :],
                                      co[:, :])
            return

        # ================= phase B =================
        wk = es.enter_context(tc.tile_pool(name="wk", bufs=1))
        rb1_wT = load("rb1_wT")
        rb2_wT = load("rb2_wT")
        rbd_wT = load("rbd_wT")
        rb1_b = load("rb1_b")
        xb_bias = load("xb_bias")
        mlp_w1T = load("mlp_w1T")
        mlp_b1 = load("mlp_b1")
        mlp_w2T = load("mlp_w2T")
        mlp_b2x2n = load("mlp_b2x2n")
        spw_r = load("spw_r")
        bn_nsb = load("bn_nsb")
        mask_mean = load("mask_mean")
        psp_wT = load("psp_wT")
        Wup = load("Wup")
        bott_wT = load("bott_wT")
        bott_b = load("bott_b")
        ident = load("ident")

        r0v = (nc.vector.partition_id() % 2) * 16

        # ---- val-independent prep: x tiles, query-side loads, masks ----
        x_raw = apool.tile([128, 4, XROWS, 34], f16)
        x_relu = apool.tile([128, 4, XROWS, 34], f16)
        for tt in (x_raw, x_relu):
            nc.vector.memset(tt[:, 2:4, :, 0:1], 0.0)
            nc.vector.memset(tt[:, 2:4, :, 33:34], 0.0)
        nc.sync.dma_start(x_raw[:, 0:2, :, :], prm["xqb_raw"][:])
        nc.sync.dma_start(x_relu[:, 0:2, :, :], prm["xqb_relu"][:])

        r1_relu = apool.tile([128, 2, XROWS, 34], f16)
        nc.vector.memset(r1_relu[:, :, 0:1, :], 0.0)
        nc.vector.memset(r1_relu[:, :, 25:26, :], 0.0)
        nc.vector.memset(r1_relu[:, :, :, 0:1], 0.0)
        nc.vector.memset(r1_relu[:, :, :, 33:34], 0.0)
        rmaskb = apool.tile([128, XROWS, 34], f16)
        nc.sync.dma_start(rmaskb[:], prm["rmask"][:].partition_broadcast(128))
        xb = apool.tile([128, 2, PIXPAD], f16)
        xbv = [xb[:, j, 0: CROWS * 34].rearrange("p (r c) -> p r c", c=34)
               for j in range(2)]
        for j in range(2):
            nc.vector.memset(xbv[j][:, :, 0:1], 0.0)
            nc.vector.memset(xbv[j][:, :, 33:34], 0.0)
        nc.vector.memset(xb[:, :, CROWS * 34:], 0.0)

        # val window tiles in x-window coordinates (26 rows = XROWS); the
        # 21-row valid band sits at dynamic row offset 5*(1-h). Pad rows are
        # zeroed statically (both possible pad bands); the valid-band writes
        # land after and overwrite any overlap.
        val_pad = apool.tile([128, 2, 832], f32)
        nc.vector.memset(val_pad[:, :, 0:160], 0.0)
        nc.vector.memset(val_pad[:, :, 672:832], 0.0)
        # x val-part pad bands (rows outside the 21-row valid window) are
        # zeroed statically; the valid band is written at a dynamic offset
        for tt in (x_raw, x_relu):
            nc.vector.memset(tt[:, 2:4, 0:5, 1:33], 0.0)
            nc.vector.memset(tt[:, 2:4, 21:26, 1:33], 0.0)

        # ---- query-side conv accumulation: fills the PE while the val
        # AllGather is in flight (j=0,1 of x are batch-query channels) ----
        psR1_cm = tc.tile_pool(name="psR1", bufs=1, space="PSUM")
        psR1 = psR1_cm.__enter__()
        psXB_cm = tc.tile_pool(name="psXB", bufs=1, space="PSUM")
        psXB = psXB_cm.__enter__()
        psW1_cm = tc.tile_pool(name="psW1", bufs=1, space="PSUM")
        psW1 = psW1_cm.__enter__()
        R1G = ((0, 1, 16), (0, 17, 8), (1, 1, 16), (1, 17, 8))
        XBG = ((0, 2, 16), (0, 18, 6), (1, 2, 16), (1, 18, 6))
        # the two short row-groups per producer share one PSUM bank
        # (independent column ranges) so all 8 accumulators fit in 6 banks
        pr_t = {}
        px_t = {}
        r1sm = psR1.tile([128, 512], f32, tag="r1s", name="r1sm")
        xbsm = psXB.tile([128, 384], f32, tag="xbs", name="xbsm")
        # a start=True matmul resets the whole PSUM bank, so shared banks are
        # zeroed once up front and every accumulation into them avoids start
        nc.tensor.matmul(r1sm[:, :], zero128[0:1, :], spin_src[0:1, 0:512],
                         start=True, stop=False, skip_group_check=True)
        nc.tensor.matmul(xbsm[:, :], zero128[0:1, :], spin_src[0:1, 0:384],
                         start=True, stop=False, skip_group_check=True)
        for m in range(2):
            pr_t[(m, 1)] = psR1.tile([128, 512], f32, tag=f"r1b{m}",
                                     name="prb")
            pr_t[(m, 17)] = r1sm[:, m * 256: m * 256 + 256]
            px_t[(m, 2)] = psXB.tile([128, 512], f32, tag=f"xbb{m}",
                                     name="pxb")
            px_t[(m, 18)] = xbsm[:, m * 192: m * 192 + 192]
        for (m, l0, nr) in R1G:
            pr = pr_t[(m, l0)]
            k = 0
            for j in range(2):
                for dy in range(3):
                    for dx in range(3):
                        nc.tensor.matmul(
                            pr[:, : nr * 32],
                            rb1_wT[:, j, dy, dx, m * 128: m * 128 + 128],
                            x_relu[:, j, l0 + dy - 1: l0 + dy - 1 + nr,
                                   dx: dx + 32],
                            start=(k == 0 and nr == 16), stop=False,
                            skip_group_check=True)
                        k += 1
        for (m, l0, nr) in XBG:
            px = px_t[(m, l0)]
            k = 0
            for j in range(2):
                for dy in range(3):
                    for dx in range(3):
                        nc.tensor.matmul(
                            px[:, : nr * 32],
                            rbd_wT[:, j, dy, dx, m * 128: m * 128 + 128],
                            x_raw[:, j, l0 + dy - 1: l0 + dy - 1 + nr,
                                  dx: dx + 32],
                            start=(k == 0 and nr == 16), stop=False,
                            skip_group_check=True)
                        k += 1
        # keep the PE clock ramped through the rest of the AllGather window
        spin_pe(psW1, 48)

        # ---- val: own window (SBUF) + partner window (AG slot), normalize,
        # window into x ----
        vs_p = wk.tile([128, 2, 672], bf, name="vs_p")
        nc.sync.dma_start(
            vs_p[:, :, :],
            arvg2[bass.ds(omh, 1), 0:256, :].rearrange(
                "s (j p) w -> s p j w", j=2))
        sp_row = wk.tile([1, 672], bf, name="sp_row")
        nc.gpsimd.dma_start(sp_row[:, :], arvg2[bass.ds(omh, 1), 256:257, :])
        for j in range(2):
            nc.vector.tensor_add(val_pad[:, j, bass.ds(vstart160, 672)],
                                 v_sb[:, j, bass.ds(myoff, 672)],
                                 vs_p[:, j, :])
        s_row = wk.tile([1, 672], f32, name="s_row")
        nc.vector.tensor_add(s_row[:, :], s_sb[0:1, bass.ds(myoff, 672)],
                             sp_row[:, :])
        inv_row = wk.tile([1, 672], f32, name="inv_row")
        nc.vector.reciprocal(inv_row[:, :], s_row[:, :])
        inv_d = dram.tile([1, 672], f32)
        nc.sync.dma_start(inv_d[:], inv_row[:, :])
        inv_b = wk.tile([128, 21, 32], f32, name="inv_b")
        nc.sync.dma_start(inv_b.rearrange("p r c -> p (r c)"),
                          inv_d.partition_broadcast(128))
        vp_v = val_pad.rearrange("p j (r c) -> p j r c", c=32)
        omh5 = omh * 5
        for j in range(2):
            nc.vector.tensor_mul(x_raw[:, 2 + j, bass.ds(omh5, 21), 1:33],
                                 vp_v[:, j, bass.ds(omh5, 21), :],
                                 inv_b[:, :, :])
            nc.scalar.activation(x_relu[:, 2 + j, :, 1:33],
                                 x_raw[:, 2 + j, :, 1:33], AF.Relu)

        # ---- val-side conv accumulation + activations ----
        for (m, l0, nr) in R1G:
            pr = pr_t[(m, l0)]
            k = 0
            for j in (2, 3):
                for dy in range(3):
                    for dx in range(3):
                        nc.tensor.matmul(
                            pr[:, : nr * 32],
                            rb1_wT[:, j, dy, dx, m * 128: m * 128 + 128],
                            x_relu[:, j, l0 + dy - 1: l0 + dy - 1 + nr,
                                   dx: dx + 32],
                            start=False, stop=(k == 17),
                            skip_group_check=True)
                        k += 1
            nc.scalar.activation(
                r1_relu[:, m, l0: l0 + nr, 1:33], pr[:, : nr * 32],
                AF.Relu, bias=rb1_b[:, m: m + 1])
            nc.vector.tensor_mul(r1_relu[:, m, l0: l0 + nr, 1:33],
                                 r1_relu[:, m, l0: l0 + nr, 1:33],
                                 rmaskb[:, l0: l0 + nr, 1:33])
        for (m, l0, nr) in XBG:
            px = px_t[(m, l0)]
            k = 0
            for j in (2, 3):
                for dy in range(3):
                    for dx in range(3):
                        nc.tensor.matmul(
                            px[:, : nr * 32],
                            rbd_wT[:, j, dy, dx, m * 128: m * 128 + 128],
                            x_raw[:, j, l0 + dy - 1: l0 + dy - 1 + nr,
                                  dx: dx + 32],
                            start=False, stop=False,
                            skip_group_check=True)
                        k += 1
            for j in range(2):
                for dy in range(3):
                    for dx in range(3):
                        nc.tensor.matmul(
                            px[:, : nr * 32],
                            rb2_wT[:, j, dy, dx, m * 128: m * 128 + 128],
                            r1_relu[:, j, l0 + dy - 1: l0 + dy - 1 + nr,
                                    dx: dx + 32],
                            start=False, stop=(k == 35),
                            skip_group_check=True)
                        k += 1
            nc.scalar.activation(
                xbv[m][:, l0 - 2: l0 - 2 + nr, 1:33], px[:, : nr * 32],
                AF.Identity, bias=xb_bias[:, m: m + 1])
        psW1_cm.__exit__(None, None, None)
        psXB_cm.__exit__(None, None, None)
        psR1_cm.__exit__(None, None, None)

        if stage == "dbg":
            nc.sync.dma_start(prm["dbg_xraw"][:], x_raw[:])
            for j in range(2):
                nc.sync.dma_start(prm["dbg_xb"][:, j], xbv[j])

        # ---- CBAM channel gate ----
        stats = wk.tile([128, 2, 2], f32, name="stats")
        for j in range(2):
            nc.vector.tensor_reduce(stats[:, j, 0:1], xbv[j][:, 3:19, 1:33],
                                    AX.XY, ALU.add)
            nc.vector.tensor_reduce(stats[:, j, 1:2], xbv[j][:, 3:19, 1:33],
                                    AX.XY, ALU.max)
        stats_d = dram.tile([256, 2], f32)
        stats_o = dram.tile([2, 256, 2], f32)
        nc.sync.dma_start(stats_d.rearrange("(j p) k -> p j k", j=2),
                          stats[:, :, :])
        # zeroed 38-stride comp rows in SBUF (borders give the conv halo);
        # the spatial-conv im2col then gathers straight from SBUF
        comp_sp = wk.tile([1, CROWS * 38], f16, name="comp_sp")
        mean_sp = wk.tile([1, CROWS * 38], f16, name="mean_sp")
        nc.vector.memset(comp_sp[:], 0.0)
        nc.vector.memset(mean_sp[:], 0.0)
        nc.gpsimd.collective_compute(
            "AllGather", ALU.bypass, replica_groups=PAIRS,
            ins=[stats_d[:].opt()], outs=[stats_o[:].opt()])
        slb = wk.tile([128, 2, 2, 2], f32, name="slb")  # [p, slot, j, stat]
        nc.sync.dma_start(slb[:, :, :, :],
                          stats_o.rearrange("s (j p) k -> p s j k", j=2))
        gate_in = wk.tile([128, 2, 2], f32, name="gate_in")
        tsum = wk.tile([128, 2, 1], f32, name="tsum")
        nc.vector.tensor_add(tsum[:, :, :], slb[:, 0, :, 0:1],
                             slb[:, 1, :, 0:1])
        nc.scalar.mul(gate_in[:, :, 0:1], tsum[:, :, :], 1.0 / 1024.0)
        nc.vector.tensor_max(gate_in[:, :, 1:2], slb[:, 0, :, 1:2],
                             slb[:, 1, :, 1:2])

        gate = wk.tile([128, 2, 1], f32, name="gate")
        ones1 = wk.tile([1, 128], f16, name="ones1")
        nc.vector.memset(ones1[:], 1.0)
        with tc.tile_pool(name="psG", bufs=1, space="PSUM") as psG:
            ph1 = psG.tile([16, 2], f32, name="ph1")
            for j in range(2):
                nc.tensor.matmul(ph1[:, :], mlp_w1T[:, j, :], gate_in[:, j, :],
                                 start=(j == 0), stop=(j == 1))
            h1 = wk.tile([16, 2], f16, name="h1")
            nc.scalar.activation(h1[:, :], ph1[:, :], AF.Relu,
                                 bias=mlp_b1[:, 0:1])
            # per-partition gate (sigmoid via the already-loaded exp table)
            for j in range(2):
                ph2 = psG.tile([128, 2], f32, tag="ph2", name="ph2")
                nc.tensor.matmul(ph2[:, :], mlp_w2T[:, j * 128: j * 128 + 128],
                                 h1[:, :], start=True, stop=True)
                h2 = wk.tile([128, 2], f32, tag="h2", name="h2")
                nc.vector.tensor_copy(h2[:, :], ph2[:, :])
                t2 = wk.tile([128, 1], f32, tag="t2", name="t2")
                nc.vector.tensor_add(t2[:, :], h2[:, 0:1], h2[:, 1:2])
                ev = wk.tile([128, 1], f32, tag="ev", name="ev")
                nc.scalar.activation(ev[:, :], t2[:, :], AF.Exp, scale=-1.0,
                                     bias=mlp_b2x2n[:, j: j + 1])
                e1 = wk.tile([128, 1], f32, tag="e1", name="e1")
                nc.scalar.activation(e1[:, :], ev[:, :], AF.Identity,
                                     bias=1.0)
                nc.vector.reciprocal(gate[:, j, :], e1[:, :])

        if stage == "dbg":
            nc.sync.dma_start(prm["dbg_gate"][:], gate[:])

        gate_sc = wk.tile([128, 2, 1], f16, name="gate_sc")
        nc.scalar.mul(gate_sc[:, :, :], gate[:, :, :], 1.0 / 256.0)

        # channel max of xb*gate via a cross-partition gpsimd reduce -- the
        # result lands directly in pixel-major layout, skipping the PE
        # transposes and one DRAM staging hop
        xcj = wk.tile([128, 2, 768], f16, name="xcj")
        for j in range(2):
            nc.vector.tensor_scalar_mul(xcj[:, j, :], xb[:, j, :],
                                        gate[:, j, 0:1])
        cmx = wk.tile([128, 768], f32, name="cmx")
        cmx2 = wk.tile([128, 768], f32, name="cmx2")
        nc.gpsimd.partition_all_reduce(cmx[:, :], xcj[:, 0, :], 128,
                                       bass_isa.ReduceOp.max)
        nc.gpsimd.partition_all_reduce(cmx2[:, :], xcj[:, 1, :], 128,
                                       bass_isa.ReduceOp.max)
        comp_row = wk.tile([1, 768], f16, name="comp_row")
        nc.vector.tensor_max(comp_row[0:1, :], cmx[0:1, :], cmx2[0:1, :])
        nc.vector.tensor_mul(
            comp_sp[0:1, :].rearrange("o (r c) -> o r c", c=38)[:, :, 2:36],
            comp_row[0:1, 0:748].rearrange("o (r c) -> o r c", c=34),
            mask_mean[0:1, 0:748].rearrange("o (r c) -> o r c", c=34))

        # channel mean of xb*gate via gate-weighted ones-matmul; the mean
        # half of comp then flows through its DRAM hops on the Pool queue
        # while the max half (slower DVE path) catches up on the SP queue.
        il = wk.tile([14, 16, 38], f16, name="il")
        mean_sb = wk.tile([1, 748], f16, name="mean_sb")
        psW2_cm = tc.tile_pool(name="psW2", bufs=1, space="PSUM")
        psW2 = psW2_cm.__enter__()
        with tc.tile_pool(name="psM", bufs=1, space="PSUM") as psM:
            pm1 = psM.tile([1, 748], f32, name="pm1")
            for j in range(2):
                for (o0, nn) in ((0, 512), (512, 236)):
                    nc.tensor.matmul(pm1[0:1, o0: o0 + nn],
                                     gate_sc[:, j, :],
                                     xb[:, j, o0: o0 + nn],
                                     start=(j == 0), stop=(j == 1))
            nc.scalar.copy(mean_sb[:, :], pm1[:, :])
        nc.vector.tensor_mul(
            mean_sp[0:1, :].rearrange("o (r c) -> o r c", c=38)[:, :, 2:36],
            mean_sb[0:1, :].rearrange("o (r c) -> o r c", c=34),
            mask_mean[0:1, 0:748].rearrange("o (r c) -> o r c", c=34))
        # keep the PE clock ramped until the spatial-conv operands land
        spin_pe(psW2, 40)

        # tracked guard-reads on the issuing queues order the untracked
        # (manual-AP) im2col gathers after the DVE mask-mul writes
        guard_d = dram.tile([2, CROWS * 38], f16)
        nc.gpsimd.dma_start(guard_d[1:2, :], mean_sp[0:1, :])
        nc.gpsimd.dma_start(
            il[7:14, :, :],
            bass.AP(mean_sp.tensor, 0, [[1, 1], [38, 7], [38, 16], [1, 38]]))
        nc.sync.dma_start(guard_d[0:1, :], comp_sp[0:1, :])
        nc.sync.dma_start(
            il[0:7, :, :],
            bass.AP(comp_sp.tensor, 0, [[1, 1], [38, 7], [38, 16], [1, 38]]))
        sigb = wk.tile([128, 16, 32], f16, name="sigb")
        with tc.tile_pool(name="psS", bufs=1, space="PSUM") as psS:
            pss = psS.tile([1, 512], f32, name="pss")
            for dx in range(7):
                nc.tensor.matmul(pss[:, :], spw_r[:, dx: dx + 1],
                                 il[:, :, dx: dx + 32],
                                 start=(dx == 0), stop=(dx == 6))
            # sigmoid via 1/(1+exp(-z)): broadcast exp(-z) down the
            # partitions first, then finish wide on the DVE straight into
            # the f16 sigb tile (no narrow row round-trips)
            se = wk.tile([1, 512], f16, name="se")
            nc.scalar.activation(se[:, :], pss[:, :], AF.Exp,
                                 scale=bn_nsb[0:1, 0:1], bias=bn_nsb[0:1, 1:2])
            sigb_ps = psS.tile([128, 512], f32, tag="sigbps", name="sigb_ps")
            nc.tensor.matmul(sigb_ps[:, :], ones1[0:1, :], se[0:1, :],
                             start=True, stop=True)
            t1 = wk.tile([128, 512], f32, name="t1")
            nc.vector.tensor_scalar_add(t1[:, :], sigb_ps[:, :], 1.0)
            with nc.allow_low_precision(reason="sigmoid output in [0,1]"):
                nc.vector.reciprocal(sigb.rearrange("p r c -> p (r c)"),
                                     t1[:, :])
        psW2_cm.__exit__(None, None, None)

        if stage == "dbg":
            nc.sync.dma_start(prm["dbg_sig"][:], se[:])
            nc.sync.dma_start(prm["dbg_il"][:], il[:])
            nc.sync.dma_start(prm["dbg_cmp"][:], comp_row[:])

        # fused = xb_own * (1 + gate * sigb): one fused tensor-scalar plus
        # one multiply per half, all on the DVE in 2x mode
        fused = apool.tile([128, 2, 16, 32], f16)
        for j in range(2):
            tm = wk.tile([128, 16, 32], f16, tag="tm", name="tm")
            nc.vector.tensor_scalar(tm[:, :, :], sigb[:, :, :],
                                    gate[:, j, 0:1], 1.0,
                                    ALU.mult, ALU.add)
            nc.vector.tensor_mul(fused[:, j, :, :], xbv[j][:, 3:19, 1:33],
                                 tm[:, :, :])

        if stage == "dbg":
            nc.sync.dma_start(prm["dbg_fused"][:], fused[:])

        # bottleneck conv: accumulate the fused-input chunks now so the PE
        # works during the pools AllGather; priors chunks finish the group
        # after the collective.
        fbv = fused.rearrange("p j r c -> p j (r c)")
        psO_cm = tc.tile_pool(name="psO", bufs=2, space="PSUM")
        psO = psO_cm.__enter__()
        po_t = []
        for m in range(2):
            po = psO.tile([128, 512], f32, tag="po", name="po")
            po_t.append(po)
            for jj in range(2):
                nc.tensor.matmul(po[:, :],
                                 bott_wT[:, 2 + jj, m * 128: m * 128 + 128],
                                 fbv[:, jj, :],
                                 start=(jj == 0), stop=False,
                                 skip_group_check=True)

        # ---- PSP pools (raw block sums over own rows) ----
        pools = wk.tile([128, 2, 43], f32, name="pools")
        for j in range(2):
            f8 = fused[:, j].rearrange("p (rb ri) (cb ci) -> p rb cb ri ci",
                                       ri=4, ci=4)
            p8v = pools[:, j, 11:43].rearrange("p (rb cb) -> p rb cb", cb=8)
            nc.vector.tensor_reduce(p8v, f8, AX.XY, ALU.add)
            p8i = pools[:, j, 11:43].rearrange(
                "p (rb ri cb ci) -> p rb cb ri ci", rb=2, ri=2, cb=4, ci=2)
            p4v = pools[:, j, 3:11].rearrange("p (rb cb) -> p rb cb", cb=4)
            nc.vector.tensor_reduce(p4v, p8i, AX.XY, ALU.add)
        p4i = pools[:, :, 3:11].rearrange(
            "p j (rb cb ci) -> p j cb rb ci", rb=2, cb=2, ci=2)
        nc.vector.tensor_reduce(
            pools[:, :, 1:3].rearrange("p j (a k) -> p j a k", a=2, k=1),
            p4i, AX.XY, ALU.add)
        nc.vector.tensor_reduce(pools[:, :, 0:1], pools[:, :, 1:3], AX.X,
                                ALU.add)

        pools16 = wk.tile([128, 2, 43], f16, name="pools16")
        nc.vector.tensor_copy(pools16[:, :, :], pools[:, :, :])
        pools_d = dram.tile([2, 128, 43], f16)
        pools_o = dram.tile([2, 2, 128, 43], f16)
        nc.sync.dma_start(pools_d.rearrange("j p k -> p j k"),
                          pools16[:, :, :])
        nc.gpsimd.collective_compute(
            "AllGather", ALU.bypass, replica_groups=PAIRS,
            ins=[pools_d[:].opt()], outs=[pools_o[:].opt()])
        # 1x1 convs on pools in TRANSPOSED layout (out partition = pool
        # block, free = psp channel). The OWN half comes straight from
        # pools16 in SBUF and runs DURING the AllGather; only the partner
        # half waits for the collective. Wup's slot dim is indexed
        # dynamically (own = rank-in-pair, partner = the other).
        SI = {1: 0, 2: 1, 4: 2, 8: 3}
        pdT_a8 = [wk.tile([32, 64], f16, tag=f"pdT_a8{s}", name=f"pdT_a8{s}")
                  for s in range(2)]
        pdT_s1 = wk.tile([1, 64], f16, name="pdT_s1")
        pdT_b = [wk.tile([8, 64], f16, tag=f"pdT_b{s}", name=f"pdT_b{s}")
                 for s in range(2)]
        pdT_c = [wk.tile([2, 64], f16, tag=f"pdT_c{s}", name=f"pdT_c{s}")
                 for s in range(2)]
        pri = [wk.tile([128, 512], f16, tag=f"pri{i}", name=f"pri{i}")
               for i in range(2)]
        ones_f = wk.tile([128, 512], f16, name="ones_f")
        nc.vector.memset(ones_f[:], 1.0)
        psP_cm = tc.tile_pool(name="psP", bufs=1, space="PSUM")
        psP = psP_cm.__enter__()
        psR_cm = tc.tile_pool(name="psR", bufs=1, space="PSUM")
        psR = psR_cm.__enter__()
        pdm_ps = psP.tile([65, 64], f32, name="pdm_ps")
        pd4_ps = psP.tile([40, 64], f32, name="pd4_ps")
        pd2_ps = psP.tile([34, 64], f32, name="pd2_ps")
        pp0 = psR.tile([128, 512], f32, tag="pp0", name="pp0")
        pp1 = psR.tile([128, 512], f32, tag="pp1", name="pp1")

        def pd_matmuls(src, sl):
            # src[j] -> [128, 43] pool partials for this half (j = ch chunk)
            for j in range(2):
                nc.tensor.matmul(pdm_ps[32 * sl: 32 * sl + 32, :],
                                 src(j, 11, 43), psp_wT[:, j, SI[8], :],
                                 start=(j == 0), stop=(j == 1),
                                 skip_group_check=True)
            for j in range(2):
                nc.tensor.matmul(pdm_ps[64:65, :],
                                 src(j, 0, 1), psp_wT[:, j, SI[1], :],
                                 start=(sl == 0 and j == 0),
                                 stop=(sl == 1 and j == 1),
                                 skip_group_check=True)
            for j in range(2):
                nc.tensor.matmul(pd4_ps[32 * sl: 32 * sl + 8, :],
                                 src(j, 3, 11), psp_wT[:, j, SI[4], :],
                                 start=(j == 0), stop=(j == 1),
                                 skip_group_check=True)
            for j in range(2):
                nc.tensor.matmul(pd2_ps[32 * sl: 32 * sl + 2, :],
                                 src(j, 1, 3), psp_wT[:, j, SI[2], :],
                                 start=(j == 0), stop=(j == 1),
                                 skip_group_check=True)

        def pd_copies(sl):
            nc.scalar.copy(pdT_a8[sl][:, :], pdm_ps[32 * sl: 32 * sl + 32, :])
            nc.vector.tensor_copy(pdT_b[sl][:, :],
                                  pd4_ps[32 * sl: 32 * sl + 8, :])
            nc.vector.tensor_copy(pdT_c[sl][:, :],
                                  pd2_ps[32 * sl: 32 * sl + 2, :])

        def upsample(sl, slot_idx):
            nc.tensor.matmul(pp0[64:128, :], pdT_c[sl][:, :],
                             Wup[0:2, bass.ds(slot_idx, 1), 0, :],
                             start=(sl == 0), stop=(sl == 1),
                             tile_position=(0, 64), skip_group_check=True)
            nc.tensor.matmul(pp1[0:64, :], pdT_b[sl][:, :],
                             Wup[0:8, bass.ds(slot_idx, 1), 1, :],
                             start=(sl == 0), stop=(sl == 1),
                             tile_position=(0, 0), skip_group_check=True)
            nc.tensor.matmul(pp1[64:128, :], pdT_a8[sl][:, :],
                             Wup[0:32, bass.ds(slot_idx, 1), 2, :],
                             start=(sl == 0), stop=(sl == 1),
                             tile_position=(0, 64), skip_group_check=True)

        psW3_cm = tc.tile_pool(name="psW3", bufs=1, space="PSUM")
        psW3 = psW3_cm.__enter__()
        # own half: runs while the collective is in flight
        pd_matmuls(lambda j, a, b: pools16[:, j, a:b], 0)
        pd_copies(0)
        upsample(0, pid2)
        # keep the PE clock ramped through the pools AllGather
        spin_pe(psW3, 88)
        # partner half: after the collective
        slp_p = wk.tile([128, 2, 43], f16, name="slp_p")
        nc.sync.dma_start(
            slp_p[:, :, :],
            pools_o[bass.ds(omh, 1)].rearrange("s j p k -> s p j k"))
        pd_matmuls(lambda j, a, b: slp_p[:, j, a:b], 1)
        pd_copies(1)
        nc.scalar.copy(pdT_s1[:, :], pdm_ps[64:65, :])
        upsample(1, omh)
        nc.tensor.matmul(pp0[0:64, :], pdT_s1[:, :], ones_f[0:1, :],
                         start=True, stop=True, skip_group_check=True)
        nc.scalar.copy(pri[0][:, :], pp0[:, :])
        nc.scalar.copy(pri[1][:, :], pp1[:, :])
        psW3_cm.__exit__(None, None, None)
        psR_cm.__exit__(None, None, None)
        psP_cm.__exit__(None, None, None)

        if stage == "dbg":
            nc.sync.dma_start(prm["dbg_pd"][0:32], pdT_a8[0][:])
            nc.sync.dma_start(prm["dbg_pd"][32:64], pdT_a8[1][:])
            nc.sync.dma_start(prm["dbg_pd"][64:65], pdT_s1[:])

        if stage == "dbg":
            nc.sync.dma_start(prm["dbg_pri0"][:], pri[0][:])
            nc.sync.dma_start(prm["dbg_pri1"][:], pri[1][:])

        out_sb = wk.tile([128, 2, 512], f32, name="out_sb")
        for m in range(2):
            po = po_t[m]
            for k in range(2):
                nc.tensor.matmul(po[:, :],
                                 bott_wT[:, k, m * 128: m * 128 + 128],
                                 pri[k][:, :],
                                 start=False, stop=(k == 1),
                                 skip_group_check=True)
            nc.scalar.activation(out_sb[:, m, :], po[:, :], AF.Relu,
                                 bias=bott_b[:, m: m + 1])
            # per-half output DMA overlaps the other half's epilogue
            nc.sync.dma_start(
                out_prm[:, m: m + 1, :, :],
                out_sb[:, m: m + 1, :].rearrange("p j (r c) -> p j r c",
                                                 c=32))
        psO_cm.__exit__(None, None, None)


# ---------------------------------------------------------------------------
# Runner
# ---------------------------------------------------------------------------

_CACHE = {}


def _get_nc(stage="full"):
    if stage not in _CACHE:
        _CACHE[stage] = build(stage)
    return _CACHE[stage]


def run_cores(inputs, stage="full"):
    nc = _get_nc(stage)
    in_maps = [prep_core_inputs(inputs, c) for c in range(N_CORES)]
    res = run_bass_kernel_spmd(nc, in_maps, list(range(N_CORES)))
    return res.results


def kernel(**inputs):
    results = run_cores(inputs, "full")
    out = np.zeros((B, 1, COUT, H, W), np.float32)
    for c in range(N_CORES):
        b, h = c // 2, c % 2
        o = results[c]["out"]                    # [128, 2, 16, 32]
        out[b, 0, :, 16 * h: 16 * h + 16, :] = (
            o.transpose(1, 0, 2, 3).reshape(COUT, 16, 32))
    return out

